# revision 25
# baseline (speedup 1.0000x reference)
"""Trainium2 Bass kernel for a dense pre-LN transformer block (v2).

Block: y = x + proj(causal_mha(LN1(x))) ; out = y + FFN(LN2(y))
Shapes (hardcoded): x [4, 2048, 1024], H=16 heads, HD=64, FF=2048, fp32 I/O.

Sharding (8 cores, no collectives): core c handles batch b=c//2 and a
balanced half of the queries (role r=c%2; A: rows [0,512)+[1536,2048),
B: rows [512,1536)).  The key/value sequence is permuted on the host per
core (own rows first) so one SPMD program serves both roles; causality is
enforced by host-built triangular masks on the diagonal chunks plus a
per-core exp-bias table (-100 => exp ~ 0) for role-dependent chunks.

v2 changes vs v1:
- LN gamma/beta folded into host-prepped weights/biases (no on-chip g/b).
- Transposes via DMA xbar (dma_start transpose=True), not the PE array.
- rsqrt for LN via DVE quake-rsqrt (no activation-table switches; the
  scalar engine only ever uses the exp/identity/relu table set).
- Flat [128,1024] masks (no broadcast APs on the DVE mask multiply).
- Softmax 1/l fused into the z evacuation; no SBUF->SBUF DMA dances.
- V streamed per (hp,j) in chunk batches (24 DMA loads, not 192).
- Attention loops j-outer so proj/LN2/FFN1 of the first query tile
  overlap the scalar-bound attention of the second tile.
"""

import numpy as np
import ml_dtypes

import concourse.bass as bass
import concourse.bacc as bacc
import concourse.tile as tile
import concourse.mybir as mybir
from concourse.bass import ts
from concourse.bass_utils import run_bass_kernel_spmd

BF16 = mybir.dt.bfloat16
F32 = mybir.dt.float32
U32 = mybir.dt.uint32
AF = mybir.ActivationFunctionType
ALU = mybir.AluOpType

S = 2048          # sequence length
E = 1024          # embedding dim
H = 16            # heads
HD = 64           # head dim
FF = 2048         # ffn hidden
P = 128           # partitions
NQ = 1024         # queries owned per core
EPS = 1e-5
NEG = -100.0      # exp bias for masked-out chunks (exp(-100) ~ 0)

EC = E // P       # 8 e-chunks
FC = FF // P      # 16 f-chunks
NCH = S // P      # 16 key chunks
HP = H // 2       # 8 head pairs
QC = NQ // P      # 8 own query row-tiles

# chunk schedule (in permuted key coordinates), identical on every core:
# q-tile 0 (own positions [0,512)):   key chunks 0-3 (diag) + 8-11 (role-dep)
# q-tile 1 (own positions [512,1024)): key chunks 0-15 (4-7 diag, 12-15 role-dep)
SCHED = [[0, 1, 2, 3, 8, 9, 10, 11], list(range(16))]
DIAG = [set(range(0, 4)), set(range(4, 8))]

_CACHE = {}
_QCONST = [None]
_EPS_T = [None]


def _build_program():
    nc = bacc.Bacc("TRN2", target_bir_lowering=False, debug=False)

    # ---- per-core dram inputs -------------------------------------------
    xp_d = nc.dram_tensor("xp", [S, E], F32, kind="ExternalInput")
    wq_d = nc.dram_tensor("wq2", [HP, EC, P, P], BF16, kind="ExternalInput")
    wk_d = nc.dram_tensor("wk2", [HP, EC, P, P], BF16, kind="ExternalInput")
    wv_d = nc.dram_tensor("wv", [EC, P, E], BF16, kind="ExternalInput")
    wp_d = nc.dram_tensor("wp", [EC, P, E], BF16, kind="ExternalInput")
    w1_d = nc.dram_tensor("w1", [FC, EC, P, P], BF16, kind="ExternalInput")
    w2_d = nc.dram_tensor("w2", [FC, P, E], BF16, kind="ExternalInput")
    b1_d = nc.dram_tensor("b1t", [P, FC], F32, kind="ExternalInput")
    bp_d = nc.dram_tensor("bproj", [E], F32, kind="ExternalInput")
    b2_d = nc.dram_tensor("b2", [E], F32, kind="ExternalInput")
    kb_d = nc.dram_tensor("kbt", [P, HP], F32, kind="ExternalInput")
    qb_d = nc.dram_tensor("qbt", [P, HP], F32, kind="ExternalInput")
    mb_d = nc.dram_tensor("mb", [24], F32, kind="ExternalInput")
    msk_d = nc.dram_tensor("msk", [4, P, 1024], BF16, kind="ExternalInput")
    out_d = nc.dram_tensor("out", [NQ, E], F32, kind="ExternalOutput")

    QSCALE = float(HD) ** -0.5

    def rstd_quake(nc, small, var_ap, tag):
        """1/sqrt(var+eps) on the DVE only ([P,1] tiles, quake + 1 Newton).
        Used mid-attention (LN2) to avoid scalar activation-table switches."""
        a = small.tile([P, 1], F32, tag=tag + "_a")
        nc.vector.tensor_scalar(out=a, in0=var_ap, scalar1=EPS, scalar2=None,
                                op0=ALU.add)
        s1 = small.tile([P, 1], U32, tag=tag + "_s")
        nc.vector.tensor_scalar(out=s1, in0=a.bitcast(U32), scalar1=1,
                                scalar2=None, op0=ALU.logical_shift_right)
        y0b = small.tile([P, 1], U32, tag=tag + "_y0b")
        nc.vector.tensor_tensor(out=y0b, in0=_QCONST[0], in1=s1,
                                op=ALU.subtract)
        y = y0b.bitcast(F32)
        for it in range(2):
            t2 = small.tile([P, 1], F32, tag=tag + f"_u{it}")
            nc.vector.tensor_tensor(out=t2, in0=y, in1=y, op=ALU.mult)
            t3 = small.tile([P, 1], F32, tag=tag + f"_v{it}")
            nc.vector.scalar_tensor_tensor(out=t3, in0=t2, scalar=-0.5,
                                           in1=a, op0=ALU.mult, op1=ALU.mult)
            u = small.tile([P, 1], F32, tag=tag + f"_x{it}")
            nc.vector.tensor_scalar(out=u, in0=t3, scalar1=1.5, scalar2=None,
                                    op0=ALU.add)
            yn = small.tile([P, 1], F32, tag=tag + f"_w{it}")
            nc.vector.tensor_tensor(out=yn, in0=y, in1=u, op=ALU.mult)
            y = yn
        return y

    def layernorm_T(nc, small, acts, x_ap, dstT, sc, tag, quake):
        """LN of one [128,E] fp32 row-tile -> bf16 transpose into
        dstT[:, :, sc*128:(sc+1)*128] via the DMA xbar.  The normalize
        apply runs on the scalar engine (Identity is in every table set);
        rstd comes from scalar Sqrt when quake=False (only safe before the
        exp table is loaded) or the DVE quake chain when True."""
        stats = small.tile([P, 2, 6], F32, tag=tag + "_bn")
        for g in range(2):
            nc.vector.bn_stats(out=stats[:, g, :], in_=x_ap[:, g * 512:(g + 1) * 512])
        mv = small.tile([P, 2], F32, tag=tag + "_mv")
        nc.vector.bn_aggr(out=mv, in_=stats)
        if quake:
            rstd = rstd_quake(nc, small, mv[:, 1:2], tag)
        else:
            std = small.tile([P, 1], F32, tag=tag + "_std")
            nc.scalar.activation(out=std, in_=mv[:, 1:2], func=AF.Sqrt,
                                 bias=_EPS_T[0], scale=1.0)
            rstd = small.tile([P, 1], F32, tag=tag + "_rstd")
            nc.vector.reciprocal(out=rstd, in_=std)
        nm = small.tile([P, 1], F32, tag=tag + "_nm")
        nc.vector.scalar_tensor_tensor(out=nm, in0=mv[:, 0:1], scalar=-1.0,
                                       in1=rstd, op0=ALU.mult, op1=ALU.mult)
        tmp = acts.tile([P, E], BF16, tag=tag + "_tmp")
        nc.scalar.activation(out=tmp, in_=x_ap, func=AF.Identity,
                             bias=nm, scale=rstd)
        nc.sync.dma_start(out=dstT[:, :, ts(sc, P)], in_=tmp, transpose=True)

    with tile.TileContext(nc) as tc:
        import contextlib
        stk = contextlib.ExitStack()
        with stk:
            const = stk.enter_context(tc.tile_pool(name="const", bufs=1))
            small = stk.enter_context(tc.tile_pool(name="small", bufs=4))
            dram = stk.enter_context(tc.tile_pool(name="dram", bufs=1, space="DRAM"))

            qconst = const.tile([P, 1], U32)
            nc.vector.memset(qconst, 0x5F3759DF)
            eps_t = const.tile([P, 1], F32)
            nc.vector.memset(eps_t, EPS)
            global _QCONST, _EPS_T
            _QCONST = [qconst]
            _EPS_T = [eps_t]

            mb_sb = const.tile([P, 24], F32)
            nc.gpsimd.dma_start(out=mb_sb, in_=mb_d[None, :].to_broadcast((P, 24)))
            b1_sb = const.tile([P, FC], F32)
            nc.gpsimd.dma_start(out=b1_sb, in_=b1_d[:, :])
            bp_sb = const.tile([P, E], F32)
            nc.gpsimd.dma_start(out=bp_sb, in_=bp_d[None, :].to_broadcast((P, E)))
            b2_sb = const.tile([P, E], F32)
            nc.gpsimd.dma_start(out=b2_sb, in_=b2_d[None, :].to_broadcast((P, E)))
            kb_sb = const.tile([P, HP], F32)
            nc.gpsimd.dma_start(out=kb_sb, in_=kb_d[:, :])
            qb_sb = const.tile([P, HP], F32)
            nc.gpsimd.dma_start(out=qb_sb, in_=qb_d[:, :])
            msk = const.tile([P, 4, 1024], BF16)
            nc.sync.dma_start(out=msk, in_=msk_d.rearrange("k p q -> p k q"))

            out1_dram = dram.tile([NQ, E], F32)
            # V streamed through DRAM: [ch, t, h, 64] values + ones col 64
            V_dram = dram.tile([NCH, P, H, HD + 1], BF16)

            # persistent across attention + tail (zT until tail proj, ln2T
            # until tail FFN1)
            big = stk.enter_context(tc.tile_pool(name="big", bufs=1))
            zT = big.tile([P, EC, NQ], BF16)
            ln2T = big.tile([P, EC, NQ], BF16)

            xstream = stk.enter_context(tc.tile_pool(name="xstream", bufs=2))
            acts = stk.enter_context(tc.tile_pool(name="acts", bufs=2))

            # ---------------- phase A/B: LN1 + V (interleaved) -----------
            # lnT + wvt live on the RIGHT side: freed mid-program while the
            # left-side attention pools are still open (strict LIFO per side).
            lnT_stk = contextlib.ExitStack()
            lnT_pool = lnT_stk.enter_context(
                tc.tile_pool(name="lnT_pool", bufs=1, side="right"))
            lnT = lnT_pool.tile([P, EC, S], BF16)
            wvt = lnT_pool.tile([P, EC, E], BF16)
            nc.gpsimd.dma_start(out=wvt, in_=wv_d.rearrange("ec e n -> e ec n"))

            attn_stk = contextlib.ExitStack()
            kq_pool = attn_stk.enter_context(tc.tile_pool(name="kq_pool", bufs=1))
            KT = [kq_pool.tile([P, S], BF16, name=f"KT{i}") for i in range(HP)]
            QT = [kq_pool.tile([P, NQ], BF16, name=f"QT{i}") for i in range(HP)]
            wstream = attn_stk.enter_context(tc.tile_pool(name="wstream", bufs=2))

            # deeper psum ring for the LN1/V/KQ phase (closed before the
            # attention psum pools open)
            pre_stk = contextlib.ExitStack()
            pre_psum = pre_stk.enter_context(
                tc.tile_pool(name="pre_psum", bufs=6, space="PSUM"))

            # All 16 LN1 chains first: their DVE/scalar/DMA hops pipeline
            # across tiles.  Emitting V right after its own LN tile would
            # park V's psum-evac copy at the DVE queue head, blocking the
            # next tile's bn_stats on this tile's full LN->transpose->matmul
            # latency chain.
            for sc in range(S // P):
                xt = xstream.tile([P, E], F32, tag="x")
                nc.scalar.dma_start(out=xt, in_=xp_d[ts(sc, P), :])
                layernorm_T(nc, small, acts, xt, lnT, sc, "ln1", quake=False)

            def v_chunk(ch):
                for half in range(2):
                    pv = pre_psum.tile([P, 512], F32, tag="mm")
                    for ec in range(EC):
                        nc.tensor.matmul(pv, lnT[:, ec, ts(ch, P)],
                                         wvt[:, ec, ts(half, 512)],
                                         start=(ec == 0), stop=(ec == EC - 1))
                    vsb = acts.tile([P, 8, HD + 1], BF16, tag="vsb")
                    nc.vector.memset(vsb[:, :, HD:HD + 1], 1.0)
                    nc.vector.tensor_copy(
                        out=vsb[:, :, 0:HD],
                        in_=pv.rearrange("p (h d) -> p h d", d=HD))
                    nc.sync.dma_start(
                        out=V_dram[ch, :, 8 * half:8 * (half + 1), :], in_=vsb)

            for ch in range(NCH):
                v_chunk(ch)

            # ---- K/Q for one head pair (6 psum chains of 8 matmuls) -----
            def load_w(hp):
                wkt = wstream.tile([P, EC, P], BF16, tag="wk")
                nc.gpsimd.dma_start(out=wkt, in_=wk_d[hp].rearrange("ec e d -> e ec d"))
                wqt = wstream.tile([P, EC, P], BF16, tag="wq")
                nc.gpsimd.dma_start(out=wqt, in_=wq_d[hp].rearrange("ec e d -> e ec d"))
                return wkt, wqt

            def emit_kq_group(hp, wkt, wqt, kind, seg, psum):
                pk = psum.tile([P, 512], F32, tag="mm", name="pk")
                wt = wkt if kind == "k" else wqt
                for ec in range(EC):
                    nc.tensor.matmul(pk, wt[:, ec], lnT[:, ec, ts(seg, 512)],
                                     start=(ec == 0), stop=(ec == EC - 1))
                if kind == "k":
                    nc.vector.scalar_tensor_tensor(
                        out=KT[hp][:, ts(seg, 512)], in0=pk, scalar=1.0,
                        in1=kb_sb[:, hp:hp + 1].to_broadcast((P, 512)),
                        op0=ALU.mult, op1=ALU.add)
                else:
                    nc.vector.scalar_tensor_tensor(
                        out=QT[hp][:, ts(seg, 512)], in0=pk, scalar=QSCALE,
                        in1=qb_sb[:, hp:hp + 1].to_broadcast((P, 512)),
                        op0=ALU.mult, op1=ALU.add)

            KQ_GROUPS = [("k", 0), ("k", 1), ("k", 2), ("k", 3),
                         ("q", 0), ("q", 1)]
            for hp in (0, 1):
                w = load_w(hp)
                for kind, seg in KQ_GROUPS:
                    emit_kq_group(hp, w[0], w[1], kind, seg, pre_psum)
            pre_stk.close()
            mm_psum = attn_stk.enter_context(
                tc.tile_pool(name="mm_psum", bufs=2, space="PSUM"))

            # ---------------- attention --------------------------------
            st_psum = attn_stk.enter_context(
                tc.tile_pool(name="st_psum", bufs=2, space="PSUM"))
            z_psum = attn_stk.enter_context(
                tc.tile_pool(name="z_psum", bufs=2, space="PSUM"))
            p_pool = attn_stk.enter_context(tc.tile_pool(name="p_pool", bufs=4))
            v_pool = attn_stk.enter_context(tc.tile_pool(name="v_pool", bufs=4))
            l_pool = attn_stk.enter_context(tc.tile_pool(name="l_pool", bufs=3))

            # FFN-phase pools (opened before j=1 so proj/FFN1 can interleave)
            ffn_stk = contextlib.ExitStack()

            def attn_tile(j, hp, vt, extra_work):
                """One (q-tile, head-pair): scores -> exp -> z, with
                extra_work[(slot_idx)] emitted into the exp gaps."""
                sched = SCHED[j]
                zp = [z_psum.tile([P, 512], F32, tag="z", name=f"zp{h}")
                      for h in range(2)]
                pend = None

                def flush_z(pend):
                    pt_, vi_, first_, last_ = pend
                    vtile = vt[vi_ // 8]
                    for h in range(2):
                        nc.tensor.matmul(zp[h][0:HD + 1], vtile[:, vi_ % 8, h],
                                         pt_[:, h], start=first_, stop=last_)

                for ci, ch in enumerate(sched):
                    st = st_psum.tile([P, 1024], F32, tag="st")
                    nc.tensor.matmul(
                        st[:, 0:512], KT[hp][0:HD, ts(ch, P)],
                        QT[hp][0:HD, ts(j, 512)],
                        start=True, stop=True, tile_position=(0, 0))
                    nc.tensor.matmul(
                        st[:, 512:1024], KT[hp][HD:P, ts(ch, P)],
                        QT[hp][HD:P, ts(j, 512)],
                        start=True, stop=True, tile_position=(HD, 0))
                    slot = (8 if j else 0) + ci
                    pt = p_pool.tile([P, 2, 512], BF16, tag="p")
                    nc.scalar.activation(
                        out=pt.rearrange("p a b -> p (a b)"), in_=st,
                        func=AF.Exp, bias=mb_sb[:, slot:slot + 1], scale=1.0)
                    if ch in DIAG[j]:
                        k = ch - 4 * j
                        nc.vector.tensor_tensor(
                            out=pt.rearrange("p a b -> p (a b)"),
                            in0=pt.rearrange("p a b -> p (a b)"),
                            in1=msk[:, k, :], op=ALU.mult)
                    if ci in extra_work:
                        extra_work[ci]()
                    if pend is not None:
                        flush_z(pend)
                    pend = (pt, ci, ci == 0, ci == len(sched) - 1)
                flush_z(pend)

                # evacuate z with fused 1/l.  DVE reciprocal is an 8-slice
                # iterative divide (~7 cyc/elem along the free dim), so run
                # it on a [128,4] reshape of the denominator row (DMA
                # round-trip keeps the latency off the compute queues).
                for h in range(2):
                    dst_p = h * HD
                    lcp = l_pool.tile([1, 512], F32, tag="lcp")
                    nc.vector.tensor_copy(out=lcp, in_=zp[h][HD:HD + 1, :])
                    lr = l_pool.tile([P, 4], F32, tag="lr")
                    nc.sync.dma_start(out=lr, in_=lcp)
                    li = l_pool.tile([P, 4], F32, tag="li")
                    nc.vector.reciprocal(out=li, in_=lr)
                    lrow = l_pool.tile([1, 512], F32, tag="lrow")
                    nc.sync.dma_start(out=lrow, in_=li)
                    lb = l_pool.tile([P, 512], F32, tag="lb")
                    nc.gpsimd.partition_broadcast(lb, lrow)
                    nc.vector.tensor_tensor(
                        out=zT[dst_p:dst_p + HD, hp, ts(j, 512)],
                        in0=zp[h][0:HD, :], in1=lb[0:HD, :],
                        op=ALU.mult)

            def load_vt(j, hp):
                """V for the (j, hp) slot list, as 8-chunk tiles."""
                hs = slice(2 * hp, 2 * hp + 2)
                tiles = []
                if j == 0:
                    vt = v_pool.tile([P, 8, 2, HD + 1], BF16, tag="vt")
                    nc.sync.dma_start(
                        out=vt[:, 0:4],
                        in_=V_dram[0:4, :, hs, :].rearrange("c t h d -> t c h d"))
                    nc.sync.dma_start(
                        out=vt[:, 4:8],
                        in_=V_dram[8:12, :, hs, :].rearrange("c t h d -> t c h d"))
                    tiles.append(vt)
                else:
                    for g in range(2):
                        vt = v_pool.tile([P, 8, 2, HD + 1], BF16, tag="vt")
                        nc.sync.dma_start(
                            out=vt,
                            in_=V_dram[8 * g:8 * g + 8, :, hs, :].rearrange(
                                "c t h d -> t c h d"))
                        tiles.append(vt)
                return tiles

            # ---- j = 0: interleave the remaining head-pairs' K/Q --------
            vt_next = load_vt(0, 0)
            for hp in range(HP):
                vt = vt_next
                if hp + 1 < HP:
                    vt_next = load_vt(0, hp + 1)
                extra = {}
                if hp + 2 < HP:
                    w = load_w(hp + 2)
                    for gi, (kind, seg) in enumerate(KQ_GROUPS):
                        slot = gi + 1
                        extra[slot] = (lambda k_=kind, s_=seg, w_=w, h_=hp + 2:
                                       emit_kq_group(h_, w_[0], w_[1], k_, s_,
                                                     mm_psum))
                attn_tile(0, hp, vt, extra)

            # lnT no longer needed (all K/Q/V computed)
            lnT_stk.close()

            # open FFN pools now that lnT is freed (right side: the left
            # attention pools close later, LIFO is per (space, side))
            wp_pool = ffn_stk.enter_context(
                tc.tile_pool(name="wp_pool", bufs=1, side="right"))
            w1s_pool = ffn_stk.enter_context(
                tc.tile_pool(name="w1s", bufs=2, side="right"))
            a0_pool = ffn_stk.enter_context(
                tc.tile_pool(name="a0_pool", bufs=1, side="right"))
            res = ffn_stk.enter_context(
                tc.tile_pool(name="res", bufs=2, side="right"))

            wpt = wp_pool.tile([P, EC, E], BF16)
            nc.gpsimd.dma_start(out=wpt, in_=wp_d.rearrange("dc d e -> d dc e"))
            # FFN hidden activations, one half-tile of queries each
            # (a_half[1] is allocated later, in the tail's pool)
            a_half = [a0_pool.tile([P, FC, 512], BF16, name="a0")]

            def proj_ln2(qc, psum):
                xo = xstream.tile([P, E], F32, tag="x")
                nc.gpsimd.dma_start(out=xo, in_=xp_d[ts(qc, P), :])
                o1 = res.tile([P, E], F32, tag="o1")
                for half in range(2):
                    po = psum.tile([P, 512], F32, tag="mm")
                    for dc in range(EC):
                        nc.tensor.matmul(po, zT[:, dc, ts(qc, P)],
                                         wpt[:, dc, ts(half, 512)],
                                         start=(dc == 0), stop=(dc == EC - 1))
                    nc.vector.scalar_tensor_tensor(
                        out=o1[:, ts(half, 512)], in0=po, scalar=1.0,
                        in1=xo[:, ts(half, 512)], op0=ALU.mult, op1=ALU.add)
                    nc.vector.tensor_tensor(
                        out=o1[:, ts(half, 512)], in0=o1[:, ts(half, 512)],
                        in1=bp_sb[:, ts(half, 512)], op=ALU.add)
                nc.gpsimd.dma_start(out=out1_dram[ts(qc, P), :], in_=o1)
                layernorm_T(nc, small, acts, o1, ln2T, qc, "ln2", quake=True)

            def ffn1(fc, half, psum):
                w1t = w1s_pool.tile([P, EC, P], BF16, tag="w1c")
                nc.gpsimd.dma_start(out=w1t, in_=w1_d[fc].rearrange("ec e f -> e ec f"))
                pa = psum.tile([P, 512], F32, tag="mm")
                for ec in range(EC):
                    nc.tensor.matmul(pa, w1t[:, ec], ln2T[:, ec, ts(half, 512)],
                                     start=(ec == 0), stop=(ec == EC - 1))
                nc.scalar.activation(out=a_half[half][:, fc, :], in_=pa,
                                     func=AF.Relu, bias=b1_sb[:, fc:fc + 1],
                                     scale=1.0)

            # ---- j = 1: interleave proj/LN2 (tile0) and FFN1 (tile0) ----
            vt_next = load_vt(1, 0)
            for hp in range(HP):
                vt = vt_next
                if hp + 1 < HP:
                    vt_next = load_vt(1, hp + 1)
                extra = {}
                if 1 <= hp <= 4:
                    extra[2] = (lambda q_=hp - 1: proj_ln2(q_, mm_psum))
                elif hp >= 5:
                    counts = {5: (0, 5), 6: (5, 5), 7: (10, 6)}
                    base, cnt = counts[hp]
                    for gi in range(cnt):
                        extra[2 * gi + 1] = (
                            lambda f_=base + gi: ffn1(f_, 0, mm_psum))
                attn_tile(1, hp, vt, extra)

            attn_stk.close()

            # ---------------- tail: proj t1, FFN1 rest, FFN2 -------------
            tail_psum = ffn_stk.enter_context(
                tc.tile_pool(name="tail_psum", bufs=4, space="PSUM"))
            w2_pool = ffn_stk.enter_context(
                tc.tile_pool(name="w2_pool", bufs=1, side="right"))
            res2 = ffn_stk.enter_context(
                tc.tile_pool(name="res2", bufs=2, side="right"))
            w2t = w2_pool.tile([P, FC, E], BF16)
            a_half.append(w2_pool.tile([P, FC, 512], BF16, name="a1"))

            def ffn2(qc):
                o1r = res2.tile([P, E], F32, tag="o1r")
                nc.gpsimd.dma_start(out=o1r, in_=out1_dram[ts(qc, P), :])
                fin = res2.tile([P, E], F32, tag="fin")
                a_sb = a_half[qc // 4]
                for half in range(2):
                    pf = tail_psum.tile([P, 512], F32, tag="mm")
                    for fc in range(FC):
                        nc.tensor.matmul(pf, a_sb[:, fc, ts(qc % 4, P)],
                                         w2t[:, fc, ts(half, 512)],
                                         start=(fc == 0), stop=(fc == FC - 1))
                    nc.vector.scalar_tensor_tensor(
                        out=fin[:, ts(half, 512)], in0=pf, scalar=1.0,
                        in1=o1r[:, ts(half, 512)], op0=ALU.mult, op1=ALU.add)
                    nc.vector.tensor_tensor(
                        out=fin[:, ts(half, 512)], in0=fin[:, ts(half, 512)],
                        in1=b2_sb[:, ts(half, 512)], op=ALU.add)
                nc.sync.dma_start(out=out_d[ts(qc, P), :], in_=fin)

            for qc in range(4, QC):
                proj_ln2(qc, tail_psum)
            nc.gpsimd.dma_start(out=w2t, in_=w2_d.rearrange("fc f e -> f fc e"))
            for qc in range(0, 4):
                ffn2(qc)
            for fc in range(FC):
                ffn1(fc, 1, tail_psum)
            for qc in range(4, QC):
                ffn2(qc)

            ffn_stk.close()

    nc.compile()
    return nc


def _perms():
    a_own = np.concatenate([np.arange(0, 512), np.arange(1536, 2048)])
    a_rest = np.arange(512, 1536)
    b_own = np.arange(512, 1536)
    b_rest = np.concatenate([np.arange(0, 512), np.arange(1536, 2048)])
    return [np.concatenate([a_own, a_rest]), np.concatenate([b_own, b_rest])], \
           [a_own, b_own]


def _mask_bias():
    mb = [np.zeros(24, np.float32), np.zeros(24, np.float32)]
    mb[0][4:8] = NEG     # role A, tile0, chunks 8-11 (future keys)
    mb[1][20:24] = NEG   # role B, tile1, chunks 12-15 (future keys)
    return mb


def _tri_masks():
    """msk[k][p, h*512+q] = 1 if q >= p + 128k else 0 (both head halves)."""
    m = np.zeros((4, P, 1024), np.float32)
    q = np.arange(512)
    p = np.arange(P)
    for k in range(4):
        keep = (q[None, :] >= p[:, None] + P * k).astype(np.float32)
        m[k, :, 0:512] = keep
        m[k, :, 512:1024] = keep
    return m.astype(ml_dtypes.bfloat16)


def _prep_shared(wq, wk, wv, w_proj, b_proj, w1, b1, w2, b2,
                 ln1_g, ln1_b, ln2_g, ln2_b):
    bf = ml_dtypes.bfloat16
    f32 = np.float32
    wq = np.asarray(wq, f32) * np.asarray(ln1_g, f32)[None, :, None]
    wk = np.asarray(wk, f32) * np.asarray(ln1_g, f32)[None, :, None]
    wv = np.asarray(wv, f32) * np.asarray(ln1_g, f32)[None, :, None]
    w1 = np.asarray(w1, f32) * np.asarray(ln2_g, f32)[:, None]

    qb = np.einsum("e,hed->hd", np.asarray(ln1_b, f32), wq) * (HD ** -0.5)
    kb = np.einsum("e,hed->hd", np.asarray(ln1_b, f32), wk)
    vb = np.einsum("e,hed->hd", np.asarray(ln1_b, f32), wv)  # [H, HD]
    bp_eff = np.asarray(b_proj, f32) + vb.reshape(-1) @ np.asarray(w_proj, f32)
    b1_eff = np.asarray(b1, f32) + np.asarray(ln2_b, f32) @ w1

    def pack_pair(w):  # [H, E, HD] -> [H/2, E/P, P, P] bf16
        wpair = w.reshape(H // 2, 2, E, HD)
        cat = np.concatenate([wpair[:, 0], wpair[:, 1]], axis=-1)  # [H/2,E,128]
        return np.ascontiguousarray(cat.reshape(H // 2, E // P, P, P)).astype(bf)

    def pack_bias(b):  # [H, HD] -> [P, HP]  (h0|h1 stacked per pair)
        return np.ascontiguousarray(
            b.reshape(HP, 2 * HD).T).astype(f32)

    shared = {
        "wq2": pack_pair(wq),
        "wk2": pack_pair(wk),
        "wv": np.ascontiguousarray(
            wv.transpose(1, 0, 2).reshape(E // P, P, E)).astype(bf),
        "wp": np.ascontiguousarray(
            np.asarray(w_proj, f32).reshape(E // P, P, E)).astype(bf),
        "w2": np.ascontiguousarray(
            np.asarray(w2, f32).reshape(FF // P, P, E)).astype(bf),
        "b1t": np.ascontiguousarray(b1_eff.reshape(FF // P, P).T).astype(f32),
        "bproj": bp_eff.astype(f32),
        "b2": np.asarray(b2, f32),
        "kbt": pack_bias(kb),
        "qbt": pack_bias(qb),
        "msk": _tri_masks(),
    }
    # w1 layout: [FC, EC, P(e), P(f)]: chunk fc holds w1[e, fc*128+f]
    w1r = w1.reshape(E // P, P, FF // P, P)           # [ec, pe, fc, pf]
    shared["w1"] = np.ascontiguousarray(
        w1r.transpose(2, 0, 1, 3)).astype(bf)          # [fc, ec, pe, pf]
    return shared


def make_in_maps(x, **weights):
    """Build the 8 per-core input dicts (and the gather info)."""
    shared = _prep_shared(**weights)
    perms, owns = _perms()
    mbs = _mask_bias()
    in_maps = []
    for c in range(8):
        b, r = c // 2, c % 2
        m = dict(shared)
        m["xp"] = np.ascontiguousarray(np.asarray(x, np.float32)[b][perms[r]])
        m["mb"] = mbs[r]
        in_maps.append(m)
    return in_maps, owns


def get_nc():
    if "nc" not in _CACHE:
        _CACHE["nc"] = _build_program()
    return _CACHE["nc"]


def kernel(x, wq, wk, wv, w_proj, b_proj, w1, b1, w2, b2,
           ln1_g, ln1_b, ln2_g, ln2_b):
    x = np.asarray(x, dtype=np.float32)
    weights = dict(wq=np.asarray(wq), wk=np.asarray(wk), wv=np.asarray(wv),
                   w_proj=np.asarray(w_proj), b_proj=np.asarray(b_proj),
                   w1=np.asarray(w1), b1=np.asarray(b1), w2=np.asarray(w2),
                   b2=np.asarray(b2), ln1_g=np.asarray(ln1_g),
                   ln1_b=np.asarray(ln1_b), ln2_g=np.asarray(ln2_g),
                   ln2_b=np.asarray(ln2_b))
    nc = get_nc()
    in_maps, owns = make_in_maps(x, **weights)
    res = run_bass_kernel_spmd(nc, in_maps, core_ids=list(range(8)))
    out = np.empty((4, S, E), dtype=np.float32)
    for c in range(8):
        b, r = c // 2, c % 2
        out[b][owns[r]] = res.results[c]["out"]
    return out


# revision 28
# speedup vs baseline: 1.0615x; 1.0615x over previous
"""Trainium2 Bass kernel for a dense pre-LN transformer block (v2).

Block: y = x + proj(causal_mha(LN1(x))) ; out = y + FFN(LN2(y))
Shapes (hardcoded): x [4, 2048, 1024], H=16 heads, HD=64, FF=2048, fp32 I/O.

Sharding (8 cores, no collectives): core c handles batch b=c//2 and a
balanced half of the queries (role r=c%2; A: rows [0,512)+[1536,2048),
B: rows [512,1536)).  The key/value sequence is permuted on the host per
core (own rows first) so one SPMD program serves both roles; causality is
enforced by host-built triangular masks on the diagonal chunks plus a
per-core exp-bias table (-100 => exp ~ 0) for role-dependent chunks.

v2 changes vs v1:
- LN gamma/beta folded into host-prepped weights/biases (no on-chip g/b).
- Transposes via DMA xbar (dma_start transpose=True), not the PE array.
- rsqrt for LN via DVE quake-rsqrt (no activation-table switches; the
  scalar engine only ever uses the exp/identity/relu table set).
- Flat [128,1024] masks (no broadcast APs on the DVE mask multiply).
- Softmax 1/l fused into the z evacuation; no SBUF->SBUF DMA dances.
- V streamed per (hp,j) in chunk batches (24 DMA loads, not 192).
- Attention loops j-outer so proj/LN2/FFN1 of the first query tile
  overlap the scalar-bound attention of the second tile.
"""

import numpy as np
import ml_dtypes

import concourse.bass as bass
import concourse.bacc as bacc
import concourse.tile as tile
import concourse.mybir as mybir
from concourse.bass import ts
from concourse.bass_utils import run_bass_kernel_spmd

BF16 = mybir.dt.bfloat16
F32 = mybir.dt.float32
U32 = mybir.dt.uint32
AF = mybir.ActivationFunctionType
ALU = mybir.AluOpType

S = 2048          # sequence length
E = 1024          # embedding dim
H = 16            # heads
HD = 64           # head dim
FF = 2048         # ffn hidden
P = 128           # partitions
NQ = 1024         # queries owned per core
EPS = 1e-5
NEG = -100.0      # exp bias for masked-out chunks (exp(-100) ~ 0)

EC = E // P       # 8 e-chunks
FC = FF // P      # 16 f-chunks
NCH = S // P      # 16 key chunks
HP = H // 2       # 8 head pairs
QC = NQ // P      # 8 own query row-tiles

# chunk schedule (in permuted key coordinates), identical on every core:
# q-tile 0 (own positions [0,512)):   key chunks 0-3 (diag) + 8-11 (role-dep)
# q-tile 1 (own positions [512,1024)): key chunks 0-15 (4-7 diag, 12-15 role-dep)
SCHED = [[0, 1, 2, 3, 8, 9, 10, 11], list(range(16))]
DIAG = [set(range(0, 4)), set(range(4, 8))]

_CACHE = {}
_QCONST = [None]
_EPS_T = [None]


def _build_program():
    nc = bacc.Bacc("TRN2", target_bir_lowering=False, debug=False)

    # ---- per-core dram inputs -------------------------------------------
    xp_d = nc.dram_tensor("xp", [S, E], F32, kind="ExternalInput")
    wq_d = nc.dram_tensor("wq2", [HP, EC, P, P], BF16, kind="ExternalInput")
    wk_d = nc.dram_tensor("wk2", [HP, EC, P, P], BF16, kind="ExternalInput")
    wv_d = nc.dram_tensor("wv", [EC, P, E], BF16, kind="ExternalInput")
    wp_d = nc.dram_tensor("wp", [EC, P, E], BF16, kind="ExternalInput")
    w1_d = nc.dram_tensor("w1", [FC, EC, P, P], BF16, kind="ExternalInput")
    w2_d = nc.dram_tensor("w2", [FC, P, E], BF16, kind="ExternalInput")
    b1_d = nc.dram_tensor("b1t", [P, FC], F32, kind="ExternalInput")
    bp_d = nc.dram_tensor("bproj", [E], F32, kind="ExternalInput")
    b2_d = nc.dram_tensor("b2", [E], F32, kind="ExternalInput")
    kb_d = nc.dram_tensor("kbt", [P, HP], F32, kind="ExternalInput")
    qb_d = nc.dram_tensor("qbt", [P, HP], F32, kind="ExternalInput")
    mb_d = nc.dram_tensor("mb", [24], F32, kind="ExternalInput")
    msk_d = nc.dram_tensor("msk", [4, P, 1024], BF16, kind="ExternalInput")
    out_d = nc.dram_tensor("out", [NQ, E], F32, kind="ExternalOutput")

    QSCALE = float(HD) ** -0.5

    def rstd_quake(nc, small, var_ap, tag):
        """1/sqrt(var+eps) on the DVE only ([P,1] tiles, quake + 1 Newton).
        Used mid-attention (LN2) to avoid scalar activation-table switches."""
        a = small.tile([P, 1], F32, tag=tag + "_a")
        nc.vector.tensor_scalar(out=a, in0=var_ap, scalar1=EPS, scalar2=None,
                                op0=ALU.add)
        s1 = small.tile([P, 1], U32, tag=tag + "_s")
        nc.vector.tensor_scalar(out=s1, in0=a.bitcast(U32), scalar1=1,
                                scalar2=None, op0=ALU.logical_shift_right)
        y0b = small.tile([P, 1], U32, tag=tag + "_y0b")
        nc.vector.tensor_tensor(out=y0b, in0=_QCONST[0], in1=s1,
                                op=ALU.subtract)
        y = y0b.bitcast(F32)
        for it in range(2):
            t2 = small.tile([P, 1], F32, tag=tag + f"_u{it}")
            nc.vector.tensor_tensor(out=t2, in0=y, in1=y, op=ALU.mult)
            t3 = small.tile([P, 1], F32, tag=tag + f"_v{it}")
            nc.vector.scalar_tensor_tensor(out=t3, in0=t2, scalar=-0.5,
                                           in1=a, op0=ALU.mult, op1=ALU.mult)
            u = small.tile([P, 1], F32, tag=tag + f"_x{it}")
            nc.vector.tensor_scalar(out=u, in0=t3, scalar1=1.5, scalar2=None,
                                    op0=ALU.add)
            yn = small.tile([P, 1], F32, tag=tag + f"_w{it}")
            nc.vector.tensor_tensor(out=yn, in0=y, in1=u, op=ALU.mult)
            y = yn
        return y

    def layernorm_T(nc, small, acts, x_ap, dstT, sc, tag, quake):
        """LN of one [128,E] fp32 row-tile -> bf16 transpose into
        dstT[:, :, sc*128:(sc+1)*128] via the DMA xbar.  The normalize
        apply runs on the scalar engine (Identity is in every table set);
        rstd comes from scalar Sqrt when quake=False (only safe before the
        exp table is loaded) or the DVE quake chain when True."""
        stats = small.tile([P, 2, 6], F32, tag=tag + "_bn")
        for g in range(2):
            nc.vector.bn_stats(out=stats[:, g, :], in_=x_ap[:, g * 512:(g + 1) * 512])
        mv = small.tile([P, 2], F32, tag=tag + "_mv")
        nc.vector.bn_aggr(out=mv, in_=stats)
        if quake:
            rstd = rstd_quake(nc, small, mv[:, 1:2], tag)
        else:
            std = small.tile([P, 1], F32, tag=tag + "_std")
            nc.scalar.activation(out=std, in_=mv[:, 1:2], func=AF.Sqrt,
                                 bias=_EPS_T[0], scale=1.0)
            rstd = small.tile([P, 1], F32, tag=tag + "_rstd")
            nc.vector.reciprocal(out=rstd, in_=std)
        nm = small.tile([P, 1], F32, tag=tag + "_nm")
        nc.vector.scalar_tensor_tensor(out=nm, in0=mv[:, 0:1], scalar=-1.0,
                                       in1=rstd, op0=ALU.mult, op1=ALU.mult)
        tmp = acts.tile([P, E], BF16, tag=tag + "_tmp")
        nc.scalar.activation(out=tmp, in_=x_ap, func=AF.Identity,
                             bias=nm, scale=rstd)
        nc.sync.dma_start(out=dstT[:, :, ts(sc, P)], in_=tmp, transpose=True)

    with tile.TileContext(nc) as tc:
        import contextlib
        stk = contextlib.ExitStack()
        with stk:
            const = stk.enter_context(tc.tile_pool(name="const", bufs=1))
            small = stk.enter_context(tc.tile_pool(name="small", bufs=4))
            dram = stk.enter_context(tc.tile_pool(name="dram", bufs=1, space="DRAM"))

            qconst = const.tile([P, 1], U32)
            nc.vector.memset(qconst, 0x5F3759DF)
            eps_t = const.tile([P, 1], F32)
            nc.vector.memset(eps_t, EPS)
            global _QCONST, _EPS_T
            _QCONST = [qconst]
            _EPS_T = [eps_t]

            mb_sb = const.tile([P, 24], F32)
            nc.gpsimd.dma_start(out=mb_sb, in_=mb_d[None, :].to_broadcast((P, 24)))
            b1_sb = const.tile([P, FC], F32)
            nc.gpsimd.dma_start(out=b1_sb, in_=b1_d[:, :])
            bp_sb = const.tile([P, E], F32)
            nc.gpsimd.dma_start(out=bp_sb, in_=bp_d[None, :].to_broadcast((P, E)))
            b2_sb = const.tile([P, E], F32)
            nc.gpsimd.dma_start(out=b2_sb, in_=b2_d[None, :].to_broadcast((P, E)))
            kb_sb = const.tile([P, HP], F32)
            nc.gpsimd.dma_start(out=kb_sb, in_=kb_d[:, :])
            qb_sb = const.tile([P, HP], F32)
            nc.gpsimd.dma_start(out=qb_sb, in_=qb_d[:, :])
            msk = const.tile([P, 4, 1024], BF16)
            nc.sync.dma_start(out=msk, in_=msk_d.rearrange("k p q -> p k q"))

            out1_dram = dram.tile([NQ, E], F32)
            # V streamed through DRAM: [ch, t, h, 64] values + ones col 64
            V_dram = dram.tile([NCH, P, H, HD + 1], BF16)

            # persistent across attention + tail (zT until tail proj, ln2T
            # until tail FFN1)
            big = stk.enter_context(tc.tile_pool(name="big", bufs=1))
            zT = big.tile([P, EC, NQ], BF16)
            ln2T = big.tile([P, EC, NQ], BF16)

            xstream = stk.enter_context(tc.tile_pool(name="xstream", bufs=3))
            acts = stk.enter_context(tc.tile_pool(name="acts", bufs=2))

            # ---------------- phase A/B: LN1 + V (interleaved) -----------
            # lnT + wvt live on the RIGHT side: freed mid-program while the
            # left-side attention pools are still open (strict LIFO per side).
            lnT_stk = contextlib.ExitStack()
            lnT_pool = lnT_stk.enter_context(
                tc.tile_pool(name="lnT_pool", bufs=1, side="right"))
            lnT = lnT_pool.tile([P, EC, S], BF16)
            wvt = lnT_pool.tile([P, EC, E], BF16)
            nc.gpsimd.dma_start(out=wvt, in_=wv_d.rearrange("ec e n -> e ec n"))

            attn_stk = contextlib.ExitStack()
            kq_pool = attn_stk.enter_context(tc.tile_pool(name="kq_pool", bufs=1))
            KT = [kq_pool.tile([P, S], BF16, name=f"KT{i}") for i in range(HP)]
            QT = [kq_pool.tile([P, NQ], BF16, name=f"QT{i}") for i in range(HP)]
            wstream = attn_stk.enter_context(tc.tile_pool(name="wstream", bufs=2))

            # deeper psum ring for the LN1/V/KQ phase (closed before the
            # attention psum pools open)
            pre_stk = contextlib.ExitStack()
            pre_psum = pre_stk.enter_context(
                tc.tile_pool(name="pre_psum", bufs=6, space="PSUM"))

            # All 16 LN1 chains first: their DVE/scalar/DMA hops pipeline
            # across tiles.  Emitting V right after its own LN tile would
            # park V's psum-evac copy at the DVE queue head, blocking the
            # next tile's bn_stats on this tile's full LN->transpose->matmul
            # latency chain.
            def load_x(sc):
                xt = xstream.tile([P, E], F32, tag="x")
                nc.sync.dma_start(out=xt, in_=xp_d[ts(sc, P), :])
                return xt
            xtiles = [load_x(sc) for sc in range(3)]
            for sc in range(S // P):
                layernorm_T(nc, small, acts, xtiles[sc], lnT, sc, "ln1",
                            quake=True)
                if sc + 3 < S // P:
                    xtiles.append(load_x(sc + 3))

            def v_chunk(ch):
                for half in range(2):
                    pv = pre_psum.tile([P, 512], F32, tag="mm")
                    for ec in range(EC):
                        nc.tensor.matmul(pv, lnT[:, ec, ts(ch, P)],
                                         wvt[:, ec, ts(half, 512)],
                                         start=(ec == 0), stop=(ec == EC - 1))
                    vsb = acts.tile([P, 8, HD + 1], BF16, tag="vsb")
                    nc.vector.memset(vsb[:, :, HD:HD + 1], 1.0)
                    nc.vector.tensor_copy(
                        out=vsb[:, :, 0:HD],
                        in_=pv.rearrange("p (h d) -> p h d", d=HD))
                    nc.sync.dma_start(
                        out=V_dram[ch, :, 8 * half:8 * (half + 1), :], in_=vsb)

            for ch in range(NCH):
                v_chunk(ch)

            # ---- K/Q for one head pair (6 psum chains of 8 matmuls) -----
            def load_w(hp):
                wkt = wstream.tile([P, EC, P], BF16, tag="wk")
                nc.gpsimd.dma_start(out=wkt, in_=wk_d[hp].rearrange("ec e d -> e ec d"))
                wqt = wstream.tile([P, EC, P], BF16, tag="wq")
                nc.gpsimd.dma_start(out=wqt, in_=wq_d[hp].rearrange("ec e d -> e ec d"))
                return wkt, wqt

            def emit_kq_group(hp, wkt, wqt, kind, seg, psum):
                pk = psum.tile([P, 512], F32, tag="mm", name="pk")
                wt = wkt if kind == "k" else wqt
                for ec in range(EC):
                    nc.tensor.matmul(pk, wt[:, ec], lnT[:, ec, ts(seg, 512)],
                                     start=(ec == 0), stop=(ec == EC - 1))
                if kind == "k":
                    nc.vector.scalar_tensor_tensor(
                        out=KT[hp][:, ts(seg, 512)], in0=pk, scalar=1.0,
                        in1=kb_sb[:, hp:hp + 1].to_broadcast((P, 512)),
                        op0=ALU.mult, op1=ALU.add)
                else:
                    nc.vector.scalar_tensor_tensor(
                        out=QT[hp][:, ts(seg, 512)], in0=pk, scalar=QSCALE,
                        in1=qb_sb[:, hp:hp + 1].to_broadcast((P, 512)),
                        op0=ALU.mult, op1=ALU.add)

            KQ_GROUPS = [("k", 0), ("k", 1), ("k", 2), ("k", 3),
                         ("q", 0), ("q", 1)]
            for hp in (0, 1):
                w = load_w(hp)
                for kind, seg in KQ_GROUPS:
                    emit_kq_group(hp, w[0], w[1], kind, seg, pre_psum)
            pre_stk.close()
            mm_psum = attn_stk.enter_context(
                tc.tile_pool(name="mm_psum", bufs=2, space="PSUM"))

            # ---------------- attention --------------------------------
            st_psum = attn_stk.enter_context(
                tc.tile_pool(name="st_psum", bufs=2, space="PSUM"))
            z_psum = attn_stk.enter_context(
                tc.tile_pool(name="z_psum", bufs=2, space="PSUM"))
            p_pool = attn_stk.enter_context(tc.tile_pool(name="p_pool", bufs=4))
            v_pool = attn_stk.enter_context(tc.tile_pool(name="v_pool", bufs=4))
            l_pool = attn_stk.enter_context(tc.tile_pool(name="l_pool", bufs=2))

            # FFN-phase pools (opened before j=1 so proj/FFN1 can interleave)
            ffn_stk = contextlib.ExitStack()

            def attn_tile(j, hp, vt, extra_work):
                """One (q-tile, head-pair): scores -> exp -> z, with
                extra_work[(slot_idx)] emitted into the exp gaps."""
                sched = SCHED[j]
                zp = [z_psum.tile([P, 512], F32, tag="z", name=f"zp{h}")
                      for h in range(2)]
                pend = None

                def flush_z(pend):
                    pt_, vi_, off_, first_, last_ = pend
                    vtile = vt[vi_ // 8]
                    for h in range(2):
                        nc.tensor.matmul(zp[h][0:HD + 1, off_:512],
                                         vtile[:, vi_ % 8, h],
                                         pt_[:, h, off_:512],
                                         start=first_, stop=last_)

                for ci, ch in enumerate(sched):
                    st = st_psum.tile([P, 1024], F32, tag="st")
                    nc.tensor.matmul(
                        st[:, 0:512], KT[hp][0:HD, ts(ch, P)],
                        QT[hp][0:HD, ts(j, 512)],
                        start=True, stop=True, tile_position=(0, 0))
                    nc.tensor.matmul(
                        st[:, 512:1024], KT[hp][HD:P, ts(ch, P)],
                        QT[hp][HD:P, ts(j, 512)],
                        start=True, stop=True, tile_position=(HD, 0))
                    slot = (8 if j else 0) + ci
                    pt = p_pool.tile([P, 2, 512], BF16, tag="p")
                    nc.scalar.activation(
                        out=pt.rearrange("p a b -> p (a b)"), in_=st,
                        func=AF.Exp, bias=mb_sb[:, slot:slot + 1], scale=1.0)
                    off = 0
                    if ch in DIAG[j]:
                        k = ch - 4 * j
                        off = P * k
                        nc.vector.tensor_tensor(
                            out=pt[:, :, off:512], in0=pt[:, :, off:512],
                            in1=msk[:, k:k + 1, off:512].to_broadcast(
                                (P, 2, 512 - off)),
                            op=ALU.mult)
                    if ci in extra_work:
                        extra_work[ci]()
                    if pend is not None:
                        flush_z(pend)
                    pend = (pt, ci, off, ci == 0, ci == len(sched) - 1)
                flush_z(pend)

                # evacuate z with fused 1/l.  DVE reciprocal is an 8-slice
                # iterative divide (~7 cyc/elem along the free dim), so run
                # it on a [128,4] reshape of the denominator row (DMA
                # round-trip keeps the latency off the compute queues).
                for h in range(2):
                    dst_p = h * HD
                    lcp = l_pool.tile([1, 512], F32, tag="lcp")
                    nc.vector.tensor_copy(out=lcp, in_=zp[h][HD:HD + 1, :])
                    lr = l_pool.tile([P, 4], F32, tag="lr")
                    nc.sync.dma_start(out=lr, in_=lcp)
                    li = l_pool.tile([P, 4], F32, tag="li")
                    nc.vector.reciprocal(out=li, in_=lr)
                    lrow = l_pool.tile([1, 512], F32, tag="lrow")
                    nc.sync.dma_start(out=lrow, in_=li)
                    lb = l_pool.tile([P, 512], F32, tag="lb")
                    nc.gpsimd.partition_broadcast(lb, lrow)
                    nc.vector.tensor_tensor(
                        out=zT[dst_p:dst_p + HD, hp, ts(j, 512)],
                        in0=zp[h][0:HD, :], in1=lb[0:HD, :],
                        op=ALU.mult)

            def load_vt(j, hp):
                """V for the (j, hp) slot list, as 8-chunk tiles."""
                hs = slice(2 * hp, 2 * hp + 2)
                tiles = []
                if j == 0:
                    vt = v_pool.tile([P, 8, 2, HD + 1], BF16, tag="vt")
                    nc.sync.dma_start(
                        out=vt[:, 0:4],
                        in_=V_dram[0:4, :, hs, :].rearrange("c t h d -> t c h d"))
                    nc.sync.dma_start(
                        out=vt[:, 4:8],
                        in_=V_dram[8:12, :, hs, :].rearrange("c t h d -> t c h d"))
                    tiles.append(vt)
                else:
                    for g in range(2):
                        vt = v_pool.tile([P, 8, 2, HD + 1], BF16, tag="vt")
                        nc.sync.dma_start(
                            out=vt,
                            in_=V_dram[8 * g:8 * g + 8, :, hs, :].rearrange(
                                "c t h d -> t c h d"))
                        tiles.append(vt)
                return tiles

            # ---- j = 0: interleave the remaining head-pairs' K/Q --------
            vt_next = load_vt(0, 0)
            for hp in range(HP):
                vt = vt_next
                if hp + 1 < HP:
                    vt_next = load_vt(0, hp + 1)
                extra = {}
                if hp + 2 < HP:
                    w = load_w(hp + 2)
                    for gi, (kind, seg) in enumerate(KQ_GROUPS):
                        slot = gi + 1
                        extra[slot] = (lambda k_=kind, s_=seg, w_=w, h_=hp + 2:
                                       emit_kq_group(h_, w_[0], w_[1], k_, s_,
                                                     mm_psum))
                attn_tile(0, hp, vt, extra)

            # lnT no longer needed (all K/Q/V computed)
            lnT_stk.close()

            # open FFN pools now that lnT is freed (right side: the left
            # attention pools close later, LIFO is per (space, side))
            wp_pool = ffn_stk.enter_context(
                tc.tile_pool(name="wp_pool", bufs=1, side="right"))
            w1s_pool = ffn_stk.enter_context(
                tc.tile_pool(name="w1s", bufs=2, side="right"))
            a0_pool = ffn_stk.enter_context(
                tc.tile_pool(name="a0_pool", bufs=1, side="right"))
            res = ffn_stk.enter_context(
                tc.tile_pool(name="res", bufs=2, side="right"))

            wpt = wp_pool.tile([P, EC, E], BF16)
            nc.gpsimd.dma_start(out=wpt, in_=wp_d.rearrange("dc d e -> d dc e"))
            # FFN hidden activations, one half-tile of queries each
            # (a_half[1] is allocated later, in the tail's pool)
            a_half = [a0_pool.tile([P, FC, 512], BF16, name="a0")]

            def proj_ln2(qc, psum):
                xo = xstream.tile([P, E], F32, tag="x")
                nc.gpsimd.dma_start(out=xo, in_=xp_d[ts(qc, P), :])
                o1 = res.tile([P, E], F32, tag="o1")
                for half in range(2):
                    po = psum.tile([P, 512], F32, tag="mm")
                    for dc in range(EC):
                        nc.tensor.matmul(po, zT[:, dc, ts(qc, P)],
                                         wpt[:, dc, ts(half, 512)],
                                         start=(dc == 0), stop=(dc == EC - 1))
                    nc.vector.scalar_tensor_tensor(
                        out=o1[:, ts(half, 512)], in0=po, scalar=1.0,
                        in1=xo[:, ts(half, 512)], op0=ALU.mult, op1=ALU.add)
                    nc.vector.tensor_tensor(
                        out=o1[:, ts(half, 512)], in0=o1[:, ts(half, 512)],
                        in1=bp_sb[:, ts(half, 512)], op=ALU.add)
                nc.gpsimd.dma_start(out=out1_dram[ts(qc, P), :], in_=o1)
                layernorm_T(nc, small, acts, o1, ln2T, qc, "ln2", quake=True)

            def ffn1(fc, half, psum):
                w1t = w1s_pool.tile([P, EC, P], BF16, tag="w1c")
                nc.gpsimd.dma_start(out=w1t, in_=w1_d[fc].rearrange("ec e f -> e ec f"))
                pa = psum.tile([P, 512], F32, tag="mm")
                for ec in range(EC):
                    nc.tensor.matmul(pa, w1t[:, ec], ln2T[:, ec, ts(half, 512)],
                                     start=(ec == 0), stop=(ec == EC - 1))
                nc.scalar.activation(out=a_half[half][:, fc, :], in_=pa,
                                     func=AF.Relu, bias=b1_sb[:, fc:fc + 1],
                                     scale=1.0)

            # ---- j = 1: interleave proj/LN2 (tile0) and FFN1 (tile0) ----
            vt_next = load_vt(1, 0)
            for hp in range(HP):
                vt = vt_next
                if hp + 1 < HP:
                    vt_next = load_vt(1, hp + 1)
                extra = {}
                if 1 <= hp <= 4:
                    extra[2] = (lambda q_=hp - 1: proj_ln2(q_, mm_psum))
                elif hp >= 5:
                    counts = {5: (0, 5), 6: (5, 5), 7: (10, 6)}
                    base, cnt = counts[hp]
                    for gi in range(cnt):
                        extra[2 * gi + 1] = (
                            lambda f_=base + gi: ffn1(f_, 0, mm_psum))
                attn_tile(1, hp, vt, extra)

            attn_stk.close()

            # ---------------- tail: proj t1, FFN1 rest, FFN2 -------------
            tail_psum = ffn_stk.enter_context(
                tc.tile_pool(name="tail_psum", bufs=4, space="PSUM"))
            w2_pool = ffn_stk.enter_context(
                tc.tile_pool(name="w2_pool", bufs=1, side="right"))
            res2 = ffn_stk.enter_context(
                tc.tile_pool(name="res2", bufs=2, side="right"))
            w2t = w2_pool.tile([P, FC, E], BF16)
            a_half.append(w2_pool.tile([P, FC, 512], BF16, name="a1"))

            def ffn2(qc):
                o1r = res2.tile([P, E], F32, tag="o1r")
                nc.gpsimd.dma_start(out=o1r, in_=out1_dram[ts(qc, P), :])
                fin = res2.tile([P, E], F32, tag="fin")
                a_sb = a_half[qc // 4]
                for half in range(2):
                    pf = tail_psum.tile([P, 512], F32, tag="mm")
                    for fc in range(FC):
                        nc.tensor.matmul(pf, a_sb[:, fc, ts(qc % 4, P)],
                                         w2t[:, fc, ts(half, 512)],
                                         start=(fc == 0), stop=(fc == FC - 1))
                    nc.vector.scalar_tensor_tensor(
                        out=fin[:, ts(half, 512)], in0=pf, scalar=1.0,
                        in1=o1r[:, ts(half, 512)], op0=ALU.mult, op1=ALU.add)
                    nc.vector.tensor_tensor(
                        out=fin[:, ts(half, 512)], in0=fin[:, ts(half, 512)],
                        in1=b2_sb[:, ts(half, 512)], op=ALU.add)
                nc.sync.dma_start(out=out_d[ts(qc, P), :], in_=fin)

            for qc in range(4, QC):
                proj_ln2(qc, tail_psum)
            nc.gpsimd.dma_start(out=w2t, in_=w2_d.rearrange("fc f e -> f fc e"))
            for qc in range(0, 4):
                ffn2(qc)
            for fc in range(FC):
                ffn1(fc, 1, tail_psum)
            for qc in range(4, QC):
                ffn2(qc)

            ffn_stk.close()

    nc.compile()
    return nc


def _perms():
    a_own = np.concatenate([np.arange(0, 512), np.arange(1536, 2048)])
    a_rest = np.arange(512, 1536)
    b_own = np.arange(512, 1536)
    b_rest = np.concatenate([np.arange(0, 512), np.arange(1536, 2048)])
    return [np.concatenate([a_own, a_rest]), np.concatenate([b_own, b_rest])], \
           [a_own, b_own]


def _mask_bias():
    mb = [np.zeros(24, np.float32), np.zeros(24, np.float32)]
    mb[0][4:8] = NEG     # role A, tile0, chunks 8-11 (future keys)
    mb[1][20:24] = NEG   # role B, tile1, chunks 12-15 (future keys)
    return mb


def _tri_masks():
    """msk[k][p, h*512+q] = 1 if q >= p + 128k else 0 (both head halves)."""
    m = np.zeros((4, P, 1024), np.float32)
    q = np.arange(512)
    p = np.arange(P)
    for k in range(4):
        keep = (q[None, :] >= p[:, None] + P * k).astype(np.float32)
        m[k, :, 0:512] = keep
        m[k, :, 512:1024] = keep
    return m.astype(ml_dtypes.bfloat16)


def _prep_shared(wq, wk, wv, w_proj, b_proj, w1, b1, w2, b2,
                 ln1_g, ln1_b, ln2_g, ln2_b):
    bf = ml_dtypes.bfloat16
    f32 = np.float32
    wq = np.asarray(wq, f32) * np.asarray(ln1_g, f32)[None, :, None]
    wk = np.asarray(wk, f32) * np.asarray(ln1_g, f32)[None, :, None]
    wv = np.asarray(wv, f32) * np.asarray(ln1_g, f32)[None, :, None]
    w1 = np.asarray(w1, f32) * np.asarray(ln2_g, f32)[:, None]

    qb = np.einsum("e,hed->hd", np.asarray(ln1_b, f32), wq) * (HD ** -0.5)
    kb = np.einsum("e,hed->hd", np.asarray(ln1_b, f32), wk)
    vb = np.einsum("e,hed->hd", np.asarray(ln1_b, f32), wv)  # [H, HD]
    bp_eff = np.asarray(b_proj, f32) + vb.reshape(-1) @ np.asarray(w_proj, f32)
    b1_eff = np.asarray(b1, f32) + np.asarray(ln2_b, f32) @ w1

    def pack_pair(w):  # [H, E, HD] -> [H/2, E/P, P, P] bf16
        wpair = w.reshape(H // 2, 2, E, HD)
        cat = np.concatenate([wpair[:, 0], wpair[:, 1]], axis=-1)  # [H/2,E,128]
        return np.ascontiguousarray(cat.reshape(H // 2, E // P, P, P)).astype(bf)

    def pack_bias(b):  # [H, HD] -> [P, HP]  (h0|h1 stacked per pair)
        return np.ascontiguousarray(
            b.reshape(HP, 2 * HD).T).astype(f32)

    shared = {
        "wq2": pack_pair(wq),
        "wk2": pack_pair(wk),
        "wv": np.ascontiguousarray(
            wv.transpose(1, 0, 2).reshape(E // P, P, E)).astype(bf),
        "wp": np.ascontiguousarray(
            np.asarray(w_proj, f32).reshape(E // P, P, E)).astype(bf),
        "w2": np.ascontiguousarray(
            np.asarray(w2, f32).reshape(FF // P, P, E)).astype(bf),
        "b1t": np.ascontiguousarray(b1_eff.reshape(FF // P, P).T).astype(f32),
        "bproj": bp_eff.astype(f32),
        "b2": np.asarray(b2, f32),
        "kbt": pack_bias(kb),
        "qbt": pack_bias(qb),
        "msk": _tri_masks(),
    }
    # w1 layout: [FC, EC, P(e), P(f)]: chunk fc holds w1[e, fc*128+f]
    w1r = w1.reshape(E // P, P, FF // P, P)           # [ec, pe, fc, pf]
    shared["w1"] = np.ascontiguousarray(
        w1r.transpose(2, 0, 1, 3)).astype(bf)          # [fc, ec, pe, pf]
    return shared


def make_in_maps(x, **weights):
    """Build the 8 per-core input dicts (and the gather info)."""
    shared = _prep_shared(**weights)
    perms, owns = _perms()
    mbs = _mask_bias()
    in_maps = []
    for c in range(8):
        b, r = c // 2, c % 2
        m = dict(shared)
        m["xp"] = np.ascontiguousarray(np.asarray(x, np.float32)[b][perms[r]])
        m["mb"] = mbs[r]
        in_maps.append(m)
    return in_maps, owns


def get_nc():
    if "nc" not in _CACHE:
        _CACHE["nc"] = _build_program()
    return _CACHE["nc"]


def kernel(x, wq, wk, wv, w_proj, b_proj, w1, b1, w2, b2,
           ln1_g, ln1_b, ln2_g, ln2_b):
    x = np.asarray(x, dtype=np.float32)
    weights = dict(wq=np.asarray(wq), wk=np.asarray(wk), wv=np.asarray(wv),
                   w_proj=np.asarray(w_proj), b_proj=np.asarray(b_proj),
                   w1=np.asarray(w1), b1=np.asarray(b1), w2=np.asarray(w2),
                   b2=np.asarray(b2), ln1_g=np.asarray(ln1_g),
                   ln1_b=np.asarray(ln1_b), ln2_g=np.asarray(ln2_g),
                   ln2_b=np.asarray(ln2_b))
    nc = get_nc()
    in_maps, owns = make_in_maps(x, **weights)
    res = run_bass_kernel_spmd(nc, in_maps, core_ids=list(range(8)))
    out = np.empty((4, S, E), dtype=np.float32)
    for c in range(8):
        b, r = c // 2, c % 2
        out[b][owns[r]] = res.results[c]["out"]
    return out


# revision 31
# speedup vs baseline: 1.1185x; 1.0537x over previous
"""Trainium2 Bass kernel for a dense pre-LN transformer block (v2).

Block: y = x + proj(causal_mha(LN1(x))) ; out = y + FFN(LN2(y))
Shapes (hardcoded): x [4, 2048, 1024], H=16 heads, HD=64, FF=2048, fp32 I/O.

Sharding (8 cores, no collectives): core c handles batch b=c//2 and a
balanced half of the queries (role r=c%2; A: rows [0,512)+[1536,2048),
B: rows [512,1536)).  The key/value sequence is permuted on the host per
core (own rows first) so one SPMD program serves both roles; causality is
enforced by host-built triangular masks on the diagonal chunks plus a
per-core exp-bias table (-100 => exp ~ 0) for role-dependent chunks.

v2 changes vs v1:
- LN gamma/beta folded into host-prepped weights/biases (no on-chip g/b).
- Transposes via DMA xbar (dma_start transpose=True), not the PE array.
- rsqrt for LN via DVE quake-rsqrt (no activation-table switches; the
  scalar engine only ever uses the exp/identity/relu table set).
- Flat [128,1024] masks (no broadcast APs on the DVE mask multiply).
- Softmax 1/l fused into the z evacuation; no SBUF->SBUF DMA dances.
- V streamed per (hp,j) in chunk batches (24 DMA loads, not 192).
- Attention loops j-outer so proj/LN2/FFN1 of the first query tile
  overlap the scalar-bound attention of the second tile.
"""

import numpy as np
import ml_dtypes

import concourse.bass as bass
import concourse.bacc as bacc
import concourse.tile as tile
import concourse.mybir as mybir
from concourse.bass import ts
from concourse.bass_utils import run_bass_kernel_spmd
from concourse.masks import make_identity

BF16 = mybir.dt.bfloat16
F32 = mybir.dt.float32
U32 = mybir.dt.uint32
AF = mybir.ActivationFunctionType
ALU = mybir.AluOpType

S = 2048          # sequence length
E = 1024          # embedding dim
H = 16            # heads
HD = 64           # head dim
FF = 2048         # ffn hidden
P = 128           # partitions
NQ = 1024         # queries owned per core
EPS = 1e-5
NEG = -100.0      # exp bias for masked-out chunks (exp(-100) ~ 0)

EC = E // P       # 8 e-chunks
FC = FF // P      # 16 f-chunks
NCH = S // P      # 16 key chunks
HP = H // 2       # 8 head pairs
QC = NQ // P      # 8 own query row-tiles

# chunk schedule (in permuted key coordinates), identical on every core:
# q-tile 0 (own positions [0,512)):   key chunks 0-3 (diag) + 8-11 (role-dep)
# q-tile 1 (own positions [512,1024)): key chunks 0-15 (4-7 diag, 12-15 role-dep)
SCHED = [[0, 1, 2, 3, 8, 9, 10, 11], list(range(16))]
DIAG = [set(range(0, 4)), set(range(4, 8))]

_CACHE = {}
_QCONST = [None]
_EPS_T = [None]
_IDENT = [None]


def _build_program():
    nc = bacc.Bacc("TRN2", target_bir_lowering=False, debug=False)

    # ---- per-core dram inputs -------------------------------------------
    xp_d = nc.dram_tensor("xp", [S, E], F32, kind="ExternalInput")
    wq_d = nc.dram_tensor("wq2", [HP, EC, P, P], BF16, kind="ExternalInput")
    wk_d = nc.dram_tensor("wk2", [HP, EC, P, P], BF16, kind="ExternalInput")
    wv_d = nc.dram_tensor("wv", [EC, P, E], BF16, kind="ExternalInput")
    wp_d = nc.dram_tensor("wp", [EC, P, E], BF16, kind="ExternalInput")
    w1_d = nc.dram_tensor("w1", [FC, EC, P, P], BF16, kind="ExternalInput")
    w2_d = nc.dram_tensor("w2", [FC, P, E], BF16, kind="ExternalInput")
    b1_d = nc.dram_tensor("b1t", [P, FC], F32, kind="ExternalInput")
    bp_d = nc.dram_tensor("bproj", [E], F32, kind="ExternalInput")
    b2_d = nc.dram_tensor("b2", [E], F32, kind="ExternalInput")
    kb_d = nc.dram_tensor("kbt", [P, HP], F32, kind="ExternalInput")
    qb_d = nc.dram_tensor("qbt", [P, HP], F32, kind="ExternalInput")
    mb_d = nc.dram_tensor("mb", [24], F32, kind="ExternalInput")
    msk_d = nc.dram_tensor("msk", [4, P, 1024], BF16, kind="ExternalInput")
    out_d = nc.dram_tensor("out", [NQ, E], F32, kind="ExternalOutput")

    QSCALE = float(HD) ** -0.5

    def rstd_quake(nc, small, var_ap, tag):
        """1/sqrt(var+eps) on the DVE only ([P,1] tiles, quake + 1 Newton).
        Used mid-attention (LN2) to avoid scalar activation-table switches."""
        a = small.tile([P, 1], F32, tag=tag + "_a")
        nc.vector.tensor_scalar(out=a, in0=var_ap, scalar1=EPS, scalar2=None,
                                op0=ALU.add)
        s1 = small.tile([P, 1], U32, tag=tag + "_s")
        nc.vector.tensor_scalar(out=s1, in0=a.bitcast(U32), scalar1=1,
                                scalar2=None, op0=ALU.logical_shift_right)
        y0b = small.tile([P, 1], U32, tag=tag + "_y0b")
        nc.vector.tensor_tensor(out=y0b, in0=_QCONST[0], in1=s1,
                                op=ALU.subtract)
        y = y0b.bitcast(F32)
        for it in range(2):
            t2 = small.tile([P, 1], F32, tag=tag + f"_u{it}")
            nc.vector.tensor_tensor(out=t2, in0=y, in1=y, op=ALU.mult)
            t3 = small.tile([P, 1], F32, tag=tag + f"_v{it}")
            nc.vector.scalar_tensor_tensor(out=t3, in0=t2, scalar=-0.5,
                                           in1=a, op0=ALU.mult, op1=ALU.mult)
            u = small.tile([P, 1], F32, tag=tag + f"_x{it}")
            nc.vector.tensor_scalar(out=u, in0=t3, scalar1=1.5, scalar2=None,
                                    op0=ALU.add)
            yn = small.tile([P, 1], F32, tag=tag + f"_w{it}")
            nc.vector.tensor_tensor(out=yn, in0=y, in1=u, op=ALU.mult)
            y = yn
        return y

    def layernorm_T(nc, small, acts, x_ap, dstT, sc, tag, quake, psum):
        """LN of one [128,E] fp32 row-tile -> bf16 transpose into
        dstT[:, :, sc*128:(sc+1)*128] via the PE array (the DMA xbar
        transpose measures ~10us per strided [128,1024] tile - unusable).
        rstd comes from the DVE quake chain (no activation-table churn);
        the normalize apply runs on the scalar engine (Identity is in
        every table set)."""
        stats = small.tile([P, 2, 6], F32, tag=tag + "_bn")
        for g in range(2):
            nc.vector.bn_stats(out=stats[:, g, :], in_=x_ap[:, g * 512:(g + 1) * 512])
        mv = small.tile([P, 2], F32, tag=tag + "_mv")
        nc.vector.bn_aggr(out=mv, in_=stats)
        rstd = rstd_quake(nc, small, mv[:, 1:2], tag)
        nm = small.tile([P, 1], F32, tag=tag + "_nm")
        nc.vector.scalar_tensor_tensor(out=nm, in0=mv[:, 0:1], scalar=-1.0,
                                       in1=rstd, op0=ALU.mult, op1=ALU.mult)
        tmp = acts.tile([P, E], BF16, tag=tag + "_tmp")
        nc.scalar.activation(out=tmp, in_=x_ap, func=AF.Identity,
                             bias=nm, scale=rstd)
        for ec in range(EC):
            tp = psum.tile([P, P], BF16, tag="mm", name="tp")
            nc.tensor.transpose(tp, tmp[:, ts(ec, P)], _IDENT[0])
            nc.vector.tensor_copy(out=dstT[:, ec, ts(sc, P)], in_=tp)

    with tile.TileContext(nc) as tc:
        import contextlib
        stk = contextlib.ExitStack()
        with stk:
            const = stk.enter_context(tc.tile_pool(name="const", bufs=1))
            small = stk.enter_context(tc.tile_pool(name="small", bufs=4))
            dram = stk.enter_context(tc.tile_pool(name="dram", bufs=1, space="DRAM"))

            qconst = const.tile([P, 1], U32)
            nc.vector.memset(qconst, 0x5F3759DF)
            ident = const.tile([P, P], BF16)
            make_identity(nc, ident)
            global _IDENT
            _IDENT = [ident]
            eps_t = const.tile([P, 1], F32)
            nc.vector.memset(eps_t, EPS)
            global _QCONST, _EPS_T
            _QCONST = [qconst]
            _EPS_T = [eps_t]

            mb_sb = const.tile([P, 24], F32)
            nc.gpsimd.dma_start(out=mb_sb, in_=mb_d[None, :].to_broadcast((P, 24)))
            b1_sb = const.tile([P, FC], F32)
            nc.gpsimd.dma_start(out=b1_sb, in_=b1_d[:, :])
            bp_sb = const.tile([P, E], F32)
            nc.gpsimd.dma_start(out=bp_sb, in_=bp_d[None, :].to_broadcast((P, E)))
            b2_sb = const.tile([P, E], F32)
            nc.gpsimd.dma_start(out=b2_sb, in_=b2_d[None, :].to_broadcast((P, E)))
            kb_sb = const.tile([P, HP], F32)
            nc.gpsimd.dma_start(out=kb_sb, in_=kb_d[:, :])
            qb_sb = const.tile([P, HP], F32)
            nc.gpsimd.dma_start(out=qb_sb, in_=qb_d[:, :])
            msk = const.tile([P, 4, 1024], BF16)
            nc.sync.dma_start(out=msk, in_=msk_d.rearrange("k p q -> p k q"))

            out1_dram = dram.tile([NQ, E], F32)
            # V streamed through DRAM: [ch, t, h, 64] values + ones col 64
            V_dram = dram.tile([NCH, P, H, HD + 1], BF16)

            # persistent across attention + tail (zT until tail proj, ln2T
            # until tail FFN1)
            big = stk.enter_context(tc.tile_pool(name="big", bufs=1))
            zT = big.tile([P, EC, NQ], BF16)
            ln2T = big.tile([P, EC, NQ], BF16)

            xstream = stk.enter_context(tc.tile_pool(name="xstream", bufs=3))
            acts = stk.enter_context(tc.tile_pool(name="acts", bufs=2))

            # ---------------- phase A/B: LN1 + V (interleaved) -----------
            # lnT + wvt live on the RIGHT side: freed mid-program while the
            # left-side attention pools are still open (strict LIFO per side).
            lnT_stk = contextlib.ExitStack()
            lnT_pool = lnT_stk.enter_context(
                tc.tile_pool(name="lnT_pool", bufs=1, side="right"))
            lnT = lnT_pool.tile([P, EC, S], BF16)
            wvt = lnT_pool.tile([P, EC, E], BF16)
            nc.gpsimd.dma_start(out=wvt, in_=wv_d.rearrange("ec e n -> e ec n"))

            attn_stk = contextlib.ExitStack()
            kq_pool = attn_stk.enter_context(tc.tile_pool(name="kq_pool", bufs=1))
            KT = [kq_pool.tile([P, S], BF16, name=f"KT{i}") for i in range(HP)]
            QT = [kq_pool.tile([P, NQ], BF16, name=f"QT{i}") for i in range(HP)]
            wstream = attn_stk.enter_context(tc.tile_pool(name="wstream", bufs=2))

            # deeper psum ring for the LN1/V/KQ phase (closed before the
            # attention psum pools open)
            pre_stk = contextlib.ExitStack()
            pre_psum = pre_stk.enter_context(
                tc.tile_pool(name="pre_psum", bufs=6, space="PSUM"))

            # All 16 LN1 chains first: their DVE/scalar/DMA hops pipeline
            # across tiles.  Emitting V right after its own LN tile would
            # park V's psum-evac copy at the DVE queue head, blocking the
            # next tile's bn_stats on this tile's full LN->transpose->matmul
            # latency chain.
            def v_chunk(ch):
                for half in range(2):
                    pv = pre_psum.tile([P, 512], F32, tag="mm")
                    for ec in range(EC):
                        nc.tensor.matmul(pv, lnT[:, ec, ts(ch, P)],
                                         wvt[:, ec, ts(half, 512)],
                                         start=(ec == 0), stop=(ec == EC - 1))
                    vsb = acts.tile([P, 8, HD + 1], BF16, tag="vsb")
                    nc.vector.memset(vsb[:, :, HD:HD + 1], 1.0)
                    nc.vector.tensor_copy(
                        out=vsb[:, :, 0:HD],
                        in_=pv.rearrange("p (h d) -> p h d", d=HD))
                    nc.sync.dma_start(
                        out=V_dram[ch, :, 8 * half:8 * (half + 1), :], in_=vsb)

            def load_x(sc):
                xt = xstream.tile([P, E], F32, tag="x")
                nc.sync.dma_start(out=xt, in_=xp_d[ts(sc, P), :])
                return xt
            xtiles = [load_x(sc) for sc in range(3)]
            for sc in range(S // P):
                layernorm_T(nc, small, acts, xtiles[sc], lnT, sc, "ln1",
                            quake=True, psum=pre_psum)
                if sc + 3 < S // P:
                    xtiles.append(load_x(sc + 3))
                if sc >= 4:
                    v_chunk(sc - 4)

            for ch in range(NCH - 4, NCH):
                v_chunk(ch)

            # ---- K/Q for one head pair (6 psum chains of 8 matmuls) -----
            def load_w(hp):
                wkt = wstream.tile([P, EC, P], BF16, tag="wk")
                nc.gpsimd.dma_start(out=wkt, in_=wk_d[hp].rearrange("ec e d -> e ec d"))
                wqt = wstream.tile([P, EC, P], BF16, tag="wq")
                nc.gpsimd.dma_start(out=wqt, in_=wq_d[hp].rearrange("ec e d -> e ec d"))
                return wkt, wqt

            def emit_kq_group(hp, wkt, wqt, kind, seg, psum):
                pk = psum.tile([P, 512], F32, tag="mm", name="pk")
                wt = wkt if kind == "k" else wqt
                for ec in range(EC):
                    nc.tensor.matmul(pk, wt[:, ec], lnT[:, ec, ts(seg, 512)],
                                     start=(ec == 0), stop=(ec == EC - 1))
                if kind == "k":
                    nc.vector.scalar_tensor_tensor(
                        out=KT[hp][:, ts(seg, 512)], in0=pk, scalar=1.0,
                        in1=kb_sb[:, hp:hp + 1].to_broadcast((P, 512)),
                        op0=ALU.mult, op1=ALU.add)
                else:
                    nc.vector.scalar_tensor_tensor(
                        out=QT[hp][:, ts(seg, 512)], in0=pk, scalar=QSCALE,
                        in1=qb_sb[:, hp:hp + 1].to_broadcast((P, 512)),
                        op0=ALU.mult, op1=ALU.add)

            KQ_GROUPS = [("k", 0), ("k", 1), ("k", 2), ("k", 3),
                         ("q", 0), ("q", 1)]
            for hp in (0, 1):
                w = load_w(hp)
                for kind, seg in KQ_GROUPS:
                    emit_kq_group(hp, w[0], w[1], kind, seg, pre_psum)
            pre_stk.close()
            mm_psum = attn_stk.enter_context(
                tc.tile_pool(name="mm_psum", bufs=2, space="PSUM"))

            # ---------------- attention --------------------------------
            st_psum = attn_stk.enter_context(
                tc.tile_pool(name="st_psum", bufs=2, space="PSUM"))
            z_psum = attn_stk.enter_context(
                tc.tile_pool(name="z_psum", bufs=2, space="PSUM"))
            p_pool = attn_stk.enter_context(tc.tile_pool(name="p_pool", bufs=4))
            v_pool = attn_stk.enter_context(tc.tile_pool(name="v_pool", bufs=4))
            l_pool = attn_stk.enter_context(tc.tile_pool(name="l_pool", bufs=2))

            # FFN-phase pools (opened before j=1 so proj/FFN1 can interleave)
            ffn_stk = contextlib.ExitStack()

            def attn_tile(j, hp, vt, extra_work):
                """One (q-tile, head-pair): scores -> exp -> z, with
                extra_work[(slot_idx)] emitted into the exp gaps."""
                sched = SCHED[j]
                zp = [z_psum.tile([P, 512], F32, tag="z", name=f"zp{h}")
                      for h in range(2)]
                pend = None

                def flush_z(pend):
                    pt_, vi_, off_, first_, last_ = pend
                    vtile = vt[vi_ // 8]
                    for h in range(2):
                        nc.tensor.matmul(zp[h][0:HD + 1, off_:512],
                                         vtile[:, vi_ % 8, h],
                                         pt_[:, h, off_:512],
                                         start=first_, stop=last_)

                for ci, ch in enumerate(sched):
                    st = st_psum.tile([P, 1024], F32, tag="st")
                    nc.tensor.matmul(
                        st[:, 0:512], KT[hp][0:HD, ts(ch, P)],
                        QT[hp][0:HD, ts(j, 512)],
                        start=True, stop=True, tile_position=(0, 0))
                    nc.tensor.matmul(
                        st[:, 512:1024], KT[hp][HD:P, ts(ch, P)],
                        QT[hp][HD:P, ts(j, 512)],
                        start=True, stop=True, tile_position=(HD, 0))
                    slot = (8 if j else 0) + ci
                    pt = p_pool.tile([P, 2, 512], BF16, tag="p")
                    nc.scalar.activation(
                        out=pt.rearrange("p a b -> p (a b)"), in_=st,
                        func=AF.Exp, bias=mb_sb[:, slot:slot + 1], scale=1.0)
                    off = 0
                    if ch in DIAG[j]:
                        k = ch - 4 * j
                        off = P * k
                        nc.vector.tensor_tensor(
                            out=pt[:, :, off:512], in0=pt[:, :, off:512],
                            in1=msk[:, k:k + 1, off:512].to_broadcast(
                                (P, 2, 512 - off)),
                            op=ALU.mult)
                    if ci in extra_work:
                        extra_work[ci]()
                    if pend is not None:
                        flush_z(pend)
                    pend = (pt, ci, off, ci == 0, ci == len(sched) - 1)
                flush_z(pend)

                # evacuate z with fused 1/l.  DVE reciprocal is an 8-slice
                # iterative divide (~7 cyc/elem along the free dim), so run
                # it on a [128,4] reshape of the denominator row (DMA
                # round-trip keeps the latency off the compute queues).
                for h in range(2):
                    dst_p = h * HD
                    lcp = l_pool.tile([1, 512], F32, tag="lcp")
                    nc.vector.tensor_copy(out=lcp, in_=zp[h][HD:HD + 1, :])
                    lr = l_pool.tile([P, 4], F32, tag="lr")
                    nc.sync.dma_start(out=lr, in_=lcp)
                    li = l_pool.tile([P, 4], F32, tag="li")
                    nc.vector.reciprocal(out=li, in_=lr)
                    lrow = l_pool.tile([1, 512], F32, tag="lrow")
                    nc.sync.dma_start(out=lrow, in_=li)
                    lb = l_pool.tile([P, 512], F32, tag="lb")
                    nc.gpsimd.partition_broadcast(lb, lrow)
                    nc.vector.tensor_tensor(
                        out=zT[dst_p:dst_p + HD, hp, ts(j, 512)],
                        in0=zp[h][0:HD, :], in1=lb[0:HD, :],
                        op=ALU.mult)

            def load_vt(j, hp):
                """V for the (j, hp) slot list, as 8-chunk tiles."""
                hs = slice(2 * hp, 2 * hp + 2)
                tiles = []
                if j == 0:
                    vt = v_pool.tile([P, 8, 2, HD + 1], BF16, tag="vt")
                    nc.sync.dma_start(
                        out=vt[:, 0:4],
                        in_=V_dram[0:4, :, hs, :].rearrange("c t h d -> t c h d"))
                    nc.sync.dma_start(
                        out=vt[:, 4:8],
                        in_=V_dram[8:12, :, hs, :].rearrange("c t h d -> t c h d"))
                    tiles.append(vt)
                else:
                    for g in range(2):
                        vt = v_pool.tile([P, 8, 2, HD + 1], BF16, tag="vt")
                        nc.sync.dma_start(
                            out=vt,
                            in_=V_dram[8 * g:8 * g + 8, :, hs, :].rearrange(
                                "c t h d -> t c h d"))
                        tiles.append(vt)
                return tiles

            # ---- j = 0: interleave the remaining head-pairs' K/Q --------
            vt_next = load_vt(0, 0)
            for hp in range(HP):
                vt = vt_next
                if hp + 1 < HP:
                    vt_next = load_vt(0, hp + 1)
                extra = {}
                if hp + 2 < HP:
                    w = load_w(hp + 2)
                    for gi, (kind, seg) in enumerate(KQ_GROUPS):
                        slot = gi + 1
                        extra[slot] = (lambda k_=kind, s_=seg, w_=w, h_=hp + 2:
                                       emit_kq_group(h_, w_[0], w_[1], k_, s_,
                                                     mm_psum))
                attn_tile(0, hp, vt, extra)

            # lnT no longer needed (all K/Q/V computed)
            lnT_stk.close()

            # open FFN pools now that lnT is freed (right side: the left
            # attention pools close later, LIFO is per (space, side))
            wp_pool = ffn_stk.enter_context(
                tc.tile_pool(name="wp_pool", bufs=1, side="right"))
            w1s_pool = ffn_stk.enter_context(
                tc.tile_pool(name="w1s", bufs=2, side="right"))
            a0_pool = ffn_stk.enter_context(
                tc.tile_pool(name="a0_pool", bufs=1, side="right"))
            res = ffn_stk.enter_context(
                tc.tile_pool(name="res", bufs=2, side="right"))

            wpt = wp_pool.tile([P, EC, E], BF16)
            nc.gpsimd.dma_start(out=wpt, in_=wp_d.rearrange("dc d e -> d dc e"))
            # FFN hidden activations, one half-tile of queries each
            # (a_half[1] is allocated later, in the tail's pool)
            a_half = [a0_pool.tile([P, FC, 512], BF16, name="a0")]

            def proj_ln2(qc, psum):
                xo = xstream.tile([P, E], F32, tag="x")
                nc.gpsimd.dma_start(out=xo, in_=xp_d[ts(qc, P), :])
                o1 = res.tile([P, E], F32, tag="o1")
                for half in range(2):
                    po = psum.tile([P, 512], F32, tag="mm")
                    for dc in range(EC):
                        nc.tensor.matmul(po, zT[:, dc, ts(qc, P)],
                                         wpt[:, dc, ts(half, 512)],
                                         start=(dc == 0), stop=(dc == EC - 1))
                    nc.vector.scalar_tensor_tensor(
                        out=o1[:, ts(half, 512)], in0=po, scalar=1.0,
                        in1=xo[:, ts(half, 512)], op0=ALU.mult, op1=ALU.add)
                    nc.vector.tensor_tensor(
                        out=o1[:, ts(half, 512)], in0=o1[:, ts(half, 512)],
                        in1=bp_sb[:, ts(half, 512)], op=ALU.add)
                nc.gpsimd.dma_start(out=out1_dram[ts(qc, P), :], in_=o1)
                layernorm_T(nc, small, acts, o1, ln2T, qc, "ln2", quake=True,
                            psum=psum)

            def ffn1(fc, half, psum):
                w1t = w1s_pool.tile([P, EC, P], BF16, tag="w1c")
                nc.gpsimd.dma_start(out=w1t, in_=w1_d[fc].rearrange("ec e f -> e ec f"))
                pa = psum.tile([P, 512], F32, tag="mm")
                for ec in range(EC):
                    nc.tensor.matmul(pa, w1t[:, ec], ln2T[:, ec, ts(half, 512)],
                                     start=(ec == 0), stop=(ec == EC - 1))
                nc.scalar.activation(out=a_half[half][:, fc, :], in_=pa,
                                     func=AF.Relu, bias=b1_sb[:, fc:fc + 1],
                                     scale=1.0)

            # ---- j = 1: interleave proj/LN2 (tile0) and FFN1 (tile0) ----
            vt_next = load_vt(1, 0)
            for hp in range(HP):
                vt = vt_next
                if hp + 1 < HP:
                    vt_next = load_vt(1, hp + 1)
                extra = {}
                if 1 <= hp <= 4:
                    extra[2] = (lambda q_=hp - 1: proj_ln2(q_, mm_psum))
                elif hp >= 5:
                    counts = {5: (0, 5), 6: (5, 5), 7: (10, 6)}
                    base, cnt = counts[hp]
                    for gi in range(cnt):
                        extra[2 * gi + 1] = (
                            lambda f_=base + gi: ffn1(f_, 0, mm_psum))
                attn_tile(1, hp, vt, extra)

            attn_stk.close()

            # ---------------- tail: proj t1, FFN1 rest, FFN2 -------------
            tail_psum = ffn_stk.enter_context(
                tc.tile_pool(name="tail_psum", bufs=4, space="PSUM"))
            w2_pool = ffn_stk.enter_context(
                tc.tile_pool(name="w2_pool", bufs=1, side="right"))
            res2 = ffn_stk.enter_context(
                tc.tile_pool(name="res2", bufs=2, side="right"))
            w2t = w2_pool.tile([P, FC, E], BF16)
            a_half.append(w2_pool.tile([P, FC, 512], BF16, name="a1"))

            def ffn2(qc):
                o1r = res2.tile([P, E], F32, tag="o1r")
                nc.gpsimd.dma_start(out=o1r, in_=out1_dram[ts(qc, P), :])
                fin = res2.tile([P, E], F32, tag="fin")
                a_sb = a_half[qc // 4]
                for half in range(2):
                    pf = tail_psum.tile([P, 512], F32, tag="mm")
                    for fc in range(FC):
                        nc.tensor.matmul(pf, a_sb[:, fc, ts(qc % 4, P)],
                                         w2t[:, fc, ts(half, 512)],
                                         start=(fc == 0), stop=(fc == FC - 1))
                    nc.vector.scalar_tensor_tensor(
                        out=fin[:, ts(half, 512)], in0=pf, scalar=1.0,
                        in1=o1r[:, ts(half, 512)], op0=ALU.mult, op1=ALU.add)
                    nc.vector.tensor_tensor(
                        out=fin[:, ts(half, 512)], in0=fin[:, ts(half, 512)],
                        in1=b2_sb[:, ts(half, 512)], op=ALU.add)
                nc.sync.dma_start(out=out_d[ts(qc, P), :], in_=fin)

            for qc in range(4, QC):
                proj_ln2(qc, tail_psum)
            nc.gpsimd.dma_start(out=w2t, in_=w2_d.rearrange("fc f e -> f fc e"))
            for qc in range(0, 4):
                ffn2(qc)
            for fc in range(FC):
                ffn1(fc, 1, tail_psum)
            for qc in range(4, QC):
                ffn2(qc)

            ffn_stk.close()

    nc.compile()
    return nc


def _perms():
    a_own = np.concatenate([np.arange(0, 512), np.arange(1536, 2048)])
    a_rest = np.arange(512, 1536)
    b_own = np.arange(512, 1536)
    b_rest = np.concatenate([np.arange(0, 512), np.arange(1536, 2048)])
    return [np.concatenate([a_own, a_rest]), np.concatenate([b_own, b_rest])], \
           [a_own, b_own]


def _mask_bias():
    mb = [np.zeros(24, np.float32), np.zeros(24, np.float32)]
    mb[0][4:8] = NEG     # role A, tile0, chunks 8-11 (future keys)
    mb[1][20:24] = NEG   # role B, tile1, chunks 12-15 (future keys)
    return mb


def _tri_masks():
    """msk[k][p, h*512+q] = 1 if q >= p + 128k else 0 (both head halves)."""
    m = np.zeros((4, P, 1024), np.float32)
    q = np.arange(512)
    p = np.arange(P)
    for k in range(4):
        keep = (q[None, :] >= p[:, None] + P * k).astype(np.float32)
        m[k, :, 0:512] = keep
        m[k, :, 512:1024] = keep
    return m.astype(ml_dtypes.bfloat16)


def _prep_shared(wq, wk, wv, w_proj, b_proj, w1, b1, w2, b2,
                 ln1_g, ln1_b, ln2_g, ln2_b):
    bf = ml_dtypes.bfloat16
    f32 = np.float32
    wq = np.asarray(wq, f32) * np.asarray(ln1_g, f32)[None, :, None]
    wk = np.asarray(wk, f32) * np.asarray(ln1_g, f32)[None, :, None]
    wv = np.asarray(wv, f32) * np.asarray(ln1_g, f32)[None, :, None]
    w1 = np.asarray(w1, f32) * np.asarray(ln2_g, f32)[:, None]

    qb = np.einsum("e,hed->hd", np.asarray(ln1_b, f32), wq) * (HD ** -0.5)
    kb = np.einsum("e,hed->hd", np.asarray(ln1_b, f32), wk)
    vb = np.einsum("e,hed->hd", np.asarray(ln1_b, f32), wv)  # [H, HD]
    bp_eff = np.asarray(b_proj, f32) + vb.reshape(-1) @ np.asarray(w_proj, f32)
    b1_eff = np.asarray(b1, f32) + np.asarray(ln2_b, f32) @ w1

    def pack_pair(w):  # [H, E, HD] -> [H/2, E/P, P, P] bf16
        wpair = w.reshape(H // 2, 2, E, HD)
        cat = np.concatenate([wpair[:, 0], wpair[:, 1]], axis=-1)  # [H/2,E,128]
        return np.ascontiguousarray(cat.reshape(H // 2, E // P, P, P)).astype(bf)

    def pack_bias(b):  # [H, HD] -> [P, HP]  (h0|h1 stacked per pair)
        return np.ascontiguousarray(
            b.reshape(HP, 2 * HD).T).astype(f32)

    shared = {
        "wq2": pack_pair(wq),
        "wk2": pack_pair(wk),
        "wv": np.ascontiguousarray(
            wv.transpose(1, 0, 2).reshape(E // P, P, E)).astype(bf),
        "wp": np.ascontiguousarray(
            np.asarray(w_proj, f32).reshape(E // P, P, E)).astype(bf),
        "w2": np.ascontiguousarray(
            np.asarray(w2, f32).reshape(FF // P, P, E)).astype(bf),
        "b1t": np.ascontiguousarray(b1_eff.reshape(FF // P, P).T).astype(f32),
        "bproj": bp_eff.astype(f32),
        "b2": np.asarray(b2, f32),
        "kbt": pack_bias(kb),
        "qbt": pack_bias(qb),
        "msk": _tri_masks(),
    }
    # w1 layout: [FC, EC, P(e), P(f)]: chunk fc holds w1[e, fc*128+f]
    w1r = w1.reshape(E // P, P, FF // P, P)           # [ec, pe, fc, pf]
    shared["w1"] = np.ascontiguousarray(
        w1r.transpose(2, 0, 1, 3)).astype(bf)          # [fc, ec, pe, pf]
    return shared


def make_in_maps(x, **weights):
    """Build the 8 per-core input dicts (and the gather info)."""
    shared = _prep_shared(**weights)
    perms, owns = _perms()
    mbs = _mask_bias()
    in_maps = []
    for c in range(8):
        b, r = c // 2, c % 2
        m = dict(shared)
        m["xp"] = np.ascontiguousarray(np.asarray(x, np.float32)[b][perms[r]])
        m["mb"] = mbs[r]
        in_maps.append(m)
    return in_maps, owns


def get_nc():
    if "nc" not in _CACHE:
        _CACHE["nc"] = _build_program()
    return _CACHE["nc"]


def kernel(x, wq, wk, wv, w_proj, b_proj, w1, b1, w2, b2,
           ln1_g, ln1_b, ln2_g, ln2_b):
    x = np.asarray(x, dtype=np.float32)
    weights = dict(wq=np.asarray(wq), wk=np.asarray(wk), wv=np.asarray(wv),
                   w_proj=np.asarray(w_proj), b_proj=np.asarray(b_proj),
                   w1=np.asarray(w1), b1=np.asarray(b1), w2=np.asarray(w2),
                   b2=np.asarray(b2), ln1_g=np.asarray(ln1_g),
                   ln1_b=np.asarray(ln1_b), ln2_g=np.asarray(ln2_g),
                   ln2_b=np.asarray(ln2_b))
    nc = get_nc()
    in_maps, owns = make_in_maps(x, **weights)
    res = run_bass_kernel_spmd(nc, in_maps, core_ids=list(range(8)))
    out = np.empty((4, S, E), dtype=np.float32)
    for c in range(8):
        b, r = c // 2, c % 2
        out[b][owns[r]] = res.results[c]["out"]
    return out


# revision 35
# speedup vs baseline: 1.1664x; 1.0429x over previous
"""Trainium2 Bass kernel for a dense pre-LN transformer block.

Block: y = x + proj(causal_mha(LN1(x))) ; out = y + FFN(LN2(y))
Shapes (hardcoded): x [4, 2048, 1024], H=16 heads, HD=64, FF=2048, fp32 I/O.

Sharding (8 cores, no collectives): core c handles batch b=c//2 and a
balanced half of the queries (role r=c%2; A: rows [0,512)+[1536,2048),
B: rows [512,1536)).  The key/value sequence is permuted on the host per
core (own rows first) so one SPMD program serves both roles; causality is
enforced by compile-time triangular affine_select masks on the diagonal
chunks plus a per-core exp-bias table (-100 => exp ~ 0) for the chunks
whose validity depends on the role.

Matmuls run in bf16 (fp32 PSUM accumulate); layernorm stats, softmax and
residuals stay fp32.  Scores are computed transposed (st[t,q]) so softmax
needs no transposes; V carries an extra ones-column so the softmax
denominator drops out of the z-matmul for free.
"""

import numpy as np
import ml_dtypes

import concourse.bass as bass
import concourse.bacc as bacc
import concourse.tile as tile
import concourse.mybir as mybir
from concourse.bass import ts
from concourse.bass_utils import run_bass_kernel_spmd
from concourse.masks import make_identity

BF16 = mybir.dt.bfloat16
F32 = mybir.dt.float32
AF = mybir.ActivationFunctionType
ALU = mybir.AluOpType

S = 2048          # sequence length
E = 1024          # embedding dim
H = 16            # heads
HD = 64           # head dim
FF = 2048         # ffn hidden
P = 128           # partitions
NQ = 1024         # queries owned per core
EPS = 1e-5
NEG = -100.0      # exp bias for masked-out chunks (exp(-100) ~ 0)

# chunk schedule (in permuted key coordinates), identical on every core:
# q-tile 0 (own positions [0,512)):   key chunks 0-3 (diag) + 8-11 (role-dep)
# q-tile 1 (own positions [512,1024)): key chunks 0-15 (4-7 diag, 12-15 role-dep)
SCHED = [[0, 1, 2, 3, 8, 9, 10, 11], list(range(16))]
DIAG = [set(range(0, 4)), set(range(4, 8))]

_CACHE = {}


def _build_program():
    nc = bacc.Bacc("TRN2", target_bir_lowering=False, debug=False)

    # ---- per-core dram inputs -------------------------------------------
    xp_d = nc.dram_tensor("xp", [S, E], F32, kind="ExternalInput")
    wq_d = nc.dram_tensor("wq2", [H // 2, E // P, P, P], BF16, kind="ExternalInput")
    wk_d = nc.dram_tensor("wk2", [H // 2, E // P, P, P], BF16, kind="ExternalInput")
    wv_d = nc.dram_tensor("wv", [E // P, P, E], BF16, kind="ExternalInput")
    wp_d = nc.dram_tensor("wp", [E // P, P, E], BF16, kind="ExternalInput")
    w1_d = nc.dram_tensor("w1", [E // P, P, FF], BF16, kind="ExternalInput")
    w2_d = nc.dram_tensor("w2", [FF // P, P, E], BF16, kind="ExternalInput")
    b1_d = nc.dram_tensor("b1t", [P, FF // P], F32, kind="ExternalInput")
    bp_d = nc.dram_tensor("bproj", [E], F32, kind="ExternalInput")
    b2_d = nc.dram_tensor("b2", [E], F32, kind="ExternalInput")
    kb_d = nc.dram_tensor("kbt", [P, H // 2], F32, kind="ExternalInput")
    qb_d = nc.dram_tensor("qbt", [P, H // 2], F32, kind="ExternalInput")
    mb_d = nc.dram_tensor("mb", [24], F32, kind="ExternalInput")
    out_d = nc.dram_tensor("out", [NQ, E], F32, kind="ExternalOutput")

    EC = E // P    # 8 e-chunks
    FC = FF // P   # 16 f-chunks
    NCH = S // P   # 16 key chunks
    HP = H // 2    # 8 head pairs

    def layernorm_to_T(tc, pools, x_ap, sc, dstT, act_pool, tp_psum):
        """LN of one [128, E] row-tile (fp32 in SBUF/psum-readable AP) then
        transpose to dstT[:, ec, sc*128:(sc+1)*128] (bf16).  gamma/beta are
        folded into the weights/biases on the host, so the evacuation is a
        plain copy."""
        nc_ = tc.nc
        small = pools["small"]
        stats = small.tile([P, 2, 6], F32, tag="bnstats")
        for g in range(2):
            nc_.vector.bn_stats(out=stats[:, g, :], in_=x_ap[:, g * 512:(g + 1) * 512])
        mv = small.tile([P, 2], F32, tag="bnaggr")
        nc_.vector.bn_aggr(out=mv, in_=stats)
        std = small.tile([P, 1], F32, tag="std")
        nc_.scalar.activation(out=std, in_=mv[:, 1:2], func=AF.Sqrt,
                              bias=pools["eps"], scale=1.0)
        rstd = small.tile([P, 1], F32, tag="rstd")
        nc_.vector.reciprocal(out=rstd, in_=std)
        nm = small.tile([P, 1], F32, tag="negmean")
        nc_.vector.scalar_tensor_tensor(out=nm, in0=mv[:, 0:1], scalar=-1.0,
                                        in1=rstd, op0=ALU.mult, op1=ALU.mult)
        tmp = act_pool.tile([P, E], BF16, tag="ln_tmp")
        nc_.scalar.activation(out=tmp, in_=x_ap, func=AF.Identity,
                              bias=nm, scale=rstd)
        for ec in range(EC):
            tp = tp_psum.tile([P, P], BF16, tag="tp")
            nc_.tensor.transpose(tp, tmp[:, ts(ec, P)], pools["ident"])
            nc_.vector.tensor_copy(out=dstT[:, ec, ts(sc, P)], in_=tp)

    with tile.TileContext(nc) as tc:
        import contextlib
        stk = contextlib.ExitStack()
        with stk:
            const = stk.enter_context(tc.tile_pool(name="const", bufs=1))
            small = stk.enter_context(tc.tile_pool(name="small", bufs=4))
            dram = stk.enter_context(tc.tile_pool(name="dram", bufs=1, space="DRAM"))

            ident = const.tile([P, P], BF16)
            make_identity(nc, ident)
            eps_t = const.tile([P, 1], F32)
            nc.vector.memset(eps_t, EPS)
            mb_sb = const.tile([P, 24], F32)
            nc.gpsimd.dma_start(out=mb_sb, in_=mb_d[None, :].to_broadcast((P, 24)))
            b1_sb = const.tile([P, FC], F32)
            nc.sync.dma_start(out=b1_sb, in_=b1_d[:, :])
            bp_sb = const.tile([P, E], F32)
            nc.gpsimd.dma_start(out=bp_sb, in_=bp_d[None, :].to_broadcast((P, E)))
            b2_sb = const.tile([P, E], F32)
            nc.gpsimd.dma_start(out=b2_sb, in_=b2_d[None, :].to_broadcast((P, E)))
            kb_sb = const.tile([P, HP], F32)
            nc.gpsimd.dma_start(out=kb_sb, in_=kb_d[:, :])
            qb_sb = const.tile([P, HP], F32)
            nc.gpsimd.dma_start(out=qb_sb, in_=qb_d[:, :])
            msk = const.tile([P, 4, 512], BF16)
            for k in range(4):
                nc.gpsimd.memset(msk[:, k, :], 1.0)
                nc.gpsimd.affine_select(
                    out=msk[:, k, :], in_=msk[:, k, :], compare_op=ALU.is_ge,
                    fill=0.0, base=-P * k, channel_multiplier=-1,
                    pattern=[[1, 512]])
            pools = {"ident": ident, "eps": eps_t, "small": small}

            out1_dram = dram.tile([NQ, E], F32)
            # V streamed through DRAM: [ch, t, h, 64] values + ones col 64
            V_dram = dram.tile([NCH, P, H, HD + 1], BF16)

            # late-phase persistent buffers (allocated first = bottom of stack)
            late = stk.enter_context(tc.tile_pool(name="late", bufs=1))
            ln2T = late.tile([P, EC, NQ], BF16)
            zT = late.tile([P, EC, NQ], BF16)

            with tc.tile_pool(name="attn", bufs=1) as attn:

                KT = [attn.tile([P, S], BF16, name=f"KT{i}") for i in range(HP)]
                QT = [attn.tile([P, NQ], BF16, name=f"QT{i}") for i in range(HP)]

                with tc.tile_pool(name="lnT_pool", bufs=1) as lnT_pool, \
                     tc.tile_pool(name="xstream", bufs=3) as xstream, \
                     tc.tile_pool(name="acts", bufs=3) as acts, \
                     tc.tile_pool(name="wstream", bufs=2) as wstream, \
                     tc.tile_pool(name="mm_psum", bufs=2, space="PSUM") as mm_psum:
                    lnT = lnT_pool.tile([P, EC, S], BF16)

                    # ---- phase 1: LN1 over all rows -> lnT [e, s] -------
                    with tc.tile_pool(name="tp_psum", bufs=2,
                                      space="PSUM") as tp_psum:
                        for sc in range(S // P):
                            xt = xstream.tile([P, E], F32, tag="x")
                            nc.sync.dma_start(out=xt, in_=xp_d[ts(sc, P), :])
                            layernorm_to_T(tc, pools, xt, sc, lnT, acts,
                                           tp_psum)

                    # ---- phase 3: V (all heads) -> DRAM -----------------
                    with tc.tile_pool(name="wv_pool", bufs=1) as wv_pool:
                        wvt = wv_pool.tile([P, EC, E], BF16)
                        nc.gpsimd.dma_start(
                            out=wvt, in_=wv_d[:, :, :].rearrange("ec e n -> e ec n"))
                        for ch in range(NCH):
                            for half in range(2):
                                pv = mm_psum.tile([P, 512], F32, tag="mm")
                                for ec in range(EC):
                                    nc.tensor.matmul(pv, lnT[:, ec, ts(ch, P)],
                                                     wvt[:, ec, ts(half, 512)],
                                                     start=(ec == 0),
                                                     stop=(ec == EC - 1))
                                vsb = acts.tile([P, 8, HD + 1], BF16, tag="vsb")
                                nc.vector.memset(vsb[:, :, HD:HD + 1], 1.0)
                                nc.vector.tensor_copy(
                                    out=vsb[:, :, 0:HD],
                                    in_=pv.rearrange("p (h d) -> p h d", d=HD))
                                nc.sync.dma_start(
                                    out=V_dram[ch, :, 8 * half:8 * (half + 1), :],
                                    in_=vsb)

                    # ---- K/Q for one head pair (6 psum groups) ----------
                    def load_w(hp):
                        wkt = wstream.tile([P, EC, P], BF16, tag="wk")
                        nc.gpsimd.dma_start(
                            out=wkt, in_=wk_d[hp].rearrange("ec e d -> e ec d"))
                        wqt = wstream.tile([P, EC, P], BF16, tag="wq")
                        nc.gpsimd.dma_start(
                            out=wqt, in_=wq_d[hp].rearrange("ec e d -> e ec d"))
                        return wkt, wqt

                    def emit_kq_group(hp, wkt, wqt, kind, seg):
                        pk = mm_psum.tile([P, 512], F32, tag="mm", name="pk")
                        wt = wkt if kind == "k" else wqt
                        for ec in range(EC):
                            nc.tensor.matmul(pk, wt[:, ec], lnT[:, ec, ts(seg, 512)],
                                             start=(ec == 0), stop=(ec == EC - 1))
                        if kind == "k":
                            nc.vector.scalar_tensor_tensor(
                                out=KT[hp][:, ts(seg, 512)], in0=pk, scalar=1.0,
                                in1=kb_sb[:, hp:hp + 1].to_broadcast((P, 512)),
                                op0=ALU.mult, op1=ALU.add)
                        else:
                            nc.vector.scalar_tensor_tensor(
                                out=QT[hp][:, ts(seg, 512)], in0=pk,
                                scalar=float(HD) ** -0.5,
                                in1=qb_sb[:, hp:hp + 1].to_broadcast((P, 512)),
                                op0=ALU.mult, op1=ALU.add)

                    KQ_GROUPS = [("k", 0), ("k", 1), ("k", 2), ("k", 3),
                                 ("q", 0), ("q", 1)]
                    w0 = load_w(0)
                    for kind, seg in KQ_GROUPS:
                        emit_kq_group(0, w0[0], w0[1], kind, seg)

                    # ---- phase 4: attention, with hp+1's K/Q matmuls ----
                    # interleaved into the exp-latency gaps ----------------
                    with tc.tile_pool(name="st_psum", bufs=2,
                                      space="PSUM") as st_psum, \
                         tc.tile_pool(name="z_psum", bufs=2,
                                      space="PSUM") as z_psum, \
                         tc.tile_pool(name="p_pool", bufs=4) as p_pool, \
                         tc.tile_pool(name="v_pool", bufs=4) as v_pool, \
                         tc.tile_pool(name="l_pool", bufs=3) as l_pool:

                        def load_vt(j, hp):
                            hs = slice(2 * hp, 2 * hp + 2)
                            vt = v_pool.tile([P, len(SCHED[j]), 2, HD + 1],
                                             BF16, tag=f"vt{j}")
                            if j == 0:
                                nc.sync.dma_start(
                                    out=vt[:, 0:4],
                                    in_=V_dram[0:4, :, hs, :].rearrange(
                                        "c t h d -> t c h d"))
                                nc.sync.dma_start(
                                    out=vt[:, 4:8],
                                    in_=V_dram[8:12, :, hs, :].rearrange(
                                        "c t h d -> t c h d"))
                            else:
                                for g in range(2):
                                    nc.sync.dma_start(
                                        out=vt[:, 8 * g:8 * g + 8],
                                        in_=V_dram[8 * g:8 * g + 8, :, hs, :]
                                        .rearrange("c t h d -> t c h d"))
                            return vt

                        vts = {(0, 0): load_vt(0, 0), (1, 0): load_vt(1, 0)}
                        for hp in range(HP):
                            kq_work = []
                            if hp + 1 < HP:
                                wn = load_w(hp + 1)
                                kq_work = list(KQ_GROUPS)
                                vts[(0, hp + 1)] = load_vt(0, hp + 1)
                                vts[(1, hp + 1)] = load_vt(1, hp + 1)
                            for j in range(2):
                                vt_t = vts.pop((j, hp))
                                zp = [z_psum.tile([P, 512], F32, tag="z",
                                                  name=f"zp{h}") for h in range(2)]
                                sched = SCHED[j]
                                pend = None

                                def flush_z(pend):
                                    pt_, vi_, off_, first_, last_ = pend
                                    for h in range(2):
                                        nc.tensor.matmul(
                                            zp[h][0:HD + 1, off_:512],
                                            vt_t[:, vi_, h],
                                            pt_[:, h, off_:512],
                                            start=first_, stop=last_)

                                for ci, ch in enumerate(sched):
                                    st = st_psum.tile([P, 1024], F32, tag="st")
                                    nc.tensor.matmul(
                                        st[:, 0:512], KT[hp][0:HD, ts(ch, P)],
                                        QT[hp][0:HD, ts(j, 512)],
                                        start=True, stop=True, tile_position=(0, 0))
                                    nc.tensor.matmul(
                                        st[:, 512:1024], KT[hp][HD:P, ts(ch, P)],
                                        QT[hp][HD:P, ts(j, 512)],
                                        start=True, stop=True, tile_position=(HD, 0))
                                    slot = (8 if j else 0) + ci
                                    pt = p_pool.tile([P, 2, 512], BF16, tag="p")
                                    nc.scalar.activation(
                                        out=pt.rearrange("p a b -> p (a b)"), in_=st,
                                        func=AF.Exp, bias=mb_sb[:, slot:slot + 1],
                                        scale=1.0)
                                    off = 0
                                    if ch in DIAG[j]:
                                        k = ch - 4 * j
                                        off = P * k
                                        nc.vector.tensor_tensor(
                                            out=pt[:, :, off:512],
                                            in0=pt[:, :, off:512],
                                            in1=msk[:, k:k + 1, off:512]
                                            .to_broadcast((P, 2, 512 - off)),
                                            op=ALU.mult)
                                    if kq_work and ci % 4 == 1:
                                        kind, seg = kq_work.pop(0)
                                        emit_kq_group(hp + 1, wn[0], wn[1],
                                                      kind, seg)
                                    if pend is not None:
                                        flush_z(pend)
                                    pend = (pt, ci, off, ci == 0,
                                            ci == len(sched) - 1)
                                flush_z(pend)

                                for h in range(2):
                                    dst_p = (h % 2) * HD
                                    zslice = zT[dst_p:dst_p + HD, hp, ts(j, 512)]
                                    lcp = l_pool.tile([1, 512], F32, tag="lcp")
                                    nc.vector.tensor_copy(
                                        out=lcp, in_=zp[h][HD:HD + 1, :])
                                    lr = l_pool.tile([P, 4], F32, tag="lr")
                                    nc.sync.dma_start(out=lr, in_=lcp)
                                    li = l_pool.tile([P, 4], F32, tag="li")
                                    nc.vector.reciprocal(out=li, in_=lr)
                                    lrow = l_pool.tile([1, 512], F32, tag="lrow")
                                    nc.sync.dma_start(out=lrow, in_=li)
                                    lb = l_pool.tile([P, 512], F32, tag="lb")
                                    nc.gpsimd.partition_broadcast(lb, lrow)
                                    nc.vector.tensor_tensor(
                                        out=zslice, in0=zp[h][0:HD, :],
                                        in1=lb[dst_p:dst_p + HD, :], op=ALU.mult)
                            for kind, seg in kq_work:   # safety flush
                                emit_kq_group(hp + 1, wn[0], wn[1], kind, seg)

            # ---- phases 5-8: proj/res1/LN2 then FFN (attn pools freed) ---
            with tc.tile_pool(name="ffnw", bufs=1) as ffnw, \
                 tc.tile_pool(name="a_pool", bufs=1) as a_pool:
                w1t = ffnw.tile([P, EC, FF], BF16)
                nc.gpsimd.dma_start(out=w1t,
                                  in_=w1_d[:, :, :].rearrange("ec e f -> e ec f"))
                w2t = ffnw.tile([P, FC, E], BF16)
                nc.gpsimd.dma_start(out=w2t,
                                  in_=w2_d[:, :, :].rearrange("fc f e -> f fc e"))
                a_sb = a_pool.tile([P, FC, NQ], BF16)

                with tc.tile_pool(name="proj_w", bufs=1) as proj_w, \
                     tc.tile_pool(name="xstream2", bufs=3) as xstream, \
                     tc.tile_pool(name="acts2", bufs=3) as acts, \
                     tc.tile_pool(name="res", bufs=3) as res, \
                     tc.tile_pool(name="tp_psum2", bufs=2, space="PSUM") as tp_psum, \
                     tc.tile_pool(name="mm_psum", bufs=2, space="PSUM") as mm_psum:
                    wpt = proj_w.tile([P, EC, E], BF16)
                    nc.gpsimd.dma_start(
                        out=wpt, in_=wp_d[:, :, :].rearrange("dc d e -> d dc e"))
                    for qc in range(NQ // P):
                        xo = xstream.tile([P, E], F32, tag="x")
                        nc.sync.dma_start(out=xo, in_=xp_d[ts(qc, P), :])
                        o1 = res.tile([P, E], F32, tag="o1")
                        for half in range(2):
                            po = mm_psum.tile([P, 512], F32, tag="mm")
                            for dc in range(EC):
                                nc.tensor.matmul(po, zT[:, dc, ts(qc, P)],
                                                 wpt[:, dc, ts(half, 512)],
                                                 start=(dc == 0), stop=(dc == EC - 1))
                            t1 = res.tile([P, 512], F32, tag="t1")
                            nc.vector.scalar_tensor_tensor(
                                out=t1, in0=po, scalar=0.0,
                                in1=xo[:, ts(half, 512)],
                                op0=ALU.bypass, op1=ALU.add)
                            nc.vector.tensor_tensor(
                                out=o1[:, ts(half, 512)], in0=t1,
                                in1=bp_sb[:, ts(half, 512)], op=ALU.add)
                        nc.sync.dma_start(out=out1_dram[ts(qc, P), :], in_=o1)
                        layernorm_to_T(tc, pools, o1, qc, ln2T, acts,
                                       tp_psum)

                # ---- phase 7: FFN mm1 + relu -----------------------------
                with tc.tile_pool(name="mm_psum2", bufs=2, space="PSUM") as mm_psum2:
                    for fc in range(FC):
                        pa = mm_psum2.tile([P, 1024], F32, tag="pa")
                        for qh in range(2):
                            for ec in range(EC):
                                nc.tensor.matmul(pa[:, ts(qh, 512)],
                                                 w1t[:, ec, ts(fc, P)],
                                                 ln2T[:, ec, ts(qh, 512)],
                                                 start=(ec == 0), stop=(ec == EC - 1))
                        nc.scalar.activation(out=a_sb[:, fc, :], in_=pa, func=AF.Relu,
                                             bias=b1_sb[:, fc:fc + 1], scale=1.0)

                    # ---- phase 8: FFN mm2 + residual2 + store ------------
                    with tc.tile_pool(name="res2", bufs=3) as res2:
                        for qc in range(NQ // P):
                            o1r = res2.tile([P, E], F32, tag="o1r")
                            nc.sync.dma_start(out=o1r, in_=out1_dram[ts(qc, P), :])
                            fin = res2.tile([P, E], F32, tag="fin")
                            for half in range(2):
                                pf = mm_psum2.tile([P, 512], F32, tag="pf")
                                for fc in range(FC):
                                    nc.tensor.matmul(pf, a_sb[:, fc, ts(qc, P)],
                                                     w2t[:, fc, ts(half, 512)],
                                                     start=(fc == 0),
                                                     stop=(fc == FC - 1))
                                t2 = res2.tile([P, 512], F32, tag="t2")
                                nc.vector.scalar_tensor_tensor(
                                    out=t2, in0=pf, scalar=0.0,
                                    in1=o1r[:, ts(half, 512)],
                                    op0=ALU.bypass, op1=ALU.add)
                                nc.vector.tensor_tensor(
                                    out=fin[:, ts(half, 512)], in0=t2,
                                    in1=b2_sb[:, ts(half, 512)], op=ALU.add)
                            nc.sync.dma_start(out=out_d[ts(qc, P), :], in_=fin)

    nc.compile()
    return nc


def _perms():
    a_own = np.concatenate([np.arange(0, 512), np.arange(1536, 2048)])
    a_rest = np.arange(512, 1536)
    b_own = np.arange(512, 1536)
    b_rest = np.concatenate([np.arange(0, 512), np.arange(1536, 2048)])
    return [np.concatenate([a_own, a_rest]), np.concatenate([b_own, b_rest])], \
           [a_own, b_own]


def _mask_bias():
    mb = [np.zeros(24, np.float32), np.zeros(24, np.float32)]
    mb[0][4:8] = NEG     # role A, tile0, chunks 8-11 (future keys)
    mb[1][20:24] = NEG   # role B, tile1, chunks 12-15 (future keys)
    return mb


def _prep_shared(wq, wk, wv, w_proj, b_proj, w1, b1, w2, b2,
                 ln1_g, ln1_b, ln2_g, ln2_b):
    """Pack weights for the kernel, folding LN gamma into the weights and
    LN beta into bias terms (K/Q biases applied at evacuation; the V bias
    commutes through softmax into b_proj; ln2_b folds into b1)."""
    bf = ml_dtypes.bfloat16
    f32 = np.float32
    HP_ = H // 2
    wq = np.asarray(wq, f32) * np.asarray(ln1_g, f32)[None, :, None]
    wk = np.asarray(wk, f32) * np.asarray(ln1_g, f32)[None, :, None]
    wv = np.asarray(wv, f32) * np.asarray(ln1_g, f32)[None, :, None]
    w1 = np.asarray(w1, f32) * np.asarray(ln2_g, f32)[:, None]

    qb = np.einsum("e,hed->hd", np.asarray(ln1_b, f32), wq) * (HD ** -0.5)
    kb = np.einsum("e,hed->hd", np.asarray(ln1_b, f32), wk)
    vb = np.einsum("e,hed->hd", np.asarray(ln1_b, f32), wv)
    bp_eff = np.asarray(b_proj, f32) + vb.reshape(-1) @ np.asarray(w_proj, f32)
    b1_eff = np.asarray(b1, f32) + np.asarray(ln2_b, f32) @ w1

    def pack_pair(w):  # [H, E, HD] -> [H/2, E/P, P, P] bf16
        wpair = w.reshape(H // 2, 2, E, HD)
        cat = np.concatenate([wpair[:, 0], wpair[:, 1]], axis=-1)  # [H/2, E, 128]
        return np.ascontiguousarray(cat.reshape(H // 2, E // P, P, P)).astype(bf)

    def pack_bias(b):  # [H, HD] -> [P, HP] (h0|h1 stacked per pair)
        return np.ascontiguousarray(b.reshape(HP_, 2 * HD).T).astype(f32)

    shared = {
        "wq2": pack_pair(wq),
        "wk2": pack_pair(wk),
        "wv": np.ascontiguousarray(
            wv.transpose(1, 0, 2).reshape(E // P, P, E)).astype(bf),
        "wp": np.ascontiguousarray(
            np.asarray(w_proj, f32).reshape(E // P, P, E)).astype(bf),
        "w1": np.ascontiguousarray(w1.reshape(E // P, P, FF)).astype(bf),
        "w2": np.ascontiguousarray(
            np.asarray(w2, f32).reshape(FF // P, P, E)).astype(bf),
        "b1t": np.ascontiguousarray(b1_eff.reshape(FF // P, P).T).astype(f32),
        "bproj": bp_eff.astype(f32),
        "b2": np.asarray(b2, f32),
        "kbt": pack_bias(kb),
        "qbt": pack_bias(qb),
    }
    return shared


def make_in_maps(x, **weights):
    """Build the 8 per-core input dicts (and the gather info)."""
    shared = _prep_shared(**weights)
    perms, owns = _perms()
    mbs = _mask_bias()
    in_maps = []
    for c in range(8):
        b, r = c // 2, c % 2
        m = dict(shared)
        m["xp"] = np.ascontiguousarray(x[b][perms[r]]).astype(np.float32)
        m["mb"] = mbs[r]
        in_maps.append(m)
    return in_maps, owns


def get_nc():
    if "nc" not in _CACHE:
        _CACHE["nc"] = _build_program()
    return _CACHE["nc"]


def kernel(x, wq, wk, wv, w_proj, b_proj, w1, b1, w2, b2,
           ln1_g, ln1_b, ln2_g, ln2_b):
    x = np.asarray(x, dtype=np.float32)
    weights = dict(wq=np.asarray(wq), wk=np.asarray(wk), wv=np.asarray(wv),
                   w_proj=np.asarray(w_proj), b_proj=np.asarray(b_proj),
                   w1=np.asarray(w1), b1=np.asarray(b1), w2=np.asarray(w2),
                   b2=np.asarray(b2), ln1_g=np.asarray(ln1_g),
                   ln1_b=np.asarray(ln1_b), ln2_g=np.asarray(ln2_g),
                   ln2_b=np.asarray(ln2_b))
    nc = get_nc()
    in_maps, owns = make_in_maps(x, **weights)
    res = run_bass_kernel_spmd(nc, in_maps, core_ids=list(range(8)))
    out = np.empty((4, S, E), dtype=np.float32)
    for c in range(8):
        b, r = c // 2, c % 2
        out[b][owns[r]] = res.results[c]["out"]
    return out



# revision 36
# speedup vs baseline: 1.1819x; 1.0132x over previous
"""Trainium2 Bass kernel for a dense pre-LN transformer block.

Block: y = x + proj(causal_mha(LN1(x))) ; out = y + FFN(LN2(y))
Shapes (hardcoded): x [4, 2048, 1024], H=16 heads, HD=64, FF=2048, fp32 I/O.

Sharding (8 cores, no collectives): core c handles batch b=c//2 and a
balanced half of the queries (role r=c%2; A: rows [0,512)+[1536,2048),
B: rows [512,1536)).  The key/value sequence is permuted on the host per
core (own rows first) so one SPMD program serves both roles; causality is
enforced by compile-time triangular affine_select masks on the diagonal
chunks plus a per-core exp-bias table (-100 => exp ~ 0) for the chunks
whose validity depends on the role.

Matmuls run in bf16 (fp32 PSUM accumulate); layernorm stats, softmax and
residuals stay fp32.  Scores are computed transposed (st[t,q]) so softmax
needs no transposes; V carries an extra ones-column so the softmax
denominator drops out of the z-matmul for free.
"""

import numpy as np
import ml_dtypes

import concourse.bass as bass
import concourse.bacc as bacc
import concourse.tile as tile
import concourse.mybir as mybir
from concourse.bass import ts
from concourse.bass_utils import run_bass_kernel_spmd
from concourse.masks import make_identity

BF16 = mybir.dt.bfloat16
F32 = mybir.dt.float32
AF = mybir.ActivationFunctionType
ALU = mybir.AluOpType

S = 2048          # sequence length
E = 1024          # embedding dim
H = 16            # heads
HD = 64           # head dim
FF = 2048         # ffn hidden
P = 128           # partitions
NQ = 1024         # queries owned per core
EPS = 1e-5
NEG = -100.0      # exp bias for masked-out chunks (exp(-100) ~ 0)

# chunk schedule (in permuted key coordinates), identical on every core:
# q-tile 0 (own positions [0,512)):   key chunks 0-3 (diag) + 8-11 (role-dep)
# q-tile 1 (own positions [512,1024)): key chunks 0-15 (4-7 diag, 12-15 role-dep)
SCHED = [[0, 1, 2, 3, 8, 9, 10, 11], list(range(16))]
DIAG = [set(range(0, 4)), set(range(4, 8))]

_CACHE = {}


def _build_program():
    nc = bacc.Bacc("TRN2", target_bir_lowering=False, debug=False)

    # ---- per-core dram inputs -------------------------------------------
    xp_d = nc.dram_tensor("xp", [S, E], F32, kind="ExternalInput")
    wq_d = nc.dram_tensor("wq2", [H // 2, E // P, P, P], BF16, kind="ExternalInput")
    wk_d = nc.dram_tensor("wk2", [H // 2, E // P, P, P], BF16, kind="ExternalInput")
    wv_d = nc.dram_tensor("wv", [E // P, P, E], BF16, kind="ExternalInput")
    wp_d = nc.dram_tensor("wp", [E // P, P, E], BF16, kind="ExternalInput")
    w1_d = nc.dram_tensor("w1", [E // P, P, FF], BF16, kind="ExternalInput")
    w2_d = nc.dram_tensor("w2", [FF // P, P, E], BF16, kind="ExternalInput")
    b1_d = nc.dram_tensor("b1t", [P, FF // P], F32, kind="ExternalInput")
    bp_d = nc.dram_tensor("bproj", [E], F32, kind="ExternalInput")
    b2_d = nc.dram_tensor("b2", [E], F32, kind="ExternalInput")
    kb_d = nc.dram_tensor("kbt", [P, H // 2], F32, kind="ExternalInput")
    qb_d = nc.dram_tensor("qbt", [P, H // 2], F32, kind="ExternalInput")
    mb_d = nc.dram_tensor("mb", [24], F32, kind="ExternalInput")
    out_d = nc.dram_tensor("out", [NQ, E], F32, kind="ExternalOutput")

    EC = E // P    # 8 e-chunks
    FC = FF // P   # 16 f-chunks
    NCH = S // P   # 16 key chunks
    HP = H // 2    # 8 head pairs

    def layernorm_to_T(tc, pools, x_ap, sc, dstT, act_pool, tp_psum):
        """LN of one [128, E] row-tile (fp32 in SBUF/psum-readable AP) then
        transpose to dstT[:, ec, sc*128:(sc+1)*128] (bf16).  gamma/beta are
        folded into the weights/biases on the host, so the evacuation is a
        plain copy."""
        nc_ = tc.nc
        small = pools["small"]
        stats = small.tile([P, 2, 6], F32, tag="bnstats")
        for g in range(2):
            nc_.vector.bn_stats(out=stats[:, g, :], in_=x_ap[:, g * 512:(g + 1) * 512])
        mv = small.tile([P, 2], F32, tag="bnaggr")
        nc_.vector.bn_aggr(out=mv, in_=stats)
        std = small.tile([P, 1], F32, tag="std")
        nc_.scalar.activation(out=std, in_=mv[:, 1:2], func=AF.Sqrt,
                              bias=pools["eps"], scale=1.0)
        rstd = small.tile([P, 1], F32, tag="rstd")
        nc_.vector.reciprocal(out=rstd, in_=std)
        nm = small.tile([P, 1], F32, tag="negmean")
        nc_.vector.scalar_tensor_tensor(out=nm, in0=mv[:, 0:1], scalar=-1.0,
                                        in1=rstd, op0=ALU.mult, op1=ALU.mult)
        tmp = act_pool.tile([P, E], BF16, tag="ln_tmp")
        nc_.scalar.activation(out=tmp, in_=x_ap, func=AF.Identity,
                              bias=nm, scale=rstd)
        for ec in range(EC):
            tp = tp_psum.tile([P, P], BF16, tag="tp")
            nc_.tensor.transpose(tp, tmp[:, ts(ec, P)], pools["ident"])
            nc_.vector.tensor_copy(out=dstT[:, ec, ts(sc, P)], in_=tp)

    with tile.TileContext(nc) as tc:
        import contextlib
        stk = contextlib.ExitStack()
        with stk:
            const = stk.enter_context(tc.tile_pool(name="const", bufs=1))
            small = stk.enter_context(tc.tile_pool(name="small", bufs=4))
            dram = stk.enter_context(tc.tile_pool(name="dram", bufs=1, space="DRAM"))

            ident = const.tile([P, P], BF16)
            make_identity(nc, ident)
            eps_t = const.tile([P, 1], F32)
            nc.vector.memset(eps_t, EPS)
            mb_sb = const.tile([P, 24], F32)
            nc.gpsimd.dma_start(out=mb_sb, in_=mb_d[None, :].to_broadcast((P, 24)))
            b1_sb = const.tile([P, FC], F32)
            nc.sync.dma_start(out=b1_sb, in_=b1_d[:, :])
            bp_sb = const.tile([P, E], F32)
            nc.gpsimd.dma_start(out=bp_sb, in_=bp_d[None, :].to_broadcast((P, E)))
            b2_sb = const.tile([P, E], F32)
            nc.gpsimd.dma_start(out=b2_sb, in_=b2_d[None, :].to_broadcast((P, E)))
            kb_sb = const.tile([P, HP], F32)
            nc.gpsimd.dma_start(out=kb_sb, in_=kb_d[:, :])
            qb_sb = const.tile([P, HP], F32)
            nc.gpsimd.dma_start(out=qb_sb, in_=qb_d[:, :])
            msk = const.tile([P, 4, 512], BF16)
            for k in range(4):
                nc.gpsimd.memset(msk[:, k, :], 1.0)
                nc.gpsimd.affine_select(
                    out=msk[:, k, :], in_=msk[:, k, :], compare_op=ALU.is_ge,
                    fill=0.0, base=-P * k, channel_multiplier=-1,
                    pattern=[[1, 512]])
            pools = {"ident": ident, "eps": eps_t, "small": small}

            out1_dram = dram.tile([NQ, E], F32)
            # V streamed through DRAM: [ch, t, h, 64] values + ones col 64
            V_dram = dram.tile([NCH, P, H, HD + 1], BF16)

            # late-phase persistent buffers (allocated first = bottom of stack)
            late = stk.enter_context(tc.tile_pool(name="late", bufs=1))
            ln2T = late.tile([P, EC, NQ], BF16)
            zT = late.tile([P, EC, NQ], BF16)

            with tc.tile_pool(name="attn", bufs=1) as attn:

                KT = [attn.tile([P, S], BF16, name=f"KT{i}") for i in range(HP)]
                QT = [attn.tile([P, NQ], BF16, name=f"QT{i}") for i in range(HP)]

                with tc.tile_pool(name="lnT_pool", bufs=1) as lnT_pool, \
                     tc.tile_pool(name="xstream", bufs=3) as xstream, \
                     tc.tile_pool(name="acts", bufs=3) as acts, \
                     tc.tile_pool(name="wstream", bufs=2) as wstream, \
                     tc.tile_pool(name="mm_psum", bufs=2, space="PSUM") as mm_psum:
                    lnT = lnT_pool.tile([P, EC, S], BF16)

                    # ---- phase 1: LN1 over all rows -> lnT [e, s] -------
                    with tc.tile_pool(name="tp_psum", bufs=2,
                                      space="PSUM") as tp_psum:
                        for sc in range(S // P):
                            xt = xstream.tile([P, E], F32, tag="x")
                            nc.sync.dma_start(out=xt, in_=xp_d[ts(sc, P), :])
                            layernorm_to_T(tc, pools, xt, sc, lnT, acts,
                                           tp_psum)

                    # ---- phase 3: V (all heads) -> DRAM -----------------
                    with tc.tile_pool(name="wv_pool", bufs=1) as wv_pool:
                        wvt = wv_pool.tile([P, EC, E], BF16)
                        nc.sync.dma_start(
                            out=wvt, in_=wv_d[:, :, :].rearrange("ec e n -> e ec n"))
                        for ch in range(NCH):
                            for half in range(2):
                                pv = mm_psum.tile([P, 512], F32, tag="mm")
                                for ec in range(EC):
                                    nc.tensor.matmul(pv, lnT[:, ec, ts(ch, P)],
                                                     wvt[:, ec, ts(half, 512)],
                                                     start=(ec == 0),
                                                     stop=(ec == EC - 1))
                                vsb = acts.tile([P, 8, HD + 1], BF16, tag="vsb")
                                nc.vector.memset(vsb[:, :, HD:HD + 1], 1.0)
                                nc.vector.tensor_copy(
                                    out=vsb[:, :, 0:HD],
                                    in_=pv.rearrange("p (h d) -> p h d", d=HD))
                                nc.sync.dma_start(
                                    out=V_dram[ch, :, 8 * half:8 * (half + 1), :],
                                    in_=vsb)

                    # ---- K/Q for one head pair (6 psum groups) ----------
                    def load_w(hp):
                        wkt = wstream.tile([P, EC, P], BF16, tag="wk")
                        nc.sync.dma_start(
                            out=wkt, in_=wk_d[hp].rearrange("ec e d -> e ec d"))
                        wqt = wstream.tile([P, EC, P], BF16, tag="wq")
                        nc.sync.dma_start(
                            out=wqt, in_=wq_d[hp].rearrange("ec e d -> e ec d"))
                        return wkt, wqt

                    def emit_kq_group(hp, wkt, wqt, kind, seg):
                        pk = mm_psum.tile([P, 512], F32, tag="mm", name="pk")
                        wt = wkt if kind == "k" else wqt
                        for ec in range(EC):
                            nc.tensor.matmul(pk, wt[:, ec], lnT[:, ec, ts(seg, 512)],
                                             start=(ec == 0), stop=(ec == EC - 1))
                        if kind == "k":
                            nc.vector.scalar_tensor_tensor(
                                out=KT[hp][:, ts(seg, 512)], in0=pk, scalar=1.0,
                                in1=kb_sb[:, hp:hp + 1].to_broadcast((P, 512)),
                                op0=ALU.mult, op1=ALU.add)
                        else:
                            nc.vector.scalar_tensor_tensor(
                                out=QT[hp][:, ts(seg, 512)], in0=pk,
                                scalar=float(HD) ** -0.5,
                                in1=qb_sb[:, hp:hp + 1].to_broadcast((P, 512)),
                                op0=ALU.mult, op1=ALU.add)

                    KQ_GROUPS = [("k", 0), ("k", 1), ("k", 2), ("k", 3),
                                 ("q", 0), ("q", 1)]
                    w0 = load_w(0)
                    for kind, seg in KQ_GROUPS:
                        emit_kq_group(0, w0[0], w0[1], kind, seg)

                    # ---- phase 4: attention, with hp+1's K/Q matmuls ----
                    # interleaved into the exp-latency gaps ----------------
                    with tc.tile_pool(name="st_psum", bufs=2,
                                      space="PSUM") as st_psum, \
                         tc.tile_pool(name="z_psum", bufs=2,
                                      space="PSUM") as z_psum, \
                         tc.tile_pool(name="p_pool", bufs=4) as p_pool, \
                         tc.tile_pool(name="v_pool", bufs=4) as v_pool, \
                         tc.tile_pool(name="l_pool", bufs=3) as l_pool:

                        for hp in range(HP):
                            kq_work = []
                            if hp + 1 < HP:
                                wn = load_w(hp + 1)
                                kq_work = list(KQ_GROUPS)
                            for j in range(2):
                                zp = [z_psum.tile([P, 512], F32, tag="z",
                                                  name=f"zp{h}") for h in range(2)]
                                sched = SCHED[j]
                                pend = None

                                def flush_z(pend):
                                    pt_, vt_, off_, first_, last_ = pend
                                    for h in range(2):
                                        nc.tensor.matmul(
                                            zp[h][0:HD + 1, off_:512],
                                            vt_[:, h],
                                            pt_[:, h, off_:512],
                                            start=first_, stop=last_)

                                for ci, ch in enumerate(sched):
                                    st = st_psum.tile([P, 1024], F32, tag="st")
                                    nc.tensor.matmul(
                                        st[:, 0:512], KT[hp][0:HD, ts(ch, P)],
                                        QT[hp][0:HD, ts(j, 512)],
                                        start=True, stop=True, tile_position=(0, 0))
                                    nc.tensor.matmul(
                                        st[:, 512:1024], KT[hp][HD:P, ts(ch, P)],
                                        QT[hp][HD:P, ts(j, 512)],
                                        start=True, stop=True, tile_position=(HD, 0))
                                    slot = (8 if j else 0) + ci
                                    pt = p_pool.tile([P, 2, 512], BF16, tag="p")
                                    nc.scalar.activation(
                                        out=pt.rearrange("p a b -> p (a b)"), in_=st,
                                        func=AF.Exp, bias=mb_sb[:, slot:slot + 1],
                                        scale=1.0)
                                    off = 0
                                    if ch in DIAG[j]:
                                        k = ch - 4 * j
                                        off = P * k
                                        nc.vector.tensor_tensor(
                                            out=pt[:, :, off:512],
                                            in0=pt[:, :, off:512],
                                            in1=msk[:, k:k + 1, off:512]
                                            .to_broadcast((P, 2, 512 - off)),
                                            op=ALU.mult)
                                    vt = v_pool.tile([P, 2, HD + 1], BF16,
                                                     tag="vt")
                                    nc.sync.dma_start(
                                        out=vt,
                                        in_=V_dram[ch, :, 2 * hp:2 * hp + 2, :])
                                    if kq_work and ci % 4 == 1:
                                        kind, seg = kq_work.pop(0)
                                        emit_kq_group(hp + 1, wn[0], wn[1],
                                                      kind, seg)
                                    if pend is not None:
                                        flush_z(pend)
                                    pend = (pt, vt, off, ci == 0,
                                            ci == len(sched) - 1)
                                flush_z(pend)

                                for h in range(2):
                                    dst_p = (h % 2) * HD
                                    zslice = zT[dst_p:dst_p + HD, hp, ts(j, 512)]
                                    lcp = l_pool.tile([1, 512], F32, tag="lcp")
                                    nc.vector.tensor_copy(
                                        out=lcp, in_=zp[h][HD:HD + 1, :])
                                    lr = l_pool.tile([P, 4], F32, tag="lr")
                                    nc.sync.dma_start(out=lr, in_=lcp)
                                    li = l_pool.tile([P, 4], F32, tag="li")
                                    nc.vector.reciprocal(out=li, in_=lr)
                                    lrow = l_pool.tile([1, 512], F32, tag="lrow")
                                    nc.sync.dma_start(out=lrow, in_=li)
                                    lb = l_pool.tile([P, 512], F32, tag="lb")
                                    nc.gpsimd.partition_broadcast(lb, lrow)
                                    nc.vector.tensor_tensor(
                                        out=zslice, in0=zp[h][0:HD, :],
                                        in1=lb[dst_p:dst_p + HD, :], op=ALU.mult)
                            for kind, seg in kq_work:   # safety flush
                                emit_kq_group(hp + 1, wn[0], wn[1], kind, seg)

            # ---- phases 5-8: proj/res1/LN2 then FFN (attn pools freed) ---
            with tc.tile_pool(name="ffnw", bufs=1) as ffnw, \
                 tc.tile_pool(name="a_pool", bufs=1) as a_pool:
                w1t = ffnw.tile([P, EC, FF], BF16)
                nc.sync.dma_start(out=w1t,
                                  in_=w1_d[:, :, :].rearrange("ec e f -> e ec f"))
                w2t = ffnw.tile([P, FC, E], BF16)
                nc.sync.dma_start(out=w2t,
                                  in_=w2_d[:, :, :].rearrange("fc f e -> f fc e"))
                a_sb = a_pool.tile([P, FC, NQ], BF16)

                with tc.tile_pool(name="proj_w", bufs=1) as proj_w, \
                     tc.tile_pool(name="xstream2", bufs=3) as xstream, \
                     tc.tile_pool(name="acts2", bufs=3) as acts, \
                     tc.tile_pool(name="res", bufs=3) as res, \
                     tc.tile_pool(name="tp_psum2", bufs=2, space="PSUM") as tp_psum, \
                     tc.tile_pool(name="mm_psum", bufs=2, space="PSUM") as mm_psum:
                    wpt = proj_w.tile([P, EC, E], BF16)
                    nc.sync.dma_start(
                        out=wpt, in_=wp_d[:, :, :].rearrange("dc d e -> d dc e"))
                    for qc in range(NQ // P):
                        xo = xstream.tile([P, E], F32, tag="x")
                        nc.sync.dma_start(out=xo, in_=xp_d[ts(qc, P), :])
                        o1 = res.tile([P, E], F32, tag="o1")
                        for half in range(2):
                            po = mm_psum.tile([P, 512], F32, tag="mm")
                            for dc in range(EC):
                                nc.tensor.matmul(po, zT[:, dc, ts(qc, P)],
                                                 wpt[:, dc, ts(half, 512)],
                                                 start=(dc == 0), stop=(dc == EC - 1))
                            t1 = res.tile([P, 512], F32, tag="t1")
                            nc.vector.scalar_tensor_tensor(
                                out=t1, in0=po, scalar=0.0,
                                in1=xo[:, ts(half, 512)],
                                op0=ALU.bypass, op1=ALU.add)
                            nc.vector.tensor_tensor(
                                out=o1[:, ts(half, 512)], in0=t1,
                                in1=bp_sb[:, ts(half, 512)], op=ALU.add)
                        nc.sync.dma_start(out=out1_dram[ts(qc, P), :], in_=o1)
                        layernorm_to_T(tc, pools, o1, qc, ln2T, acts,
                                       tp_psum)

                # ---- phase 7: FFN mm1 + relu -----------------------------
                with tc.tile_pool(name="mm_psum2", bufs=2, space="PSUM") as mm_psum2:
                    for fc in range(FC):
                        pa = mm_psum2.tile([P, 1024], F32, tag="pa")
                        for qh in range(2):
                            for ec in range(EC):
                                nc.tensor.matmul(pa[:, ts(qh, 512)],
                                                 w1t[:, ec, ts(fc, P)],
                                                 ln2T[:, ec, ts(qh, 512)],
                                                 start=(ec == 0), stop=(ec == EC - 1))
                        nc.scalar.activation(out=a_sb[:, fc, :], in_=pa, func=AF.Relu,
                                             bias=b1_sb[:, fc:fc + 1], scale=1.0)

                    # ---- phase 8: FFN mm2 + residual2 + store ------------
                    with tc.tile_pool(name="res2", bufs=3) as res2:
                        for qc in range(NQ // P):
                            o1r = res2.tile([P, E], F32, tag="o1r")
                            nc.sync.dma_start(out=o1r, in_=out1_dram[ts(qc, P), :])
                            fin = res2.tile([P, E], F32, tag="fin")
                            for half in range(2):
                                pf = mm_psum2.tile([P, 512], F32, tag="pf")
                                for fc in range(FC):
                                    nc.tensor.matmul(pf, a_sb[:, fc, ts(qc, P)],
                                                     w2t[:, fc, ts(half, 512)],
                                                     start=(fc == 0),
                                                     stop=(fc == FC - 1))
                                t2 = res2.tile([P, 512], F32, tag="t2")
                                nc.vector.scalar_tensor_tensor(
                                    out=t2, in0=pf, scalar=0.0,
                                    in1=o1r[:, ts(half, 512)],
                                    op0=ALU.bypass, op1=ALU.add)
                                nc.vector.tensor_tensor(
                                    out=fin[:, ts(half, 512)], in0=t2,
                                    in1=b2_sb[:, ts(half, 512)], op=ALU.add)
                            nc.sync.dma_start(out=out_d[ts(qc, P), :], in_=fin)

    nc.compile()
    return nc


def _perms():
    a_own = np.concatenate([np.arange(0, 512), np.arange(1536, 2048)])
    a_rest = np.arange(512, 1536)
    b_own = np.arange(512, 1536)
    b_rest = np.concatenate([np.arange(0, 512), np.arange(1536, 2048)])
    return [np.concatenate([a_own, a_rest]), np.concatenate([b_own, b_rest])], \
           [a_own, b_own]


def _mask_bias():
    mb = [np.zeros(24, np.float32), np.zeros(24, np.float32)]
    mb[0][4:8] = NEG     # role A, tile0, chunks 8-11 (future keys)
    mb[1][20:24] = NEG   # role B, tile1, chunks 12-15 (future keys)
    return mb


def _prep_shared(wq, wk, wv, w_proj, b_proj, w1, b1, w2, b2,
                 ln1_g, ln1_b, ln2_g, ln2_b):
    """Pack weights for the kernel, folding LN gamma into the weights and
    LN beta into bias terms (K/Q biases applied at evacuation; the V bias
    commutes through softmax into b_proj; ln2_b folds into b1)."""
    bf = ml_dtypes.bfloat16
    f32 = np.float32
    HP_ = H // 2
    wq = np.asarray(wq, f32) * np.asarray(ln1_g, f32)[None, :, None]
    wk = np.asarray(wk, f32) * np.asarray(ln1_g, f32)[None, :, None]
    wv = np.asarray(wv, f32) * np.asarray(ln1_g, f32)[None, :, None]
    w1 = np.asarray(w1, f32) * np.asarray(ln2_g, f32)[:, None]

    qb = np.einsum("e,hed->hd", np.asarray(ln1_b, f32), wq) * (HD ** -0.5)
    kb = np.einsum("e,hed->hd", np.asarray(ln1_b, f32), wk)
    vb = np.einsum("e,hed->hd", np.asarray(ln1_b, f32), wv)
    bp_eff = np.asarray(b_proj, f32) + vb.reshape(-1) @ np.asarray(w_proj, f32)
    b1_eff = np.asarray(b1, f32) + np.asarray(ln2_b, f32) @ w1

    def pack_pair(w):  # [H, E, HD] -> [H/2, E/P, P, P] bf16
        wpair = w.reshape(H // 2, 2, E, HD)
        cat = np.concatenate([wpair[:, 0], wpair[:, 1]], axis=-1)  # [H/2, E, 128]
        return np.ascontiguousarray(cat.reshape(H // 2, E // P, P, P)).astype(bf)

    def pack_bias(b):  # [H, HD] -> [P, HP] (h0|h1 stacked per pair)
        return np.ascontiguousarray(b.reshape(HP_, 2 * HD).T).astype(f32)

    shared = {
        "wq2": pack_pair(wq),
        "wk2": pack_pair(wk),
        "wv": np.ascontiguousarray(
            wv.transpose(1, 0, 2).reshape(E // P, P, E)).astype(bf),
        "wp": np.ascontiguousarray(
            np.asarray(w_proj, f32).reshape(E // P, P, E)).astype(bf),
        "w1": np.ascontiguousarray(w1.reshape(E // P, P, FF)).astype(bf),
        "w2": np.ascontiguousarray(
            np.asarray(w2, f32).reshape(FF // P, P, E)).astype(bf),
        "b1t": np.ascontiguousarray(b1_eff.reshape(FF // P, P).T).astype(f32),
        "bproj": bp_eff.astype(f32),
        "b2": np.asarray(b2, f32),
        "kbt": pack_bias(kb),
        "qbt": pack_bias(qb),
    }
    return shared


def make_in_maps(x, **weights):
    """Build the 8 per-core input dicts (and the gather info)."""
    shared = _prep_shared(**weights)
    perms, owns = _perms()
    mbs = _mask_bias()
    in_maps = []
    for c in range(8):
        b, r = c // 2, c % 2
        m = dict(shared)
        m["xp"] = np.ascontiguousarray(x[b][perms[r]]).astype(np.float32)
        m["mb"] = mbs[r]
        in_maps.append(m)
    return in_maps, owns


def get_nc():
    if "nc" not in _CACHE:
        _CACHE["nc"] = _build_program()
    return _CACHE["nc"]


def kernel(x, wq, wk, wv, w_proj, b_proj, w1, b1, w2, b2,
           ln1_g, ln1_b, ln2_g, ln2_b):
    x = np.asarray(x, dtype=np.float32)
    weights = dict(wq=np.asarray(wq), wk=np.asarray(wk), wv=np.asarray(wv),
                   w_proj=np.asarray(w_proj), b_proj=np.asarray(b_proj),
                   w1=np.asarray(w1), b1=np.asarray(b1), w2=np.asarray(w2),
                   b2=np.asarray(b2), ln1_g=np.asarray(ln1_g),
                   ln1_b=np.asarray(ln1_b), ln2_g=np.asarray(ln2_g),
                   ln2_b=np.asarray(ln2_b))
    nc = get_nc()
    in_maps, owns = make_in_maps(x, **weights)
    res = run_bass_kernel_spmd(nc, in_maps, core_ids=list(range(8)))
    out = np.empty((4, S, E), dtype=np.float32)
    for c in range(8):
        b, r = c // 2, c % 2
        out[b][owns[r]] = res.results[c]["out"]
    return out



# revision 39
# speedup vs baseline: 1.1836x; 1.0015x over previous
"""Trainium2 Bass kernel for a dense pre-LN transformer block.

Block: y = x + proj(causal_mha(LN1(x))) ; out = y + FFN(LN2(y))
Shapes (hardcoded): x [4, 2048, 1024], H=16 heads, HD=64, FF=2048, fp32 I/O.

Sharding (8 cores, no collectives): core c handles batch b=c//2 and a
balanced half of the queries (role r=c%2; A: rows [0,512)+[1536,2048),
B: rows [512,1536)).  The key/value sequence is permuted on the host per
core (own rows first) so one SPMD program serves both roles; causality is
enforced by compile-time triangular affine_select masks on the diagonal
chunks plus a per-core exp-bias table (-100 => exp ~ 0) for the chunks
whose validity depends on the role.

Matmuls run in bf16 (fp32 PSUM accumulate); layernorm stats, softmax and
residuals stay fp32.  Scores are computed transposed (st[t,q]) so softmax
needs no transposes; V carries an extra ones-column so the softmax
denominator drops out of the z-matmul for free.

Optimizations on top of the original schedule:
- LN gamma/beta folded into host-prepped weights and bias slots (K/Q
  biases at psum evacuation, the V bias through softmax into b_proj,
  ln2_b into b1), so LN transpose evacuation is a plain bf16 copy.
- Diagonal score chunks shrink the z-matmul moving width and the mask
  multiply to the causally-valid suffix (N = 512 - 128k).
- The softmax 1/l is fused into the z evacuation multiply.
"""

import numpy as np
import ml_dtypes

import concourse.bass as bass
import concourse.bacc as bacc
import concourse.tile as tile
import concourse.mybir as mybir
from concourse.bass import ts
from concourse.bass_utils import run_bass_kernel_spmd
from concourse.masks import make_identity

BF16 = mybir.dt.bfloat16
F32 = mybir.dt.float32
AF = mybir.ActivationFunctionType
ALU = mybir.AluOpType

S = 2048          # sequence length
E = 1024          # embedding dim
H = 16            # heads
HD = 64           # head dim
FF = 2048         # ffn hidden
P = 128           # partitions
NQ = 1024         # queries owned per core
EPS = 1e-5
NEG = -100.0      # exp bias for masked-out chunks (exp(-100) ~ 0)

# chunk schedule (in permuted key coordinates), identical on every core:
# q-tile 0 (own positions [0,512)):   key chunks 0-3 (diag) + 8-11 (role-dep)
# q-tile 1 (own positions [512,1024)): key chunks 0-15 (4-7 diag, 12-15 role-dep)
SCHED = [[0, 1, 2, 3, 8, 9, 10, 11], list(range(16))]
DIAG = [set(range(0, 4)), set(range(4, 8))]

_CACHE = {}


def _build_program():
    nc = bacc.Bacc("TRN2", target_bir_lowering=False, debug=False)

    # ---- per-core dram inputs -------------------------------------------
    xp_d = nc.dram_tensor("xp", [S, E], F32, kind="ExternalInput")
    wq_d = nc.dram_tensor("wq2", [H // 2, E // P, P, P], BF16, kind="ExternalInput")
    wk_d = nc.dram_tensor("wk2", [H // 2, E // P, P, P], BF16, kind="ExternalInput")
    wv_d = nc.dram_tensor("wv", [E // P, P, E], BF16, kind="ExternalInput")
    wp_d = nc.dram_tensor("wp", [E // P, P, E], BF16, kind="ExternalInput")
    w1_d = nc.dram_tensor("w1", [E // P, P, FF], BF16, kind="ExternalInput")
    w2_d = nc.dram_tensor("w2", [FF // P, P, E], BF16, kind="ExternalInput")
    b1_d = nc.dram_tensor("b1t", [P, FF // P], F32, kind="ExternalInput")
    bp_d = nc.dram_tensor("bproj", [E], F32, kind="ExternalInput")
    b2_d = nc.dram_tensor("b2", [E], F32, kind="ExternalInput")
    kb_d = nc.dram_tensor("kbt", [P, H // 2], F32, kind="ExternalInput")
    qb_d = nc.dram_tensor("qbt", [P, H // 2], F32, kind="ExternalInput")
    mb_d = nc.dram_tensor("mb", [24], F32, kind="ExternalInput")
    out_d = nc.dram_tensor("out", [NQ, E], F32, kind="ExternalOutput")

    EC = E // P    # 8 e-chunks
    FC = FF // P   # 16 f-chunks
    NCH = S // P   # 16 key chunks
    HP = H // 2    # 8 head pairs

    def layernorm_to_T(tc, pools, x_ap, sc, dstT, act_pool, tp_psum):
        """LN of one [128, E] row-tile (fp32 in SBUF/psum-readable AP) then
        transpose to dstT[:, ec, sc*128:(sc+1)*128] (bf16).  gamma/beta are
        folded into the weights/biases on the host, so the evacuation is a
        plain copy."""
        nc_ = tc.nc
        small = pools["small"]
        stats = small.tile([P, 2, 6], F32, tag="bnstats")
        for g in range(2):
            nc_.vector.bn_stats(out=stats[:, g, :], in_=x_ap[:, g * 512:(g + 1) * 512])
        mv = small.tile([P, 2], F32, tag="bnaggr")
        nc_.vector.bn_aggr(out=mv, in_=stats)
        std = small.tile([P, 1], F32, tag="std")
        nc_.scalar.activation(out=std, in_=mv[:, 1:2], func=AF.Sqrt,
                              bias=pools["eps"], scale=1.0)
        rstd = small.tile([P, 1], F32, tag="rstd")
        nc_.vector.reciprocal(out=rstd, in_=std)
        nm = small.tile([P, 1], F32, tag="negmean")
        nc_.vector.scalar_tensor_tensor(out=nm, in0=mv[:, 0:1], scalar=-1.0,
                                        in1=rstd, op0=ALU.mult, op1=ALU.mult)
        tmp = act_pool.tile([P, E], BF16, tag="ln_tmp")
        nc_.scalar.activation(out=tmp, in_=x_ap, func=AF.Identity,
                              bias=nm, scale=rstd)
        for ec in range(EC):
            tp = tp_psum.tile([P, P], BF16, tag="tp")
            nc_.tensor.transpose(tp, tmp[:, ts(ec, P)], pools["ident"])
            nc_.vector.tensor_copy(out=dstT[:, ec, ts(sc, P)], in_=tp)

    with tile.TileContext(nc) as tc:
        import contextlib
        stk = contextlib.ExitStack()
        with stk:
            const = stk.enter_context(tc.tile_pool(name="const", bufs=1))
            small = stk.enter_context(tc.tile_pool(name="small", bufs=4))
            dram = stk.enter_context(tc.tile_pool(name="dram", bufs=1, space="DRAM"))

            ident = const.tile([P, P], BF16)
            make_identity(nc, ident)
            eps_t = const.tile([P, 1], F32)
            nc.vector.memset(eps_t, EPS)
            mb_sb = const.tile([P, 24], F32)
            nc.gpsimd.dma_start(out=mb_sb, in_=mb_d[None, :].to_broadcast((P, 24)))
            b1_sb = const.tile([P, FC], F32)
            nc.sync.dma_start(out=b1_sb, in_=b1_d[:, :])
            bp_sb = const.tile([P, E], F32)
            nc.gpsimd.dma_start(out=bp_sb, in_=bp_d[None, :].to_broadcast((P, E)))
            b2_sb = const.tile([P, E], F32)
            nc.gpsimd.dma_start(out=b2_sb, in_=b2_d[None, :].to_broadcast((P, E)))
            kb_sb = const.tile([P, HP], F32)
            nc.gpsimd.dma_start(out=kb_sb, in_=kb_d[:, :])
            qb_sb = const.tile([P, HP], F32)
            nc.gpsimd.dma_start(out=qb_sb, in_=qb_d[:, :])
            msk = const.tile([P, 4, 512], BF16)
            for k in range(4):
                nc.gpsimd.memset(msk[:, k, :], 1.0)
                nc.gpsimd.affine_select(
                    out=msk[:, k, :], in_=msk[:, k, :], compare_op=ALU.is_ge,
                    fill=0.0, base=-P * k, channel_multiplier=-1,
                    pattern=[[1, 512]])
            pools = {"ident": ident, "eps": eps_t, "small": small}

            out1_dram = dram.tile([NQ, E], F32)
            # V streamed through DRAM: [ch, t, h, 64] values + ones col 64
            V_dram = dram.tile([NCH, P, H, HD + 1], BF16)

            # late-phase persistent buffers (allocated first = bottom of stack)
            late = stk.enter_context(tc.tile_pool(name="late", bufs=1))
            ln2T = late.tile([P, EC, NQ], BF16)
            zT = late.tile([P, EC, NQ], BF16)

            with tc.tile_pool(name="attn", bufs=1) as attn:

                KT = [attn.tile([P, S], BF16, name=f"KT{i}") for i in range(HP)]
                QT = [attn.tile([P, NQ], BF16, name=f"QT{i}") for i in range(HP)]

                with tc.tile_pool(name="lnT_pool", bufs=1) as lnT_pool, \
                     tc.tile_pool(name="xstream", bufs=3) as xstream, \
                     tc.tile_pool(name="acts", bufs=3) as acts, \
                     tc.tile_pool(name="wstream", bufs=2) as wstream, \
                     tc.tile_pool(name="mm_psum", bufs=2, space="PSUM") as mm_psum:
                    lnT = lnT_pool.tile([P, EC, S], BF16)

                    # ---- phase 1: LN1 over all rows -> lnT [e, s] -------
                    with tc.tile_pool(name="tp_psum", bufs=2,
                                      space="PSUM") as tp_psum:
                        for sc in range(S // P):
                            xt = xstream.tile([P, E], F32, tag="x")
                            nc.sync.dma_start(out=xt, in_=xp_d[ts(sc, P), :])
                            layernorm_to_T(tc, pools, xt, sc, lnT, acts,
                                           tp_psum)

                    # ---- phase 3: V (all heads) -> DRAM -----------------
                    with tc.tile_pool(name="wv_pool", bufs=1) as wv_pool:
                        wvt = wv_pool.tile([P, EC, E], BF16)
                        nc.sync.dma_start(
                            out=wvt, in_=wv_d[:, :, :].rearrange("ec e n -> e ec n"))
                        for ch in range(NCH):
                            for half in range(2):
                                pv = mm_psum.tile([P, 512], F32, tag="mm")
                                for ec in range(EC):
                                    nc.tensor.matmul(pv, lnT[:, ec, ts(ch, P)],
                                                     wvt[:, ec, ts(half, 512)],
                                                     start=(ec == 0),
                                                     stop=(ec == EC - 1))
                                vsb = acts.tile([P, 8, HD + 1], BF16, tag="vsb")
                                nc.vector.memset(vsb[:, :, HD:HD + 1], 1.0)
                                nc.vector.tensor_copy(
                                    out=vsb[:, :, 0:HD],
                                    in_=pv.rearrange("p (h d) -> p h d", d=HD))
                                nc.sync.dma_start(
                                    out=V_dram[ch, :, 8 * half:8 * (half + 1), :],
                                    in_=vsb)

                    # ---- K/Q for one head pair (6 psum groups) ----------
                    def load_w(hp):
                        wkt = wstream.tile([P, EC, P], BF16, tag="wk")
                        nc.sync.dma_start(
                            out=wkt, in_=wk_d[hp].rearrange("ec e d -> e ec d"))
                        wqt = wstream.tile([P, EC, P], BF16, tag="wq")
                        nc.sync.dma_start(
                            out=wqt, in_=wq_d[hp].rearrange("ec e d -> e ec d"))
                        return wkt, wqt

                    def emit_kq_group(hp, wkt, wqt, kind, seg):
                        pk = mm_psum.tile([P, 512], F32, tag="mm", name="pk")
                        wt = wkt if kind == "k" else wqt
                        for ec in range(EC):
                            nc.tensor.matmul(pk, wt[:, ec], lnT[:, ec, ts(seg, 512)],
                                             start=(ec == 0), stop=(ec == EC - 1))
                        if kind == "k":
                            nc.vector.scalar_tensor_tensor(
                                out=KT[hp][:, ts(seg, 512)], in0=pk, scalar=1.0,
                                in1=kb_sb[:, hp:hp + 1].to_broadcast((P, 512)),
                                op0=ALU.mult, op1=ALU.add)
                        else:
                            nc.vector.scalar_tensor_tensor(
                                out=QT[hp][:, ts(seg, 512)], in0=pk,
                                scalar=float(HD) ** -0.5,
                                in1=qb_sb[:, hp:hp + 1].to_broadcast((P, 512)),
                                op0=ALU.mult, op1=ALU.add)

                    KQ_GROUPS = [("k", 0), ("k", 1), ("k", 2), ("k", 3),
                                 ("q", 0), ("q", 1)]
                    w0 = load_w(0)
                    for kind, seg in KQ_GROUPS:
                        emit_kq_group(0, w0[0], w0[1], kind, seg)

                    # ---- phase 4: attention, with hp+1's K/Q matmuls ----
                    # interleaved into the exp-latency gaps ----------------
                    with tc.tile_pool(name="st_psum", bufs=2,
                                      space="PSUM") as st_psum, \
                         tc.tile_pool(name="z_psum", bufs=2,
                                      space="PSUM") as z_psum, \
                         tc.tile_pool(name="p_pool", bufs=4) as p_pool, \
                         tc.tile_pool(name="v_pool", bufs=4) as v_pool, \
                         tc.tile_pool(name="l_pool", bufs=3) as l_pool:

                        for hp in range(HP):
                            kq_work = []
                            if hp + 1 < HP:
                                wn = load_w(hp + 1)
                                kq_work = list(KQ_GROUPS)
                            for j in range(2):
                                zp = [z_psum.tile([P, 512], F32, tag="z",
                                                  name=f"zp{h}") for h in range(2)]
                                sched = SCHED[j]
                                pend = None

                                def flush_z(pend):
                                    pt_, vt_, off_, first_, last_ = pend
                                    for h in range(2):
                                        nc.tensor.matmul(
                                            zp[h][0:HD + 1, off_:512],
                                            vt_[:, h],
                                            pt_[:, h, off_:512],
                                            start=first_, stop=last_)

                                for ci, ch in enumerate(sched):
                                    st = st_psum.tile([P, 1024], F32, tag="st")
                                    nc.tensor.matmul(
                                        st[:, 0:512], KT[hp][0:HD, ts(ch, P)],
                                        QT[hp][0:HD, ts(j, 512)],
                                        start=True, stop=True, tile_position=(0, 0))
                                    nc.tensor.matmul(
                                        st[:, 512:1024], KT[hp][HD:P, ts(ch, P)],
                                        QT[hp][HD:P, ts(j, 512)],
                                        start=True, stop=True, tile_position=(HD, 0))
                                    slot = (8 if j else 0) + ci
                                    pt = p_pool.tile([P, 2, 512], BF16, tag="p")
                                    nc.scalar.activation(
                                        out=pt.rearrange("p a b -> p (a b)"), in_=st,
                                        func=AF.Exp, bias=mb_sb[:, slot:slot + 1],
                                        scale=1.0)
                                    off = 0
                                    if ch in DIAG[j]:
                                        k = ch - 4 * j
                                        off = P * k
                                        nc.vector.tensor_tensor(
                                            out=pt[:, :, off:512],
                                            in0=pt[:, :, off:512],
                                            in1=msk[:, k:k + 1, off:512]
                                            .to_broadcast((P, 2, 512 - off)),
                                            op=ALU.mult)
                                    vt = v_pool.tile([P, 2, HD + 1], BF16,
                                                     tag="vt")
                                    nc.sync.dma_start(
                                        out=vt,
                                        in_=V_dram[ch, :, 2 * hp:2 * hp + 2, :])
                                    if kq_work and ci % 4 == 1:
                                        kind, seg = kq_work.pop(0)
                                        emit_kq_group(hp + 1, wn[0], wn[1],
                                                      kind, seg)
                                    if pend is not None:
                                        flush_z(pend)
                                    pend = (pt, vt, off, ci == 0,
                                            ci == len(sched) - 1)
                                flush_z(pend)

                                for h in range(2):
                                    dst_p = (h % 2) * HD
                                    zslice = zT[dst_p:dst_p + HD, hp, ts(j, 512)]
                                    lcp = l_pool.tile([1, 512], F32, tag="lcp")
                                    nc.vector.tensor_copy(
                                        out=lcp, in_=zp[h][HD:HD + 1, :])
                                    lr = l_pool.tile([P, 4], F32, tag="lr")
                                    nc.sync.dma_start(out=lr, in_=lcp)
                                    li = l_pool.tile([P, 4], F32, tag="li")
                                    nc.vector.reciprocal(out=li, in_=lr)
                                    lrow = l_pool.tile([1, 512], F32, tag="lrow")
                                    nc.sync.dma_start(out=lrow, in_=li)
                                    lb = l_pool.tile([P, 512], F32, tag="lb")
                                    nc.gpsimd.partition_broadcast(lb, lrow)
                                    nc.vector.tensor_tensor(
                                        out=zslice, in0=zp[h][0:HD, :],
                                        in1=lb[dst_p:dst_p + HD, :], op=ALU.mult)
                            for kind, seg in kq_work:   # safety flush
                                emit_kq_group(hp + 1, wn[0], wn[1], kind, seg)

            # ---- phases 5-8: proj/res1/LN2 then FFN (attn pools freed) ---
            with tc.tile_pool(name="ffnw", bufs=1) as ffnw, \
                 tc.tile_pool(name="a_pool", bufs=1) as a_pool:
                w1t = ffnw.tile([P, EC, FF], BF16)
                nc.sync.dma_start(out=w1t,
                                  in_=w1_d[:, :, :].rearrange("ec e f -> e ec f"))
                w2t = ffnw.tile([P, FC, E], BF16)
                nc.sync.dma_start(out=w2t,
                                  in_=w2_d[:, :, :].rearrange("fc f e -> f fc e"))
                a_sb = a_pool.tile([P, FC, NQ], BF16)

                with tc.tile_pool(name="proj_w", bufs=1) as proj_w, \
                     tc.tile_pool(name="xstream2", bufs=3) as xstream, \
                     tc.tile_pool(name="acts2", bufs=3) as acts, \
                     tc.tile_pool(name="res", bufs=3) as res, \
                     tc.tile_pool(name="tp_psum2", bufs=2, space="PSUM") as tp_psum, \
                     tc.tile_pool(name="mm_psum", bufs=2, space="PSUM") as mm_psum:
                    wpt = proj_w.tile([P, EC, E], BF16)
                    nc.sync.dma_start(
                        out=wpt, in_=wp_d[:, :, :].rearrange("dc d e -> d dc e"))
                    for qc in range(NQ // P):
                        xo = xstream.tile([P, E], F32, tag="x")
                        nc.sync.dma_start(out=xo, in_=xp_d[ts(qc, P), :])
                        o1 = res.tile([P, E], F32, tag="o1")
                        for half in range(2):
                            po = mm_psum.tile([P, 512], F32, tag="mm")
                            for dc in range(EC):
                                nc.tensor.matmul(po, zT[:, dc, ts(qc, P)],
                                                 wpt[:, dc, ts(half, 512)],
                                                 start=(dc == 0), stop=(dc == EC - 1))
                            t1 = res.tile([P, 512], F32, tag="t1")
                            nc.vector.scalar_tensor_tensor(
                                out=t1, in0=po, scalar=0.0,
                                in1=xo[:, ts(half, 512)],
                                op0=ALU.bypass, op1=ALU.add)
                            nc.vector.tensor_tensor(
                                out=o1[:, ts(half, 512)], in0=t1,
                                in1=bp_sb[:, ts(half, 512)], op=ALU.add)
                        nc.sync.dma_start(out=out1_dram[ts(qc, P), :], in_=o1)
                        layernorm_to_T(tc, pools, o1, qc, ln2T, acts,
                                       tp_psum)

                # ---- phase 7: FFN mm1 + relu -----------------------------
                with tc.tile_pool(name="mm_psum2", bufs=2, space="PSUM") as mm_psum2:
                    for fc in range(FC):
                        pa = mm_psum2.tile([P, 1024], F32, tag="pa")
                        for qh in range(2):
                            for ec in range(EC):
                                nc.tensor.matmul(pa[:, ts(qh, 512)],
                                                 w1t[:, ec, ts(fc, P)],
                                                 ln2T[:, ec, ts(qh, 512)],
                                                 start=(ec == 0), stop=(ec == EC - 1))
                        nc.scalar.activation(out=a_sb[:, fc, :], in_=pa, func=AF.Relu,
                                             bias=b1_sb[:, fc:fc + 1], scale=1.0)

                    # ---- phase 8: FFN mm2 + residual2 + store ------------
                    with tc.tile_pool(name="res2", bufs=3) as res2:
                        for qc in range(NQ // P):
                            o1r = res2.tile([P, E], F32, tag="o1r")
                            nc.sync.dma_start(out=o1r, in_=out1_dram[ts(qc, P), :])
                            fin = res2.tile([P, E], F32, tag="fin")
                            for half in range(2):
                                pf = mm_psum2.tile([P, 512], F32, tag="pf")
                                for fc in range(FC):
                                    nc.tensor.matmul(pf, a_sb[:, fc, ts(qc, P)],
                                                     w2t[:, fc, ts(half, 512)],
                                                     start=(fc == 0),
                                                     stop=(fc == FC - 1))
                                t2 = res2.tile([P, 512], F32, tag="t2")
                                nc.vector.scalar_tensor_tensor(
                                    out=t2, in0=pf, scalar=0.0,
                                    in1=o1r[:, ts(half, 512)],
                                    op0=ALU.bypass, op1=ALU.add)
                                nc.vector.tensor_tensor(
                                    out=fin[:, ts(half, 512)], in0=t2,
                                    in1=b2_sb[:, ts(half, 512)], op=ALU.add)
                            nc.sync.dma_start(out=out_d[ts(qc, P), :], in_=fin)

    nc.compile()
    return nc


def _perms():
    a_own = np.concatenate([np.arange(0, 512), np.arange(1536, 2048)])
    a_rest = np.arange(512, 1536)
    b_own = np.arange(512, 1536)
    b_rest = np.concatenate([np.arange(0, 512), np.arange(1536, 2048)])
    return [np.concatenate([a_own, a_rest]), np.concatenate([b_own, b_rest])], \
           [a_own, b_own]


def _mask_bias():
    mb = [np.zeros(24, np.float32), np.zeros(24, np.float32)]
    mb[0][4:8] = NEG     # role A, tile0, chunks 8-11 (future keys)
    mb[1][20:24] = NEG   # role B, tile1, chunks 12-15 (future keys)
    return mb


def _prep_shared(wq, wk, wv, w_proj, b_proj, w1, b1, w2, b2,
                 ln1_g, ln1_b, ln2_g, ln2_b):
    """Pack weights for the kernel, folding LN gamma into the weights and
    LN beta into bias terms (K/Q biases applied at evacuation; the V bias
    commutes through softmax into b_proj; ln2_b folds into b1)."""
    bf = ml_dtypes.bfloat16
    f32 = np.float32
    HP_ = H // 2
    wq = np.asarray(wq, f32) * np.asarray(ln1_g, f32)[None, :, None]
    wk = np.asarray(wk, f32) * np.asarray(ln1_g, f32)[None, :, None]
    wv = np.asarray(wv, f32) * np.asarray(ln1_g, f32)[None, :, None]
    w1 = np.asarray(w1, f32) * np.asarray(ln2_g, f32)[:, None]

    qb = np.einsum("e,hed->hd", np.asarray(ln1_b, f32), wq) * (HD ** -0.5)
    kb = np.einsum("e,hed->hd", np.asarray(ln1_b, f32), wk)
    vb = np.einsum("e,hed->hd", np.asarray(ln1_b, f32), wv)
    bp_eff = np.asarray(b_proj, f32) + vb.reshape(-1) @ np.asarray(w_proj, f32)
    b1_eff = np.asarray(b1, f32) + np.asarray(ln2_b, f32) @ w1

    def pack_pair(w):  # [H, E, HD] -> [H/2, E/P, P, P] bf16
        wpair = w.reshape(H // 2, 2, E, HD)
        cat = np.concatenate([wpair[:, 0], wpair[:, 1]], axis=-1)  # [H/2, E, 128]
        return np.ascontiguousarray(cat.reshape(H // 2, E // P, P, P)).astype(bf)

    def pack_bias(b):  # [H, HD] -> [P, HP] (h0|h1 stacked per pair)
        return np.ascontiguousarray(b.reshape(HP_, 2 * HD).T).astype(f32)

    shared = {
        "wq2": pack_pair(wq),
        "wk2": pack_pair(wk),
        "wv": np.ascontiguousarray(
            wv.transpose(1, 0, 2).reshape(E // P, P, E)).astype(bf),
        "wp": np.ascontiguousarray(
            np.asarray(w_proj, f32).reshape(E // P, P, E)).astype(bf),
        "w1": np.ascontiguousarray(w1.reshape(E // P, P, FF)).astype(bf),
        "w2": np.ascontiguousarray(
            np.asarray(w2, f32).reshape(FF // P, P, E)).astype(bf),
        "b1t": np.ascontiguousarray(b1_eff.reshape(FF // P, P).T).astype(f32),
        "bproj": bp_eff.astype(f32),
        "b2": np.asarray(b2, f32),
        "kbt": pack_bias(kb),
        "qbt": pack_bias(qb),
    }
    return shared


def make_in_maps(x, **weights):
    """Build the 8 per-core input dicts (and the gather info)."""
    shared = _prep_shared(**weights)
    perms, owns = _perms()
    mbs = _mask_bias()
    in_maps = []
    for c in range(8):
        b, r = c // 2, c % 2
        m = dict(shared)
        m["xp"] = np.ascontiguousarray(x[b][perms[r]]).astype(np.float32)
        m["mb"] = mbs[r]
        in_maps.append(m)
    return in_maps, owns


def get_nc():
    if "nc" not in _CACHE:
        _CACHE["nc"] = _build_program()
    return _CACHE["nc"]


def kernel(x, wq, wk, wv, w_proj, b_proj, w1, b1, w2, b2,
           ln1_g, ln1_b, ln2_g, ln2_b):
    x = np.asarray(x, dtype=np.float32)
    weights = dict(wq=np.asarray(wq), wk=np.asarray(wk), wv=np.asarray(wv),
                   w_proj=np.asarray(w_proj), b_proj=np.asarray(b_proj),
                   w1=np.asarray(w1), b1=np.asarray(b1), w2=np.asarray(w2),
                   b2=np.asarray(b2), ln1_g=np.asarray(ln1_g),
                   ln1_b=np.asarray(ln1_b), ln2_g=np.asarray(ln2_g),
                   ln2_b=np.asarray(ln2_b))
    nc = get_nc()
    in_maps, owns = make_in_maps(x, **weights)
    res = run_bass_kernel_spmd(nc, in_maps, core_ids=list(range(8)))
    out = np.empty((4, S, E), dtype=np.float32)
    for c in range(8):
        b, r = c // 2, c % 2
        out[b][owns[r]] = res.results[c]["out"]
    return out



# revision 40
# speedup vs baseline: 1.2203x; 1.0310x over previous
"""Trainium2 Bass kernel for a dense pre-LN transformer block.

Block: y = x + proj(causal_mha(LN1(x))) ; out = y + FFN(LN2(y))
Shapes (hardcoded): x [4, 2048, 1024], H=16 heads, HD=64, FF=2048, fp32 I/O.

Sharding (8 cores, no collectives): core c handles batch b=c//2 and a
balanced half of the queries (role r=c%2; A: rows [0,512)+[1536,2048),
B: rows [512,1536)).  The key/value sequence is permuted on the host per
core (own rows first) so one SPMD program serves both roles; causality is
enforced by compile-time triangular affine_select masks on the diagonal
chunks plus a per-core exp-bias table (-100 => exp ~ 0) for the chunks
whose validity depends on the role.

Matmuls run in bf16 (fp32 PSUM accumulate); layernorm stats, softmax and
residuals stay fp32.  Scores are computed transposed (st[t,q]) so softmax
needs no transposes; V carries an extra ones-column so the softmax
denominator drops out of the z-matmul for free.

Optimizations on top of the original schedule:
- LN gamma/beta folded into host-prepped weights and bias slots (K/Q
  biases at psum evacuation, the V bias through softmax into b_proj,
  ln2_b into b1), so LN transpose evacuation is a plain bf16 copy.
- Diagonal score chunks shrink the z-matmul moving width and the mask
  multiply to the causally-valid suffix (N = 512 - 128k).
- The softmax 1/l is fused into the z evacuation multiply.
"""

import numpy as np
import ml_dtypes

import concourse.bass as bass
import concourse.bacc as bacc
import concourse.tile as tile
import concourse.mybir as mybir
from concourse.bass import ts
from concourse.bass_utils import run_bass_kernel_spmd
from concourse.masks import make_identity

BF16 = mybir.dt.bfloat16
F32 = mybir.dt.float32
AF = mybir.ActivationFunctionType
ALU = mybir.AluOpType

S = 2048          # sequence length
E = 1024          # embedding dim
H = 16            # heads
HD = 64           # head dim
FF = 2048         # ffn hidden
P = 128           # partitions
NQ = 1024         # queries owned per core
EPS = 1e-5
NEG = -100.0      # exp bias for masked-out chunks (exp(-100) ~ 0)

# chunk schedule (in permuted key coordinates), identical on every core:
# q-tile 0 (own positions [0,512)):   key chunks 0-3 (diag) + 8-11 (role-dep)
# q-tile 1 (own positions [512,1024)): key chunks 0-15 (4-7 diag, 12-15 role-dep)
SCHED = [[0, 1, 2, 3, 8, 9, 10, 11], list(range(16))]
DIAG = [set(range(0, 4)), set(range(4, 8))]

_CACHE = {}


def _build_program():
    nc = bacc.Bacc("TRN2", target_bir_lowering=False, debug=False)

    # ---- per-core dram inputs -------------------------------------------
    xp_d = nc.dram_tensor("xp", [S, E], F32, kind="ExternalInput")
    wq_d = nc.dram_tensor("wq2", [H // 2, E // P, P, P], BF16, kind="ExternalInput")
    wk_d = nc.dram_tensor("wk2", [H // 2, E // P, P, P], BF16, kind="ExternalInput")
    wv_d = nc.dram_tensor("wv", [E // P, P, E], BF16, kind="ExternalInput")
    wp_d = nc.dram_tensor("wp", [E // P, P, E], BF16, kind="ExternalInput")
    w1_d = nc.dram_tensor("w1", [E // P, P, FF], BF16, kind="ExternalInput")
    w2_d = nc.dram_tensor("w2", [FF // P, P, E], BF16, kind="ExternalInput")
    b1_d = nc.dram_tensor("b1t", [P, FF // P], F32, kind="ExternalInput")
    bp_d = nc.dram_tensor("bproj", [E], F32, kind="ExternalInput")
    b2_d = nc.dram_tensor("b2", [E], F32, kind="ExternalInput")
    kb_d = nc.dram_tensor("kbt", [P, H // 2], F32, kind="ExternalInput")
    qb_d = nc.dram_tensor("qbt", [P, H // 2], F32, kind="ExternalInput")
    mb_d = nc.dram_tensor("mb", [24], F32, kind="ExternalInput")
    out_d = nc.dram_tensor("out", [NQ, E], F32, kind="ExternalOutput")

    EC = E // P    # 8 e-chunks
    FC = FF // P   # 16 f-chunks
    NCH = S // P   # 16 key chunks
    HP = H // 2    # 8 head pairs

    def layernorm_to_T(tc, pools, x_ap, sc, dstT, act_pool, tp_psum):
        """LN of one [128, E] row-tile (fp32 in SBUF/psum-readable AP) then
        transpose to dstT[:, ec, sc*128:(sc+1)*128] (bf16).  gamma/beta are
        folded into the weights/biases on the host, so the evacuation is a
        plain copy."""
        nc_ = tc.nc
        small = pools["small"]
        stats = small.tile([P, 2, 6], F32, tag="bnstats")
        for g in range(2):
            nc_.vector.bn_stats(out=stats[:, g, :], in_=x_ap[:, g * 512:(g + 1) * 512])
        mv = small.tile([P, 2], F32, tag="bnaggr")
        nc_.vector.bn_aggr(out=mv, in_=stats)
        std = small.tile([P, 1], F32, tag="std")
        nc_.scalar.activation(out=std, in_=mv[:, 1:2], func=AF.Sqrt,
                              bias=pools["eps"], scale=1.0)
        rstd = small.tile([P, 1], F32, tag="rstd")
        nc_.vector.reciprocal(out=rstd, in_=std)
        nm = small.tile([P, 1], F32, tag="negmean")
        nc_.vector.scalar_tensor_tensor(out=nm, in0=mv[:, 0:1], scalar=-1.0,
                                        in1=rstd, op0=ALU.mult, op1=ALU.mult)
        tmp = act_pool.tile([P, E], BF16, tag="ln_tmp")
        nc_.scalar.activation(out=tmp, in_=x_ap, func=AF.Identity,
                              bias=nm, scale=rstd)
        for ec in range(EC):
            tp = tp_psum.tile([P, P], BF16, tag="tp")
            nc_.tensor.transpose(tp, tmp[:, ts(ec, P)], pools["ident"])
            nc_.vector.tensor_copy(out=dstT[:, ec, ts(sc, P)], in_=tp)

    with tile.TileContext(nc) as tc:
        import contextlib
        stk = contextlib.ExitStack()
        with stk:
            const = stk.enter_context(tc.tile_pool(name="const", bufs=1))
            small = stk.enter_context(tc.tile_pool(name="small", bufs=4))
            dram = stk.enter_context(tc.tile_pool(name="dram", bufs=1, space="DRAM"))

            ident = const.tile([P, P], BF16)
            make_identity(nc, ident)
            eps_t = const.tile([P, 1], F32)
            nc.vector.memset(eps_t, EPS)
            mb_sb = const.tile([P, 24], F32)
            nc.gpsimd.dma_start(out=mb_sb, in_=mb_d[None, :].to_broadcast((P, 24)))
            b1_sb = const.tile([P, FC], F32)
            nc.sync.dma_start(out=b1_sb, in_=b1_d[:, :])
            bp_sb = const.tile([P, E], F32)
            nc.gpsimd.dma_start(out=bp_sb, in_=bp_d[None, :].to_broadcast((P, E)))
            b2_sb = const.tile([P, E], F32)
            nc.gpsimd.dma_start(out=b2_sb, in_=b2_d[None, :].to_broadcast((P, E)))
            kb_sb = const.tile([P, HP], F32)
            nc.gpsimd.dma_start(out=kb_sb, in_=kb_d[:, :])
            qb_sb = const.tile([P, HP], F32)
            nc.gpsimd.dma_start(out=qb_sb, in_=qb_d[:, :])
            msk = const.tile([P, 4, 512], BF16)
            for k in range(4):
                nc.gpsimd.memset(msk[:, k, :], 1.0)
                nc.gpsimd.affine_select(
                    out=msk[:, k, :], in_=msk[:, k, :], compare_op=ALU.is_ge,
                    fill=0.0, base=-P * k, channel_multiplier=-1,
                    pattern=[[1, 512]])
            pools = {"ident": ident, "eps": eps_t, "small": small}

            out1_dram = dram.tile([NQ, E], F32)
            # V streamed through DRAM: [ch, t, h, 64] values + ones col 64
            V_dram = dram.tile([NCH, P, H, HD + 1], BF16)

            # late-phase persistent buffers (allocated first = bottom of stack)
            late = stk.enter_context(tc.tile_pool(name="late", bufs=1))
            ln2T = late.tile([P, EC, NQ], BF16)
            zT = late.tile([P, EC, NQ], BF16)
            wpt = late.tile([P, EC, E], BF16)

            with tc.tile_pool(name="attn", bufs=1) as attn:

                KT = [attn.tile([P, S], BF16, name=f"KT{i}") for i in range(HP)]
                QT = [attn.tile([P, NQ], BF16, name=f"QT{i}") for i in range(HP)]

                with tc.tile_pool(name="lnT_pool", bufs=1) as lnT_pool, \
                     tc.tile_pool(name="xstream", bufs=3) as xstream, \
                     tc.tile_pool(name="acts", bufs=3) as acts, \
                     tc.tile_pool(name="wstream", bufs=2) as wstream, \
                     tc.tile_pool(name="mm_psum", bufs=2, space="PSUM") as mm_psum:
                    lnT = lnT_pool.tile([P, EC, S], BF16)

                    # ---- phase 1: LN1 over all rows -> lnT [e, s] -------
                    with tc.tile_pool(name="tp_psum", bufs=2,
                                      space="PSUM") as tp_psum:
                        for sc in range(S // P):
                            xt = xstream.tile([P, E], F32, tag="x")
                            nc.sync.dma_start(out=xt, in_=xp_d[ts(sc, P), :])
                            layernorm_to_T(tc, pools, xt, sc, lnT, acts,
                                           tp_psum)

                    # ---- phase 3: V (all heads) -> DRAM -----------------
                    with tc.tile_pool(name="wv_pool", bufs=1) as wv_pool:
                        wvt = wv_pool.tile([P, EC, E], BF16)
                        nc.sync.dma_start(
                            out=wvt, in_=wv_d[:, :, :].rearrange("ec e n -> e ec n"))
                        for ch in range(NCH):
                            for half in range(2):
                                pv = mm_psum.tile([P, 512], F32, tag="mm")
                                for ec in range(EC):
                                    nc.tensor.matmul(pv, lnT[:, ec, ts(ch, P)],
                                                     wvt[:, ec, ts(half, 512)],
                                                     start=(ec == 0),
                                                     stop=(ec == EC - 1))
                                vsb = acts.tile([P, 8, HD + 1], BF16, tag="vsb")
                                nc.vector.memset(vsb[:, :, HD:HD + 1], 1.0)
                                nc.vector.tensor_copy(
                                    out=vsb[:, :, 0:HD],
                                    in_=pv.rearrange("p (h d) -> p h d", d=HD))
                                nc.sync.dma_start(
                                    out=V_dram[ch, :, 8 * half:8 * (half + 1), :],
                                    in_=vsb)

                    # ---- K/Q for one head pair (6 psum groups) ----------
                    def load_w(hp):
                        wkt = wstream.tile([P, EC, P], BF16, tag="wk")
                        nc.sync.dma_start(
                            out=wkt, in_=wk_d[hp].rearrange("ec e d -> e ec d"))
                        wqt = wstream.tile([P, EC, P], BF16, tag="wq")
                        nc.sync.dma_start(
                            out=wqt, in_=wq_d[hp].rearrange("ec e d -> e ec d"))
                        return wkt, wqt

                    def emit_kq_group(hp, wkt, wqt, kind, seg):
                        pk = mm_psum.tile([P, 512], F32, tag="mm", name="pk")
                        wt = wkt if kind == "k" else wqt
                        for ec in range(EC):
                            nc.tensor.matmul(pk, wt[:, ec], lnT[:, ec, ts(seg, 512)],
                                             start=(ec == 0), stop=(ec == EC - 1))
                        if kind == "k":
                            nc.vector.scalar_tensor_tensor(
                                out=KT[hp][:, ts(seg, 512)], in0=pk, scalar=1.0,
                                in1=kb_sb[:, hp:hp + 1].to_broadcast((P, 512)),
                                op0=ALU.mult, op1=ALU.add)
                        else:
                            nc.vector.scalar_tensor_tensor(
                                out=QT[hp][:, ts(seg, 512)], in0=pk,
                                scalar=float(HD) ** -0.5,
                                in1=qb_sb[:, hp:hp + 1].to_broadcast((P, 512)),
                                op0=ALU.mult, op1=ALU.add)

                    KQ_GROUPS = [("k", 0), ("k", 1), ("k", 2), ("k", 3),
                                 ("q", 0), ("q", 1)]
                    w0 = load_w(0)
                    for kind, seg in KQ_GROUPS:
                        emit_kq_group(0, w0[0], w0[1], kind, seg)
                    # prefetch the proj weight so phase 5 starts without a
                    # 2MB DMA in its critical path
                    nc.sync.dma_start(
                        out=wpt, in_=wp_d[:, :, :].rearrange("dc d e -> d dc e"))

                    # ---- phase 4: attention, with hp+1's K/Q matmuls ----
                    # interleaved into the exp-latency gaps ----------------
                    with tc.tile_pool(name="st_psum", bufs=2,
                                      space="PSUM") as st_psum, \
                         tc.tile_pool(name="z_psum", bufs=2,
                                      space="PSUM") as z_psum, \
                         tc.tile_pool(name="p_pool", bufs=4) as p_pool, \
                         tc.tile_pool(name="v_pool", bufs=6) as v_pool, \
                         tc.tile_pool(name="l_pool", bufs=3) as l_pool:

                        for hp in range(HP):
                            kq_work = []
                            if hp + 1 < HP:
                                wn = load_w(hp + 1)
                                kq_work = list(KQ_GROUPS)
                            for j in range(2):
                                zp = [z_psum.tile([P, 512], F32, tag="z",
                                                  name=f"zp{h}") for h in range(2)]
                                sched = SCHED[j]
                                pend = None

                                def flush_z(pend):
                                    pt_, vt_, off_, first_, last_ = pend
                                    for h in range(2):
                                        nc.tensor.matmul(
                                            zp[h][0:HD + 1, off_:512],
                                            vt_[:, h],
                                            pt_[:, h, off_:512],
                                            start=first_, stop=last_)

                                for ci, ch in enumerate(sched):
                                    st = st_psum.tile([P, 1024], F32, tag="st")
                                    nc.tensor.matmul(
                                        st[:, 0:512], KT[hp][0:HD, ts(ch, P)],
                                        QT[hp][0:HD, ts(j, 512)],
                                        start=True, stop=True, tile_position=(0, 0))
                                    nc.tensor.matmul(
                                        st[:, 512:1024], KT[hp][HD:P, ts(ch, P)],
                                        QT[hp][HD:P, ts(j, 512)],
                                        start=True, stop=True, tile_position=(HD, 0))
                                    slot = (8 if j else 0) + ci
                                    pt = p_pool.tile([P, 2, 512], BF16, tag="p")
                                    nc.scalar.activation(
                                        out=pt.rearrange("p a b -> p (a b)"), in_=st,
                                        func=AF.Exp, bias=mb_sb[:, slot:slot + 1],
                                        scale=1.0)
                                    off = 0
                                    if ch in DIAG[j]:
                                        k = ch - 4 * j
                                        off = P * k
                                        nc.vector.tensor_tensor(
                                            out=pt[:, :, off:512],
                                            in0=pt[:, :, off:512],
                                            in1=msk[:, k:k + 1, off:512]
                                            .to_broadcast((P, 2, 512 - off)),
                                            op=ALU.mult)
                                    vt = v_pool.tile([P, 2, HD + 1], BF16,
                                                     tag="vt")
                                    nc.sync.dma_start(
                                        out=vt,
                                        in_=V_dram[ch, :, 2 * hp:2 * hp + 2, :])
                                    if kq_work and ci % 4 == 1:
                                        kind, seg = kq_work.pop(0)
                                        emit_kq_group(hp + 1, wn[0], wn[1],
                                                      kind, seg)
                                    if pend is not None:
                                        flush_z(pend)
                                    pend = (pt, vt, off, ci == 0,
                                            ci == len(sched) - 1)
                                flush_z(pend)

                                for h in range(2):
                                    dst_p = (h % 2) * HD
                                    zslice = zT[dst_p:dst_p + HD, hp, ts(j, 512)]
                                    lcp = l_pool.tile([1, 512], F32, tag="lcp")
                                    nc.vector.tensor_copy(
                                        out=lcp, in_=zp[h][HD:HD + 1, :])
                                    lr = l_pool.tile([P, 4], F32, tag="lr")
                                    nc.sync.dma_start(out=lr, in_=lcp)
                                    li = l_pool.tile([P, 4], F32, tag="li")
                                    nc.vector.reciprocal(out=li, in_=lr)
                                    lrow = l_pool.tile([1, 512], F32, tag="lrow")
                                    nc.sync.dma_start(out=lrow, in_=li)
                                    lb = l_pool.tile([P, 512], F32, tag="lb")
                                    nc.gpsimd.partition_broadcast(lb, lrow)
                                    nc.vector.tensor_tensor(
                                        out=zslice, in0=zp[h][0:HD, :],
                                        in1=lb[dst_p:dst_p + HD, :], op=ALU.mult)
                            for kind, seg in kq_work:   # safety flush
                                emit_kq_group(hp + 1, wn[0], wn[1], kind, seg)

            # ---- phases 5-8: proj/res1/LN2 then FFN (attn pools freed) ---
            with tc.tile_pool(name="ffnw", bufs=1) as ffnw, \
                 tc.tile_pool(name="a_pool", bufs=1) as a_pool:
                w1t = ffnw.tile([P, EC, FF], BF16)
                nc.sync.dma_start(out=w1t,
                                  in_=w1_d[:, :, :].rearrange("ec e f -> e ec f"))
                w2t = ffnw.tile([P, FC, E], BF16)
                nc.sync.dma_start(out=w2t,
                                  in_=w2_d[:, :, :].rearrange("fc f e -> f fc e"))
                a_sb = a_pool.tile([P, FC, NQ], BF16)

                with tc.tile_pool(name="xstream2", bufs=3) as xstream, \
                     tc.tile_pool(name="acts2", bufs=3) as acts, \
                     tc.tile_pool(name="res", bufs=3) as res, \
                     tc.tile_pool(name="tp_psum2", bufs=2, space="PSUM") as tp_psum, \
                     tc.tile_pool(name="mm_psum", bufs=2, space="PSUM") as mm_psum:
                    for qc in range(NQ // P):
                        xo = xstream.tile([P, E], F32, tag="x")
                        nc.gpsimd.dma_start(out=xo, in_=xp_d[ts(qc, P), :])
                        o1 = res.tile([P, E], F32, tag="o1")
                        for half in range(2):
                            po = mm_psum.tile([P, 512], F32, tag="mm")
                            for dc in range(EC):
                                nc.tensor.matmul(po, zT[:, dc, ts(qc, P)],
                                                 wpt[:, dc, ts(half, 512)],
                                                 start=(dc == 0), stop=(dc == EC - 1))
                            t1 = res.tile([P, 512], F32, tag="t1")
                            nc.vector.scalar_tensor_tensor(
                                out=t1, in0=po, scalar=0.0,
                                in1=xo[:, ts(half, 512)],
                                op0=ALU.bypass, op1=ALU.add)
                            nc.vector.tensor_tensor(
                                out=o1[:, ts(half, 512)], in0=t1,
                                in1=bp_sb[:, ts(half, 512)], op=ALU.add)
                        nc.gpsimd.dma_start(out=out1_dram[ts(qc, P), :], in_=o1)
                        layernorm_to_T(tc, pools, o1, qc, ln2T, acts,
                                       tp_psum)

                # ---- phase 7: FFN mm1 + relu -----------------------------
                with tc.tile_pool(name="mm_psum2", bufs=2, space="PSUM") as mm_psum2:
                    for fc in range(FC):
                        pa = mm_psum2.tile([P, 1024], F32, tag="pa")
                        for qh in range(2):
                            for ec in range(EC):
                                nc.tensor.matmul(pa[:, ts(qh, 512)],
                                                 w1t[:, ec, ts(fc, P)],
                                                 ln2T[:, ec, ts(qh, 512)],
                                                 start=(ec == 0), stop=(ec == EC - 1))
                        nc.scalar.activation(out=a_sb[:, fc, :], in_=pa, func=AF.Relu,
                                             bias=b1_sb[:, fc:fc + 1], scale=1.0)

                    # ---- phase 8: FFN mm2 + residual2 + store ------------
                    with tc.tile_pool(name="res2", bufs=3) as res2:
                        for qc in range(NQ // P):
                            o1r = res2.tile([P, E], F32, tag="o1r")
                            nc.gpsimd.dma_start(out=o1r, in_=out1_dram[ts(qc, P), :])
                            fin = res2.tile([P, E], F32, tag="fin")
                            for half in range(2):
                                pf = mm_psum2.tile([P, 512], F32, tag="pf")
                                for fc in range(FC):
                                    nc.tensor.matmul(pf, a_sb[:, fc, ts(qc, P)],
                                                     w2t[:, fc, ts(half, 512)],
                                                     start=(fc == 0),
                                                     stop=(fc == FC - 1))
                                t2 = res2.tile([P, 512], F32, tag="t2")
                                nc.vector.scalar_tensor_tensor(
                                    out=t2, in0=pf, scalar=0.0,
                                    in1=o1r[:, ts(half, 512)],
                                    op0=ALU.bypass, op1=ALU.add)
                                nc.vector.tensor_tensor(
                                    out=fin[:, ts(half, 512)], in0=t2,
                                    in1=b2_sb[:, ts(half, 512)], op=ALU.add)
                            nc.sync.dma_start(out=out_d[ts(qc, P), :], in_=fin)

    nc.compile()
    return nc


def _perms():
    a_own = np.concatenate([np.arange(0, 512), np.arange(1536, 2048)])
    a_rest = np.arange(512, 1536)
    b_own = np.arange(512, 1536)
    b_rest = np.concatenate([np.arange(0, 512), np.arange(1536, 2048)])
    return [np.concatenate([a_own, a_rest]), np.concatenate([b_own, b_rest])], \
           [a_own, b_own]


def _mask_bias():
    mb = [np.zeros(24, np.float32), np.zeros(24, np.float32)]
    mb[0][4:8] = NEG     # role A, tile0, chunks 8-11 (future keys)
    mb[1][20:24] = NEG   # role B, tile1, chunks 12-15 (future keys)
    return mb


def _prep_shared(wq, wk, wv, w_proj, b_proj, w1, b1, w2, b2,
                 ln1_g, ln1_b, ln2_g, ln2_b):
    """Pack weights for the kernel, folding LN gamma into the weights and
    LN beta into bias terms (K/Q biases applied at evacuation; the V bias
    commutes through softmax into b_proj; ln2_b folds into b1)."""
    bf = ml_dtypes.bfloat16
    f32 = np.float32
    HP_ = H // 2
    wq = np.asarray(wq, f32) * np.asarray(ln1_g, f32)[None, :, None]
    wk = np.asarray(wk, f32) * np.asarray(ln1_g, f32)[None, :, None]
    wv = np.asarray(wv, f32) * np.asarray(ln1_g, f32)[None, :, None]
    w1 = np.asarray(w1, f32) * np.asarray(ln2_g, f32)[:, None]

    qb = np.einsum("e,hed->hd", np.asarray(ln1_b, f32), wq) * (HD ** -0.5)
    kb = np.einsum("e,hed->hd", np.asarray(ln1_b, f32), wk)
    vb = np.einsum("e,hed->hd", np.asarray(ln1_b, f32), wv)
    bp_eff = np.asarray(b_proj, f32) + vb.reshape(-1) @ np.asarray(w_proj, f32)
    b1_eff = np.asarray(b1, f32) + np.asarray(ln2_b, f32) @ w1

    def pack_pair(w):  # [H, E, HD] -> [H/2, E/P, P, P] bf16
        wpair = w.reshape(H // 2, 2, E, HD)
        cat = np.concatenate([wpair[:, 0], wpair[:, 1]], axis=-1)  # [H/2, E, 128]
        return np.ascontiguousarray(cat.reshape(H // 2, E // P, P, P)).astype(bf)

    def pack_bias(b):  # [H, HD] -> [P, HP] (h0|h1 stacked per pair)
        return np.ascontiguousarray(b.reshape(HP_, 2 * HD).T).astype(f32)

    shared = {
        "wq2": pack_pair(wq),
        "wk2": pack_pair(wk),
        "wv": np.ascontiguousarray(
            wv.transpose(1, 0, 2).reshape(E // P, P, E)).astype(bf),
        "wp": np.ascontiguousarray(
            np.asarray(w_proj, f32).reshape(E // P, P, E)).astype(bf),
        "w1": np.ascontiguousarray(w1.reshape(E // P, P, FF)).astype(bf),
        "w2": np.ascontiguousarray(
            np.asarray(w2, f32).reshape(FF // P, P, E)).astype(bf),
        "b1t": np.ascontiguousarray(b1_eff.reshape(FF // P, P).T).astype(f32),
        "bproj": bp_eff.astype(f32),
        "b2": np.asarray(b2, f32),
        "kbt": pack_bias(kb),
        "qbt": pack_bias(qb),
    }
    return shared


def make_in_maps(x, **weights):
    """Build the 8 per-core input dicts (and the gather info)."""
    shared = _prep_shared(**weights)
    perms, owns = _perms()
    mbs = _mask_bias()
    in_maps = []
    for c in range(8):
        b, r = c // 2, c % 2
        m = dict(shared)
        m["xp"] = np.ascontiguousarray(x[b][perms[r]]).astype(np.float32)
        m["mb"] = mbs[r]
        in_maps.append(m)
    return in_maps, owns


def get_nc():
    if "nc" not in _CACHE:
        _CACHE["nc"] = _build_program()
    return _CACHE["nc"]


def kernel(x, wq, wk, wv, w_proj, b_proj, w1, b1, w2, b2,
           ln1_g, ln1_b, ln2_g, ln2_b):
    x = np.asarray(x, dtype=np.float32)
    weights = dict(wq=np.asarray(wq), wk=np.asarray(wk), wv=np.asarray(wv),
                   w_proj=np.asarray(w_proj), b_proj=np.asarray(b_proj),
                   w1=np.asarray(w1), b1=np.asarray(b1), w2=np.asarray(w2),
                   b2=np.asarray(b2), ln1_g=np.asarray(ln1_g),
                   ln1_b=np.asarray(ln1_b), ln2_g=np.asarray(ln2_g),
                   ln2_b=np.asarray(ln2_b))
    nc = get_nc()
    in_maps, owns = make_in_maps(x, **weights)
    res = run_bass_kernel_spmd(nc, in_maps, core_ids=list(range(8)))
    out = np.empty((4, S, E), dtype=np.float32)
    for c in range(8):
        b, r = c // 2, c % 2
        out[b][owns[r]] = res.results[c]["out"]
    return out

